# revision 1
# baseline (speedup 1.0000x reference)
"""Trainium2 Bass kernel for nn_Attention_66546223284383.

Strategy: pure data-parallel over batch B=16 -> 2 batches per core x 8 cores.
Per core, per batch:
  qkvT = (BN-folded W)^T @ x^T           (h on partitions, n free)
  per head: scoresT = k^T.T @ qT         (m on partitions, n free; attn scale
            folded into q weights), exp on ACT (scores are small: no max-sub),
            oT = [v|1]^T @ expT          (65 rows: 64 o-dims + denominator),
            PE-transpose -> normalize -> stage o to DRAM (n, c) bf16.
  conv branch: v reflowed via DRAM to (channel, spatial), hardswish + 9-tap
            depthwise conv on VectorE with per-partition tap weights.
  proj:     O2^T tiles read back via XBAR transpose DMA (handles the torch
            "raw reshape" (B,N,H,d)->(B,DH,N) as a flat re-chunk),
            xo = O2 @ proj_w^T + vc  (r on partitions, j free)
  out:      out = xo^T @ out_w^T + out_b -> (1024, 512) fp32.
All matmuls bf16 inputs with fp32 PSUM accumulation.
"""
import sys
import numpy as np

sys.path.insert(0, "/opt/trn_rl_repo")

import ml_dtypes  # noqa: E402

BF16 = ml_dtypes.bfloat16

KD, H, D, DH, DIM, IMG, S, N, B = 32, 8, 64, 512, 512, 1024, 32, 1044, 16
EPS = 1e-5
NCORES = 8
BPC = B // NCORES  # batches per core
NT = [(i * 128, 128) for i in range(8)] + [(1024, 20)]   # 1044 partition tiles
NCH = [(0, 512), (512, 512), (1024, 20)]                 # 1044 free chunks

_cached = {}


def _build():
    from concourse import bacc, tile
    import concourse.bass as bass
    import concourse.mybir as mybir
    from concourse.masks import make_identity

    dt = mybir.dt
    alu = mybir.AluOpType
    act_exp = mybir.ActivationFunctionType.Exp

    nc = bacc.Bacc(None, target_bir_lowering=False, debug=False)

    xs = nc.declare_dram_parameter("xs", [BPC, N, DIM], dt.bfloat16, isOutput=False)
    # wqkv cols: 0:768 = 6 padded qk tiles (3 q tiles then 3 k tiles, 3 heads
    # per tile at offsets 0/32/64); 768:1288 = v channels hh*65+j (j==64 is a
    # ones channel: zero weights, bias 1 -> softmax denominator column)
    wqkv = nc.declare_dram_parameter("wqkv", [DIM, 1288], dt.bfloat16, isOutput=False)
    bqkv = nc.declare_dram_parameter("bqkv", [128, 6], dt.float32, isOutput=False)
    bv = nc.declare_dram_parameter("bv", [1, 520], dt.float32, isOutput=False)
    wproj = nc.declare_dram_parameter("wproj", [N, IMG], dt.bfloat16, isOutput=False)
    wout = nc.declare_dram_parameter("wout", [DH, DIM], dt.bfloat16, isOutput=False)
    tapw = nc.declare_dram_parameter("tapw", [128, 36], dt.float32, isOutput=False)
    cbp = nc.declare_dram_parameter("cbp", [128, 4], dt.float32, isOutput=False)
    pbp = nc.declare_dram_parameter("pbp", [1, IMG], dt.float32, isOutput=False)
    obp = nc.declare_dram_parameter("obp", [1, DIM], dt.float32, isOutput=False)
    out_ext = nc.declare_dram_parameter("out", [BPC, IMG, DIM], dt.float32, isOutput=True)

    o_nat = nc.dram_tensor("o_nat", [BPC, N, DH], dt.bfloat16)
    v_dram = nc.dram_tensor("v_dram", [BPC, H, N, D], dt.bfloat16)

    with tile.TileContext(nc) as tc:
        with (
            tc.tile_pool(name="w", bufs=1) as pw,
            tc.tile_pool(name="xT", bufs=8) as pxT,
            tc.tile_pool(name="qkvT", bufs=12) as pqk,
            tc.tile_pool(name="vnat", bufs=18) as pvn,
            tc.tile_pool(name="exp", bufs=3) as pexp,
            tc.tile_pool(name="oTs", bufs=2) as poTs,
            tc.tile_pool(name="small", bufs=4) as psm,
            tc.tile_pool(name="conv", bufs=2) as pcv,
            tc.tile_pool(name="cin", bufs=3) as pcin,
            tc.tile_pool(name="vc", bufs=8) as pvc,
            tc.tile_pool(name="o2t", bufs=1) as po2,
            tc.tile_pool(name="xo", bufs=5) as pxo,
            # PSUM budget (8 banks): sc 2x(128,1024)=4, ot 3x(65,512)=3, mm 1
            tc.tile_pool(name="pssc", bufs=2, space=bass.MemorySpace.PSUM) as pssc,
            tc.tile_pool(name="psmm", bufs=1, space=bass.MemorySpace.PSUM) as psmm,
            tc.tile_pool(name="psot", bufs=3, space=bass.MemorySpace.PSUM) as psot,
        ):
            # ---- constants / weights ----
            id_sb = pw.tile([128, 128], dt.bfloat16, tag="id")
            make_identity(nc, id_sb[:])
            wqkv_sb = pw.tile([128, 4, 1288], dt.bfloat16, tag="wqkv")
            nc.sync.dma_start(wqkv_sb[:], wqkv[:].rearrange("(k p) h -> p k h", p=128))
            bqkv_sb = pw.tile([128, 6], dt.float32, tag="bqkv")
            nc.sync.dma_start(bqkv_sb[:], bqkv[:])
            bv_sb = pw.tile([1, 520], dt.float32, tag="bv")
            nc.sync.dma_start(bv_sb[:], bv[:])
            bvbc = pw.tile([128, 520], dt.float32, tag="bvbc")
            nc.gpsimd.partition_broadcast(bvbc[:], bv_sb[:])
            wproj_sb = []
            for mt, (m0, msz) in enumerate(NT):
                t = pw.tile([msz, 1024], dt.bfloat16, tag=f"wproj{mt}")
                nc.sync.dma_start(t[:], wproj[m0:m0 + msz, :])
                wproj_sb.append(t)
            wout_sb = pw.tile([128, 4, DIM], dt.bfloat16, tag="wout")
            nc.sync.dma_start(wout_sb[:], wout[:].rearrange("(k p) c -> p k c", p=128))
            tapw_sb = pw.tile([128, 36], dt.float32, tag="tapw")
            nc.sync.dma_start(tapw_sb[:], tapw[:])
            cb_sb = pw.tile([128, 4], dt.float32, tag="cb")
            nc.sync.dma_start(cb_sb[:], cbp[:])
            pb_sb = pw.tile([1, IMG], dt.float32, tag="pb")
            nc.sync.dma_start(pb_sb[:], pbp[:])
            ob_sb = pw.tile([1, DIM], dt.float32, tag="ob")
            nc.sync.dma_start(ob_sb[:], obp[:])
            pbbc = pw.tile([128, IMG], dt.float32, tag="pbbc")
            nc.gpsimd.partition_broadcast(pbbc[:], pb_sb[:])
            obbc = pw.tile([128, DIM], dt.float32, tag="obbc")
            nc.gpsimd.partition_broadcast(obbc[:], ob_sb[:])

            qkvT_all, vnat_all, vc_all = {}, {}, {}

            # ======== phase 1: x^T + qk + v for BOTH batches (PE-dense) ========
            for b in range(BPC):
                xT = []
                for cb4 in range(4):
                    t = pxT.tile([128, N], dt.bfloat16, tag="xT")
                    c0 = cb4 * 128
                    nc.sync.dma_start_transpose(t[:, 0:1040], xs[b, 0:1040, c0:c0 + 128])
                    nc.sync.dma_start(
                        t[:, 1040:N], xs[b, 1040:N, c0:c0 + 128].rearrange("a b -> b a")
                    )
                    xT.append(t)

                # 6 padded qk tiles: 0-2 q heads (3/tile @ 0,32,64), 3-5 k heads
                qkvT = []
                for mt6 in range(6):
                    t = pqk.tile([128, N], dt.bfloat16, tag="qkvT")
                    pss = [
                        pssc.tile([128, 512], dt.float32, tag="sc", name=f"qk{c}")
                        for c in range(2)
                    ] + [psmm.tile([128, 20], dt.float32, tag="mm", name="qk2")]
                    for kc in range(4):
                        for c, (ci, cw) in enumerate(NCH):
                            nc.tensor.matmul(
                                pss[c][:, 0:cw] if c < 2 else pss[c][:],
                                wqkv_sb[:, kc, mt6 * 128:(mt6 + 1) * 128],
                                xT[kc][:, ci:ci + cw],
                                start=(kc == 0), stop=(kc == 3),
                            )
                    for c, (ci, cw) in enumerate(NCH):
                        nc.vector.tensor_scalar(
                            t[:, ci:ci + cw],
                            pss[c][:, 0:cw] if c < 2 else pss[c][:],
                            bqkv_sb[:, mt6:mt6 + 1], None, op0=alu.add,
                        )
                    qkvT.append(t)
                qkvT_all[b] = qkvT

                # v directly in (n, 8*65) layout with ones channels
                vnat = []
                for nt, (n0, nsz) in enumerate(NT):
                    t = pvn.tile([nsz, 520], dt.bfloat16, tag="vnat")
                    pss = [
                        pssc.tile([nsz, 512], dt.float32, tag="sc", name=f"vn{c}")
                        for c in range(2)
                    ]
                    for kc in range(4):
                        for c in range(2):
                            nc.tensor.matmul(
                                pss[c][:, 0:260],
                                xT[kc][:, n0:n0 + nsz],
                                wqkv_sb[:, kc, 768 + c * 260:768 + (c + 1) * 260],
                                start=(kc == 0), stop=(kc == 3),
                            )
                    for c in range(2):
                        nc.vector.tensor_tensor(
                            t[:, c * 260:(c + 1) * 260], pss[c][:, 0:260],
                            bvbc[0:nsz, c * 260:(c + 1) * 260], op=alu.add,
                        )
                    # stage all 8 heads' v in ONE DMA on the gpsimd queue
                    # (keeps the serial sync queue short)
                    nc.gpsimd.dma_start(
                        v_dram[b].rearrange("h n d -> n h d")[n0:n0 + nsz],
                        t[:].rearrange("p (h dd) -> p h dd", h=8)[:, :, 0:D],
                    )
                    vnat.append(t)
                vnat_all[b] = vnat

            # ======== phase 2: all 16 heads' attention (+conv on DVE) ========
            for b in range(BPC):
                qkvT, vnat = qkvT_all[b], vnat_all[b]
                vc_tiles = []
                for hh in range(H):
                    # q/k at base partitions {0,32,64} in padded tiles
                    qo = (hh % 3) * KD
                    qT = qkvT[hh // 3][qo:qo + KD, :]
                    kT = qkvT[3 + hh // 3][qo:qo + KD, :]

                    oT_ps = [
                        psot.tile([D + 1, cw], dt.float32, tag="ot", name=f"ot{k}")
                        for k, (_, cw) in enumerate(NCH)
                    ]
                    for mt, (m0, msz) in enumerate(NT):
                        et = pexp.tile([128, N], dt.bfloat16, tag="exp")
                        # scores: one 2-bank psum tile (chunks bank-aligned),
                        # tail rides the mm slot; ONE exp per region
                        sc = pssc.tile([msz, 1024], dt.float32, tag="sc")
                        sct = psmm.tile([msz, 20], dt.float32, tag="mm")
                        for (ci, cw) in ((0, 512), (512, 512)):
                            nc.tensor.matmul(
                                sc[:, ci:ci + cw], kT[:, m0:m0 + msz],
                                qT[:, ci:ci + cw], start=True, stop=True,
                            )
                        nc.tensor.matmul(
                            sct[:], kT[:, m0:m0 + msz], qT[:, 1024:N],
                            start=True, stop=True,
                        )
                        nc.scalar.activation(et[0:msz, 0:1024], sc[:], act_exp)
                        nc.scalar.activation(et[0:msz, 1024:N], sct[:], act_exp)
                        for k, (ci, cw) in enumerate(NCH):
                            nc.tensor.matmul(
                                oT_ps[k][:],
                                vnat[mt][:, hh * 65:hh * 65 + D + 1],
                                et[0:msz, ci:ci + cw],
                                start=(mt == 0), stop=(mt == 8),
                            )
                    oT_sb = poTs.tile([D + 1, N], dt.bfloat16, tag="oTs")
                    for k, (ci, cw) in enumerate(NCH):
                        nc.vector.tensor_copy(oT_sb[:, ci:ci + cw], oT_ps[k][:])
                    # transpose back, normalize, stage o
                    for nt, (n0, nsz) in enumerate(NT):
                        tp2 = psmm.tile([nsz, D + 1], dt.bfloat16, tag="mm")
                        nc.tensor.transpose(
                            tp2[:], oT_sb[:, n0:n0 + nsz], id_sb[0:D + 1, 0:D + 1]
                        )
                        rcp = psm.tile([nsz, 1], dt.float32, tag="rcp")
                        nc.vector.reciprocal(rcp[:], tp2[:, D:D + 1])
                        onrm = psm.tile([nsz, D], dt.bfloat16, tag="onrm")
                        nc.vector.tensor_scalar(
                            onrm[:], tp2[:, 0:D], rcp[:], None, op0=alu.mult
                        )
                        nc.gpsimd.dma_start(
                            o_nat[b, n0:n0 + nsz, hh * D:(hh + 1) * D], onrm[:]
                        )

                    # conv per channel-tile once its two heads are staged
                    # (bf16 DVE ops for the 2x mode)
                    if hh % 2 == 1:
                        ct = hh // 2
                        cin = pcin.tile([128, 1024], dt.bfloat16, tag="cin")
                        for hl in range(2):
                            src = v_dram[b, 2 * ct + hl, 0:1024, :].rearrange(
                                "(c n2) d -> c (n2 d)", c=64
                            )
                            nc.scalar.dma_start(cin[hl * 64:(hl + 1) * 64, :], src)
                        u = pcv.tile([128, 1024], dt.bfloat16, tag="u")
                        nc.vector.tensor_scalar(
                            u[:], cin[:], 1.0 / 6.0, 0.5, op0=alu.mult, op1=alu.add
                        )
                        nc.vector.tensor_scalar(
                            u[:], u[:], 1.0, 0.0, op0=alu.min, op1=alu.max
                        )
                        hs = pcv.tile([128, 32, 32], dt.bfloat16, tag="hs")
                        nc.vector.tensor_tensor(
                            hs[:].rearrange("p a b -> p (a b)"), u[:], cin[:],
                            op=alu.mult,
                        )
                        pad = pcv.tile([128, 34, 34], dt.bfloat16, tag="pad")
                        nc.vector.memset(pad[:], 0.0)
                        nc.vector.tensor_copy(pad[:, 1:33, 1:33], hs[:])
                        acc = pcv.tile([128, 1024], dt.bfloat16, tag="acc")
                        t2 = pcv.tile([128, 1024], dt.bfloat16, tag="t2")
                        for tap in range(9):
                            dy, dx = tap // 3, tap % 3
                            view = pad[:, dy:dy + 32, dx:dx + 32]
                            wsl = tapw_sb[:, ct * 9 + tap:ct * 9 + tap + 1]
                            if tap == 0:
                                nc.vector.tensor_scalar(
                                    acc[:].rearrange("p (a b) -> p a b", a=32),
                                    view, wsl, None, op0=alu.mult,
                                )
                            else:
                                nc.vector.tensor_scalar(
                                    t2[:].rearrange("p (a b) -> p a b", a=32),
                                    view, wsl, None, op0=alu.mult,
                                )
                                nc.vector.tensor_tensor(acc[:], acc[:], t2[:], op=alu.add)
                        nc.vector.tensor_scalar(
                            acc[:], acc[:], cb_sb[:, ct:ct + 1], None, op0=alu.add
                        )
                        vct = pvc.tile([128, 1024], dt.bfloat16, tag="vc")
                        nc.vector.tensor_tensor(vct[:], acc[:], pbbc[:], op=alu.add)
                        vc_tiles.append(vct)
                vc_all[b] = vc_tiles

            # ======== phase 3: proj + out per batch ========
            for b in range(BPC):
                vc_tiles = vc_all[b]
                O2v = o_nat[b].rearrange("n c -> (n c)").rearrange("(r m) -> r m", m=N)
                o2t = []
                for mt, (m0, msz) in enumerate(NT):
                    t = po2.tile([msz, DH], dt.bfloat16, tag=f"o2t{mt}")
                    if msz == 128:
                        nc.sync.dma_start_transpose(t[:], O2v[:, m0:m0 + msz])
                    else:
                        nc.sync.dma_start(
                            t[:], O2v[:, m0:m0 + msz].rearrange("a b -> b a")
                        )
                    o2t.append(t)
                xo_sb = [
                    pxo.tile([128, 1024], dt.bfloat16, tag="xo", name=f"xo{rt}")
                    for rt in range(4)
                ]
                for rt in range(4):
                    pss = [
                        pssc.tile([128, 512], dt.float32, tag="sc", name=f"xop{c}")
                        for c in range(2)
                    ]
                    for mt, (m0, msz) in enumerate(NT):
                        for c, jc in enumerate((0, 512)):
                            nc.tensor.matmul(
                                pss[c][:],
                                o2t[mt][0:msz, rt * 128:(rt + 1) * 128],
                                wproj_sb[mt][0:msz, jc:jc + 512],
                                start=(mt == 0), stop=(mt == 8),
                            )
                    for c, jc in enumerate((0, 512)):
                        nc.vector.tensor_tensor(
                            xo_sb[rt][:, jc:jc + 512], pss[c][:],
                            vc_tiles[rt][:, jc:jc + 512], op=alu.add,
                        )

                for jt in range(8):
                    ps = psmm.tile([128, 512], dt.float32, tag="mm")
                    for rt in range(4):
                        nc.tensor.matmul(
                            ps[:],
                            xo_sb[rt][:, jt * 128:(jt + 1) * 128],
                            wout_sb[:, rt, :],
                            start=(rt == 0), stop=(rt == 3),
                        )
                    osb = psm.tile([128, DIM], dt.float32, tag="outsb")
                    nc.vector.tensor_tensor(osb[:], ps[:], obbc[:], op=alu.add)
                    nc.scalar.dma_start(out_ext[b, jt * 128:(jt + 1) * 128, :], osb[:])

    nc.compile()
    return nc


def _prep_weights(qkv_w, bn_gamma, bn_beta, bn_mean, bn_var,
                  conv_w, conv_b, proj_w, proj_b, out_w, out_b):
    s = bn_gamma / np.sqrt(bn_var + EPS)
    bias = bn_beta - bn_mean * s
    Wt = (qkv_w * s[:, None]).T.copy()
    bvec = bias.copy()
    scale = KD ** -0.5
    for hh in range(H):
        Wt[:, hh * 128:hh * 128 + KD] *= scale
        bvec[hh * 128:hh * 128 + KD] *= scale
    # scatter channels into padded 1288 layout (see _build head slicing):
    # cols 0:768 q/k tiles (3 heads/tile at 0,32,64); 768:1288 v channels
    # hh*65+j with a ones channel (zero weight, bias 1) at j=64
    Wn = np.zeros((DIM, 1288), Wt.dtype)
    bn = np.zeros(1288, bvec.dtype)
    for hh in range(H):
        qdst = (hh // 3) * 128 + (hh % 3) * 32
        kdst = 384 + (hh // 3) * 128 + (hh % 3) * 32
        vdst = 768 + hh * 65
        qsrc, ksrc, vsrc = hh * 128, hh * 128 + 32, hh * 128 + 64
        Wn[:, qdst:qdst + 32] = Wt[:, qsrc:qsrc + 32]
        bn[qdst:qdst + 32] = bvec[qsrc:qsrc + 32]
        Wn[:, kdst:kdst + 32] = Wt[:, ksrc:ksrc + 32]
        bn[kdst:kdst + 32] = bvec[ksrc:ksrc + 32]
        Wn[:, vdst:vdst + 64] = Wt[:, vsrc:vsrc + 64]
        bn[vdst:vdst + 64] = bvec[vsrc:vsrc + 64]
        bn[vdst + 64] = 1.0
    return {
        "wqkv": np.ascontiguousarray(Wn).astype(BF16),
        "bqkv": np.ascontiguousarray(bn[:768].reshape(6, 128).T).astype(np.float32),
        "bv": np.ascontiguousarray(bn[768:].reshape(1, 520)).astype(np.float32),
        "wproj": np.ascontiguousarray(proj_w.T).astype(BF16),
        "wout": np.ascontiguousarray(out_w.T).astype(BF16),
        "tapw": np.ascontiguousarray(
            conv_w[:, 0].reshape(4, 128, 9).transpose(1, 0, 2).reshape(128, 36)
        ).astype(np.float32),
        "cbp": np.ascontiguousarray(conv_b.reshape(4, 128).T).astype(np.float32),
        "pbp": proj_b.reshape(1, IMG).astype(np.float32),
        "obp": out_b.reshape(1, DIM).astype(np.float32),
    }


def run(trace=False, tmpdir=None, **inputs):
    from concourse.bass_utils import run_bass_kernel_spmd

    if "nc" not in _cached:
        _cached["nc"] = _build()
    nc = _cached["nc"]

    w = _prep_weights(**{k: np.asarray(v) for k, v in inputs.items() if k != "x"})
    x = np.asarray(inputs["x"]).astype(BF16)
    in_maps = []
    for c in range(NCORES):
        m = dict(w)
        m["xs"] = np.ascontiguousarray(x[c * BPC:(c + 1) * BPC])
        in_maps.append(m)
    res = run_bass_kernel_spmd(
        nc, in_maps, core_ids=list(range(NCORES)), trace=trace, tmpdir=tmpdir
    )
    out = np.concatenate([np.asarray(r["out"]) for r in res.results], axis=0)
    return out.astype(np.float32), res.exec_time_ns


def kernel(**inputs):
    out, _ = run(trace=False, **inputs)
    return out


if __name__ == "__main__":
    print("building graph...")
    nc = _build()
    print("build OK:", len(nc.m.functions[0].allocations), "allocations")



# revision 6
# speedup vs baseline: 1.3384x; 1.3384x over previous
"""Trainium2 Bass kernel for nn_Attention_66546223284383.

Strategy: pure data-parallel over batch B=16 -> 2 batches per core x 8 cores.
Per core, per batch:
  qkvT = (BN-folded W)^T @ x^T           (h on partitions, n free)
  Attention m-dim padded 1044->1152 (zero k/v pad rows contribute
  exp(0)*0 = 0), so all 9 m-tiles are uniform 128 rows.
  q/k packed 4 heads per 128-partition tile at offsets {0,32,64,96};
  scores for 4 heads run CONCURRENTLY via PE row tiling
  (tile_position=(32i,0), K=32 each).
  n-chunk-major softmax: per (group, chunk<=512): per m-tile:
  4 row-tiled score MMs -> 4 psum banks -> exp on ACT -> 4 oT MMs
  ([v|1]^T @ exp, 65 rows: 64 o-dims + denominator) accumulating into
  4 more banks.  n-tail (1024:1044) handled in a merged pass: one
  [128, 9*20] accumulator + ONE exp per head.
  PE-transpose oT -> normalize (ACT Copy with per-partition reciprocal
  scale) -> stage o to DRAM (n, c) bf16.
  conv branch: v reflowed via DRAM to (channel, spatial), hardswish +
  9-tap depthwise conv on VectorE with per-partition tap weights.
  proj:     O2^T tiles read back via XBAR transpose DMA (handles the torch
            "raw reshape" (B,N,H,d)->(B,DH,N) as a flat re-chunk),
            xo = O2 @ proj_w^T + vc  (r on partitions, j free)
  out:      out = xo^T @ out_w^T + out_b -> (1024, 512) fp32.
All matmuls bf16 inputs with fp32 PSUM accumulation.
"""
import sys
import numpy as np

sys.path.insert(0, "/opt/trn_rl_repo")

import ml_dtypes  # noqa: E402

BF16 = ml_dtypes.bfloat16

KD, H, D, DH, DIM, IMG, S = 32, 8, 64, 512, 512, 1024, 32
N, B = 1044, 16
EPS = 1e-5
NCORES = 8
BPC = B // NCORES  # batches per core
MP = 1152          # padded attention m-dim (9 x 128)
NMT = 9            # m-tiles, all 128 rows
NTT = [(i * 128, 128) for i in range(8)] + [(1024, 20)]  # real-n tiles
CHUNKS = [(0, 512), (512, 512)]                          # big n chunks
TLO, TLW = 1024, 20                                      # n tail

_cached = {}


def _build():
    from concourse import bacc, tile
    import concourse.bass as bass
    import concourse.mybir as mybir
    from concourse.masks import make_identity

    dt = mybir.dt
    alu = mybir.AluOpType
    act_exp = mybir.ActivationFunctionType.Exp
    act_copy = mybir.ActivationFunctionType.Copy

    nc = bacc.Bacc(None, target_bir_lowering=False, debug=False)

    xs = nc.declare_dram_parameter("xs", [BPC, N, DIM], dt.bfloat16, isOutput=False)
    # wqkv cols: 0:256 = 2 q tiles (4 heads each at offsets 0/32/64/96),
    # 256:512 = 2 k tiles, 512:1032 = v channels hh*65+j (j==64 is a ones
    # channel: zero weights, bias 1 -> softmax denominator column)
    wqkv = nc.declare_dram_parameter("wqkv", [DIM, 1032], dt.bfloat16, isOutput=False)
    bqkv = nc.declare_dram_parameter("bqkv", [128, 4], dt.float32, isOutput=False)
    bv = nc.declare_dram_parameter("bv", [1, 520], dt.float32, isOutput=False)
    wproj = nc.declare_dram_parameter("wproj", [N, IMG], dt.bfloat16, isOutput=False)
    wout = nc.declare_dram_parameter("wout", [DH, DIM], dt.bfloat16, isOutput=False)
    tapw = nc.declare_dram_parameter("tapw", [128, 36], dt.float32, isOutput=False)
    cpb = nc.declare_dram_parameter("cpb", [128, 4, IMG], dt.bfloat16, isOutput=False)
    obp = nc.declare_dram_parameter("obp", [1, DIM], dt.float32, isOutput=False)
    out_ext = nc.declare_dram_parameter("out", [BPC, IMG, DIM], dt.float32, isOutput=True)

    o_nat = nc.dram_tensor("o_nat", [BPC, N, DH], dt.bfloat16)
    v_dram = nc.dram_tensor("v_dram", [BPC, H, N, D], dt.bfloat16)

    with tile.TileContext(nc) as tc:
        with (
            tc.tile_pool(name="w", bufs=1) as pw,
            tc.tile_pool(name="xT", bufs=8) as pxT,
            tc.tile_pool(name="qkvT", bufs=8) as pqk,
            tc.tile_pool(name="vnat", bufs=18) as pvn,
            tc.tile_pool(name="exp", bufs=10) as pexp,
            tc.tile_pool(name="expt", bufs=4) as pext,
            tc.tile_pool(name="oTs", bufs=6) as poTs,
            tc.tile_pool(name="small", bufs=6) as psm,
            tc.tile_pool(name="conv", bufs=2) as pcv,
            tc.tile_pool(name="cin", bufs=3) as pcin,
            tc.tile_pool(name="vc", bufs=8) as pvc,
            tc.tile_pool(name="o2t", bufs=1) as po2,
            tc.tile_pool(name="xo", bufs=5) as pxo,
            # PSUM budget (8 banks): sc 4x(128,512)=4, ot 4x(65,512)=4
            tc.tile_pool(name="pssc", bufs=4, space=bass.MemorySpace.PSUM) as pssc,
            tc.tile_pool(name="psot", bufs=4, space=bass.MemorySpace.PSUM) as psot,
        ):
            # ---- constants / weights ----
            id_sb = pw.tile([128, 128], dt.bfloat16, tag="id")
            make_identity(nc, id_sb[:])
            wqkv_sb = pw.tile([128, 4, 1032], dt.bfloat16, tag="wqkv")
            nc.sync.dma_start(wqkv_sb[:], wqkv[:].rearrange("(k p) h -> p k h", p=128))
            bqkv_sb = pw.tile([128, 4], dt.float32, tag="bqkv")
            nc.sync.dma_start(bqkv_sb[:], bqkv[:])
            bv_sb = pw.tile([1, 520], dt.float32, tag="bv")
            nc.sync.dma_start(bv_sb[:], bv[:])
            bvbc = pw.tile([128, 520], dt.float32, tag="bvbc")
            nc.gpsimd.partition_broadcast(bvbc[:], bv_sb[:])
            wproj_sb = []
            for mt, (m0, msz) in enumerate(NTT):
                t = pw.tile([msz, 1024], dt.bfloat16, tag=f"wproj{mt}")
                nc.sync.dma_start(t[:], wproj[m0:m0 + msz, :])
                wproj_sb.append(t)
            wout_sb = pw.tile([128, 4, DIM], dt.bfloat16, tag="wout")
            nc.sync.dma_start(wout_sb[:], wout[:].rearrange("(k p) c -> p k c", p=128))
            tapw_sb = pw.tile([128, 36], dt.float32, tag="tapw")
            nc.sync.dma_start(tapw_sb[:], tapw[:])
            cpb_sb = pw.tile([128, 4, IMG], dt.bfloat16, tag="cpb")
            nc.sync.dma_start(cpb_sb[:], cpb[:])
            ob_sb = pw.tile([1, DIM], dt.float32, tag="ob")
            nc.sync.dma_start(ob_sb[:], obp[:])
            obbc = pw.tile([128, DIM], dt.float32, tag="obbc")
            nc.gpsimd.partition_broadcast(obbc[:], ob_sb[:])

            qkvT_all, vnat_all, vc_all = {}, {}, {}

            # ======== phase 1: x^T + qk + v for BOTH batches (PE-dense) ========
            for b in range(BPC):
                xT = []
                for cb4 in range(4):
                    t = pxT.tile([128, N], dt.bfloat16, tag="xT")
                    c0 = cb4 * 128
                    nc.sync.dma_start_transpose(t[:, 0:1040], xs[b, 0:1040, c0:c0 + 128])
                    nc.sync.dma_start(
                        t[:, 1040:N], xs[b, 1040:N, c0:c0 + 128].rearrange("a b -> b a")
                    )
                    xT.append(t)

                # 4 qk tiles: t 0-1 q heads (4/tile @ 0,32,64,96), 2-3 k heads
                # k tiles padded to MP cols; pad region zeroed (never biased)
                qkvT = []
                for t4 in range(4):
                    is_k = t4 >= 2
                    t = pqk.tile([128, MP], dt.bfloat16, tag="qkvT")
                    if is_k:
                        nc.gpsimd.memset(t[:, N:MP], 0.0)
                    pss = [
                        pssc.tile([128, 512], dt.float32, tag="sc", name=f"qk{c}")
                        for c in range(2)
                    ] + [psot.tile([128, 20], dt.float32, tag="ot", name="qk2")]
                    for kc in range(4):
                        for c, (ci, cw) in enumerate(((0, 512), (512, 512), (1024, 20))):
                            nc.tensor.matmul(
                                pss[c][:, 0:cw] if c < 2 else pss[c][:, 0:20],
                                wqkv_sb[:, kc, t4 * 128:(t4 + 1) * 128],
                                xT[kc][:, ci:ci + cw],
                                start=(kc == 0), stop=(kc == 3),
                            )
                    for c, (ci, cw) in enumerate(((0, 512), (512, 512), (1024, 20))):
                        nc.vector.tensor_scalar(
                            t[:, ci:ci + cw],
                            pss[c][:, 0:cw] if c < 2 else pss[c][:, 0:20],
                            bqkv_sb[:, t4:t4 + 1], None, op0=alu.add,
                        )
                    qkvT.append(t)
                qkvT_all[b] = qkvT

                # v directly in (n, 8*65) layout with ones channels; m-tile 8
                # padded with zero rows (20 real)
                vnat = []
                for mt in range(NMT):
                    m0 = mt * 128
                    mreal = 128 if mt < 8 else 20
                    t = pvn.tile([128, 520], dt.bfloat16, tag="vnat")
                    if mreal < 128:
                        nc.gpsimd.memset(t[:], 0.0)
                    pss = [
                        pssc.tile([128, 260], dt.float32, tag="sc", name=f"vn{c}")
                        for c in range(2)
                    ]
                    for kc in range(4):
                        for c in range(2):
                            nc.tensor.matmul(
                                pss[c][0:mreal, 0:260],
                                xT[kc][:, m0:m0 + mreal],
                                wqkv_sb[:, kc, 512 + c * 260:512 + (c + 1) * 260],
                                start=(kc == 0), stop=(kc == 3),
                            )
                    for c in range(2):
                        nc.vector.tensor_tensor(
                            t[0:mreal, c * 260:(c + 1) * 260], pss[c][0:mreal, 0:260],
                            bvbc[0:mreal, c * 260:(c + 1) * 260], op=alu.add,
                        )
                    # stage all 8 heads' v in ONE DMA on the gpsimd queue
                    nc.gpsimd.dma_start(
                        v_dram[b].rearrange("h n d -> n h d")[m0:m0 + mreal],
                        t[0:mreal].rearrange("p (h dd) -> p h dd", h=8)[:, :, 0:D],
                    )
                    vnat.append(t)
                vnat_all[b] = vnat

            # ======== phase 2: attention, 4-head groups, n-chunk-major ========
            for b in range(BPC):
                qkvT, vnat = qkvT_all[b], vnat_all[b]
                vc_tiles = []
                for g in range(2):
                    qTt, kTt = qkvT[g], qkvT[2 + g]
                    heads = [g * 4 + i for i in range(4)]
                    oT_sb = {}
                    for i, hh in enumerate(heads):
                        oT_sb[hh] = poTs.tile(
                            [D + 1, N], dt.bfloat16, tag="oTs", name=f"oTs{hh}"
                        )
                    # big n-chunks
                    for (ci, cw) in CHUNKS:
                        ot_ps = [
                            psot.tile([D + 1, cw], dt.float32, tag="ot", name=f"ot{i}")
                            for i in range(4)
                        ]
                        for mt in range(NMT):
                            m0 = mt * 128
                            ets = []
                            for i in range(4):
                                sc = pssc.tile([128, cw], dt.float32, tag="sc")
                                nc.tensor.matmul(
                                    sc[:], kTt[32 * i:32 * i + 32, m0:m0 + 128],
                                    qTt[32 * i:32 * i + 32, ci:ci + cw],
                                    start=True, stop=True, tile_position=(32 * i, 0),
                                )
                                et = pexp.tile([128, cw], dt.bfloat16, tag="exp")
                                nc.scalar.activation(et[:], sc[:], act_exp)
                                ets.append(et)
                            for i, hh in enumerate(heads):
                                nc.tensor.matmul(
                                    ot_ps[i][:],
                                    vnat[mt][:, hh * 65:hh * 65 + D + 1],
                                    ets[i][:],
                                    start=(mt == 0), stop=(mt == NMT - 1),
                                )
                        for i, hh in enumerate(heads):
                            nc.vector.tensor_copy(oT_sb[hh][:, ci:ci + cw], ot_ps[i][:])
                    # merged n-tail pass: [128, 9*20] accumulators, 1 exp/head
                    tacc = [
                        pssc.tile([128, NMT * TLW], dt.float32, tag="sc", name=f"tl{i}")
                        for i in range(4)
                    ]
                    for mt in range(NMT):
                        m0 = mt * 128
                        for i in range(4):
                            nc.tensor.matmul(
                                tacc[i][:, mt * TLW:(mt + 1) * TLW],
                                kTt[32 * i:32 * i + 32, m0:m0 + 128],
                                qTt[32 * i:32 * i + 32, TLO:TLO + TLW],
                                start=True, stop=True, tile_position=(32 * i, 0),
                            )
                    for i, hh in enumerate(heads):
                        ett = pext.tile([128, NMT * TLW], dt.bfloat16, tag="expt")
                        nc.scalar.activation(ett[:], tacc[i][:], act_exp)
                        ot_t = psot.tile([D + 1, TLW], dt.float32, tag="ot")
                        for mt in range(NMT):
                            nc.tensor.matmul(
                                ot_t[:],
                                vnat[mt][:, hh * 65:hh * 65 + D + 1],
                                ett[:, mt * TLW:(mt + 1) * TLW],
                                start=(mt == 0), stop=(mt == NMT - 1),
                            )
                        nc.vector.tensor_copy(oT_sb[hh][:, TLO:TLO + TLW], ot_t[:])
                    # transpose back, normalize (ACT Copy w/ rcp scale), stage o
                    for i, hh in enumerate(heads):
                        for nt, (n0, nsz) in enumerate(NTT):
                            tp2 = psot.tile([nsz, D + 1], dt.bfloat16, tag="ot")
                            nc.tensor.transpose(
                                tp2[:], oT_sb[hh][:, n0:n0 + nsz], id_sb[0:D + 1, 0:D + 1]
                            )
                            rcp = psm.tile([nsz, 1], dt.float32, tag="rcp")
                            nc.vector.reciprocal(rcp[:], tp2[:, D:D + 1])
                            onrm = psm.tile([nsz, D], dt.bfloat16, tag="onrm")
                            nc.scalar.activation(
                                onrm[:], tp2[:, 0:D], act_copy, scale=rcp[:]
                            )
                            nc.gpsimd.dma_start(
                                o_nat[b, n0:n0 + nsz, hh * D:(hh + 1) * D], onrm[:]
                            )

                    # conv for this group's 2 channel-tiles (bf16 DVE ops)
                    for ct in (2 * g, 2 * g + 1):
                        cin = pcin.tile([128, 1024], dt.bfloat16, tag="cin")
                        for hl in range(2):
                            src = v_dram[b, 2 * ct + hl, 0:1024, :].rearrange(
                                "(c n2) d -> c (n2 d)", c=64
                            )
                            nc.scalar.dma_start(cin[hl * 64:(hl + 1) * 64, :], src)
                        u = pcv.tile([128, 1024], dt.bfloat16, tag="u")
                        nc.vector.tensor_scalar(
                            u[:], cin[:], 1.0 / 6.0, 0.5, op0=alu.mult, op1=alu.add
                        )
                        nc.vector.tensor_scalar(
                            u[:], u[:], 1.0, 0.0, op0=alu.min, op1=alu.max
                        )
                        hs = pcv.tile([128, 32, 32], dt.bfloat16, tag="hs")
                        nc.vector.tensor_tensor(
                            hs[:].rearrange("p a b -> p (a b)"), u[:], cin[:],
                            op=alu.mult,
                        )
                        pad = pcv.tile([128, 34, 34], dt.bfloat16, tag="pad")
                        nc.gpsimd.memset(pad[:], 0.0)
                        nc.vector.tensor_copy(pad[:, 1:33, 1:33], hs[:])
                        acc = pcv.tile([128, 1024], dt.bfloat16, tag="acc")
                        t2 = pcv.tile([128, 1024], dt.bfloat16, tag="t2")
                        for tap in range(9):
                            dy, dx = tap // 3, tap % 3
                            view = pad[:, dy:dy + 32, dx:dx + 32]
                            wsl = tapw_sb[:, ct * 9 + tap:ct * 9 + tap + 1]
                            if tap == 0:
                                nc.vector.tensor_scalar(
                                    acc[:].rearrange("p (a b) -> p a b", a=32),
                                    view, wsl, None, op0=alu.mult,
                                )
                            else:
                                nc.vector.tensor_scalar(
                                    t2[:].rearrange("p (a b) -> p a b", a=32),
                                    view, wsl, None, op0=alu.mult,
                                )
                                nc.vector.tensor_tensor(acc[:], acc[:], t2[:], op=alu.add)
                        vct = pvc.tile([128, 1024], dt.bfloat16, tag="vc")
                        nc.vector.tensor_tensor(vct[:], acc[:], cpb_sb[:, ct, :], op=alu.add)
                        vc_tiles.append(vct)
                vc_all[b] = vc_tiles

            # ======== phase 3: proj + out per batch ========
            for b in range(BPC):
                vc_tiles = vc_all[b]
                O2v = o_nat[b].rearrange("n c -> (n c)").rearrange("(r m) -> r m", m=N)
                o2t = []
                for mt, (m0, msz) in enumerate(NTT):
                    t = po2.tile([msz, DH], dt.bfloat16, tag=f"o2t{mt}")
                    if msz == 128:
                        nc.sync.dma_start_transpose(t[:], O2v[:, m0:m0 + msz])
                    else:
                        nc.sync.dma_start(
                            t[:], O2v[:, m0:m0 + msz].rearrange("a b -> b a")
                        )
                    o2t.append(t)
                xo_sb = [
                    pxo.tile([128, 1024], dt.bfloat16, tag="xo", name=f"xo{rt}")
                    for rt in range(4)
                ]
                for rt in range(4):
                    pss = [
                        pssc.tile([128, 512], dt.float32, tag="sc", name=f"xop{c}")
                        for c in range(2)
                    ]
                    for mt, (m0, msz) in enumerate(NTT):
                        for c, jc in enumerate((0, 512)):
                            nc.tensor.matmul(
                                pss[c][:],
                                o2t[mt][0:msz, rt * 128:(rt + 1) * 128],
                                wproj_sb[mt][0:msz, jc:jc + 512],
                                start=(mt == 0), stop=(mt == 8),
                            )
                    for c, jc in enumerate((0, 512)):
                        nc.vector.tensor_tensor(
                            xo_sb[rt][:, jc:jc + 512], pss[c][:],
                            vc_tiles[rt][:, jc:jc + 512], op=alu.add,
                        )

                for jt in range(8):
                    ps = pssc.tile([128, 512], dt.float32, tag="sc")
                    for rt in range(4):
                        nc.tensor.matmul(
                            ps[:],
                            xo_sb[rt][:, jt * 128:(jt + 1) * 128],
                            wout_sb[:, rt, :],
                            start=(rt == 0), stop=(rt == 3),
                        )
                    osb = psm.tile([128, DIM], dt.float32, tag="outsb")
                    nc.vector.tensor_tensor(osb[:], ps[:], obbc[:], op=alu.add)
                    nc.scalar.dma_start(out_ext[b, jt * 128:(jt + 1) * 128, :], osb[:])

    nc.compile()
    return nc


def _prep_weights(qkv_w, bn_gamma, bn_beta, bn_mean, bn_var,
                  conv_w, conv_b, proj_w, proj_b, out_w, out_b):
    s = bn_gamma / np.sqrt(bn_var + EPS)
    bias = bn_beta - bn_mean * s
    Wt = (qkv_w * s[:, None]).T.copy()
    bvec = bias.copy()
    scale = KD ** -0.5
    for hh in range(H):
        Wt[:, hh * 128:hh * 128 + KD] *= scale
        bvec[hh * 128:hh * 128 + KD] *= scale
    # scatter channels into 1032 layout (see _build head slicing):
    # cols 0:256 q tiles (4 heads/tile at 0,32,64,96), 256:512 k tiles,
    # 512:1032 v channels hh*65+j with a ones channel (zero weight, bias 1)
    Wn = np.zeros((DIM, 1032), Wt.dtype)
    bn = np.zeros(1032, bvec.dtype)
    for hh in range(H):
        qdst = (hh // 4) * 128 + (hh % 4) * 32
        kdst = 256 + (hh // 4) * 128 + (hh % 4) * 32
        vdst = 512 + hh * 65
        qsrc, ksrc, vsrc = hh * 128, hh * 128 + 32, hh * 128 + 64
        Wn[:, qdst:qdst + 32] = Wt[:, qsrc:qsrc + 32]
        bn[qdst:qdst + 32] = bvec[qsrc:qsrc + 32]
        Wn[:, kdst:kdst + 32] = Wt[:, ksrc:ksrc + 32]
        bn[kdst:kdst + 32] = bvec[ksrc:ksrc + 32]
        Wn[:, vdst:vdst + 64] = Wt[:, vsrc:vsrc + 64]
        bn[vdst:vdst + 64] = bvec[vsrc:vsrc + 64]
        bn[vdst + 64] = 1.0
    cpb_t = (conv_b.reshape(4, 128).T[:, :, None]
             + proj_b.reshape(1, 1, IMG))              # [128, 4, 1024]
    return {
        "wqkv": np.ascontiguousarray(Wn).astype(BF16),
        "bqkv": np.ascontiguousarray(bn[:512].reshape(4, 128).T).astype(np.float32),
        "bv": np.ascontiguousarray(bn[512:].reshape(1, 520)).astype(np.float32),
        "wproj": np.ascontiguousarray(proj_w.T).astype(BF16),
        "wout": np.ascontiguousarray(out_w.T).astype(BF16),
        "tapw": np.ascontiguousarray(
            conv_w[:, 0].reshape(4, 128, 9).transpose(1, 0, 2).reshape(128, 36)
        ).astype(np.float32),
        "cpb": np.ascontiguousarray(cpb_t).astype(BF16),
        "obp": out_b.reshape(1, DIM).astype(np.float32),
    }


def run(trace=False, tmpdir=None, **inputs):
    from concourse.bass_utils import run_bass_kernel_spmd

    if "nc" not in _cached:
        _cached["nc"] = _build()
    nc = _cached["nc"]

    w = _prep_weights(**{k: np.asarray(v) for k, v in inputs.items() if k != "x"})
    x = np.asarray(inputs["x"]).astype(BF16)
    in_maps = []
    for c in range(NCORES):
        m = dict(w)
        m["xs"] = np.ascontiguousarray(x[c * BPC:(c + 1) * BPC])
        in_maps.append(m)
    res = run_bass_kernel_spmd(
        nc, in_maps, core_ids=list(range(NCORES)), trace=trace, tmpdir=tmpdir
    )
    out = np.concatenate([np.asarray(r["out"]) for r in res.results], axis=0)
    return out.astype(np.float32), res.exec_time_ns


def kernel(**inputs):
    out, _ = run(trace=False, **inputs)
    return out


if __name__ == "__main__":
    print("building graph...")
    nc = _build()
    print("build OK:", len(nc.m.functions[0].allocations), "allocations")


# revision 17
# speedup vs baseline: 1.3588x; 1.0153x over previous
"""Trainium2 Bass kernel for nn_Attention_66546223284383.

Strategy: pure data-parallel over batch B=16 -> 2 batches per core x 8 cores.
Per core, per batch:
  qkvT = (BN-folded W)^T @ x^T           (h on partitions, n free)
  Attention m-dim padded 1044->1152 (zero k/v pad rows contribute
  exp(0)*0 = 0), so all 9 m-tiles are uniform 128 rows.
  q/k packed 4 heads per 128-partition tile at offsets {0,32,64,96};
  scores for 4 heads run CONCURRENTLY via PE row tiling
  (tile_position=(32i,0), K=32 each).
  n-chunk-major softmax: per (group, chunk<=512): per m-tile:
  4 row-tiled score MMs -> 4 psum banks -> exp on ACT -> 4 oT MMs
  ([v|1]^T @ exp, 65 rows: 64 o-dims + denominator) accumulating into
  4 more banks.  n-tail (1024:1044) handled in a merged pass: one
  [128, 9*20] accumulator + ONE exp per head.
  PE-transpose oT -> normalize (ACT Copy with per-partition reciprocal
  scale) -> stage o to DRAM (n, c) bf16.
  conv branch: v reflowed via DRAM to (channel, spatial), hardswish +
  9-tap depthwise conv on VectorE with per-partition tap weights.
  proj:     O2^T tiles read back via XBAR transpose DMA (handles the torch
            "raw reshape" (B,N,H,d)->(B,DH,N) as a flat re-chunk),
            xo = O2 @ proj_w^T + vc  (r on partitions, j free)
  out:      out = xo^T @ out_w^T + out_b -> (1024, 512) fp32.
All matmuls bf16 inputs with fp32 PSUM accumulation.
"""
import sys
import numpy as np

sys.path.insert(0, "/opt/trn_rl_repo")

import ml_dtypes  # noqa: E402

BF16 = ml_dtypes.bfloat16

KD, H, D, DH, DIM, IMG, S = 32, 8, 64, 512, 512, 1024, 32
N, B = 1044, 16
EPS = 1e-5
NCORES = 8
BPC = B // NCORES  # batches per core
MP = 1152          # padded attention m-dim (9 x 128)
NMT = 9            # m-tiles, all 128 rows
NTT = [(i * 128, 128) for i in range(8)] + [(1024, 20)]  # real-n tiles
CHUNKS = [(0, 512), (512, 512)]                          # big n chunks
TLO, TLW = 1024, 20                                      # n tail

_cached = {}


def _build():
    from concourse import bacc, tile
    import concourse.bass as bass
    import concourse.mybir as mybir
    from concourse.masks import make_identity

    dt = mybir.dt
    alu = mybir.AluOpType
    act_exp = mybir.ActivationFunctionType.Exp
    act_copy = mybir.ActivationFunctionType.Copy

    nc = bacc.Bacc(None, target_bir_lowering=False, debug=False)

    xs = nc.declare_dram_parameter("xs", [BPC, N, DIM], dt.bfloat16, isOutput=False)
    # wqkv cols: 0:256 = 2 q tiles (4 heads each at offsets 0/32/64/96),
    # 256:512 = 2 k tiles, 512:1032 = v channels hh*65+j (j==64 is a ones
    # channel: zero weights, bias 1 -> softmax denominator column)
    wqkv = nc.declare_dram_parameter("wqkv", [DIM, 1032], dt.bfloat16, isOutput=False)
    bqkv = nc.declare_dram_parameter("bqkv", [128, 4], dt.float32, isOutput=False)
    bv = nc.declare_dram_parameter("bv", [1, 520], dt.float32, isOutput=False)
    wproj = nc.declare_dram_parameter("wproj", [N, IMG], dt.bfloat16, isOutput=False)
    wout = nc.declare_dram_parameter("wout", [DH, DIM], dt.bfloat16, isOutput=False)
    tapw = nc.declare_dram_parameter("tapw", [128, 36], dt.float32, isOutput=False)
    cpb = nc.declare_dram_parameter("cpb", [128, 4, IMG], dt.bfloat16, isOutput=False)
    obp = nc.declare_dram_parameter("obp", [1, DIM], dt.float32, isOutput=False)
    out_ext = nc.declare_dram_parameter("out", [BPC, IMG, DIM], dt.float32, isOutput=True)

    o_nat = nc.dram_tensor("o_nat", [BPC, N, DH], dt.bfloat16)
    v_dram = nc.dram_tensor("v_dram", [BPC, H, N, D], dt.bfloat16)

    import contextlib
    with tile.TileContext(nc) as tc:
        with contextlib.ExitStack() as _st:
            ec = _st.enter_context
            pw = ec(tc.tile_pool(name="w", bufs=1))
            pxT = ec(tc.tile_pool(name="xT", bufs=7))
            pqk = ec(tc.tile_pool(name="qkvT", bufs=8))
            pvn = ec(tc.tile_pool(name="vnat", bufs=18))
            pexp = ec(tc.tile_pool(name="exp", bufs=13))
            pext = ec(tc.tile_pool(name="expt", bufs=2))
            poTs = ec(tc.tile_pool(name="oTs", bufs=5))
            psm = ec(tc.tile_pool(name="small", bufs=6))
            posb = ec(tc.tile_pool(name="osb", bufs=2))
            pcv = ec(tc.tile_pool(name="conv", bufs=1))
            pcin = ec(tc.tile_pool(name="cin", bufs=2))
            pvc = ec(tc.tile_pool(name="vc", bufs=8))
            po2 = ec(tc.tile_pool(name="o2t", bufs=1))
            pxo = ec(tc.tile_pool(name="xo", bufs=4))
            # PSUM budget (8 banks): sc 2x(128,1024)=4, ot 2x(65,512)=2,
            # fl 2x(128,512)=2 (fillers: qkv/vnat/proj/out)
            pssc = ec(tc.tile_pool(name="pssc", bufs=2, space=bass.MemorySpace.PSUM))
            psot = ec(tc.tile_pool(name="psot", bufs=2, space=bass.MemorySpace.PSUM))
            psfl = ec(tc.tile_pool(name="psfl", bufs=2, space=bass.MemorySpace.PSUM))
            # ---- constants / weights ----
            id_sb = pw.tile([128, 128], dt.bfloat16, tag="id")
            make_identity(nc, id_sb[:])
            wqkv_sb = pw.tile([128, 4, 1032], dt.bfloat16, tag="wqkv")
            nc.sync.dma_start(wqkv_sb[:], wqkv[:].rearrange("(k p) h -> p k h", p=128))
            bqkv_sb = pw.tile([128, 4], dt.float32, tag="bqkv")
            nc.sync.dma_start(bqkv_sb[:], bqkv[:])
            bv_sb = pw.tile([1, 520], dt.float32, tag="bv")
            nc.sync.dma_start(bv_sb[:], bv[:])
            bvbc = pw.tile([128, 520], dt.float32, tag="bvbc")
            nc.gpsimd.partition_broadcast(bvbc[:], bv_sb[:])
            wproj_sb = []
            for mt, (m0, msz) in enumerate(NTT):
                t = pw.tile([msz, 1024], dt.bfloat16, tag=f"wproj{mt}")
                nc.sync.dma_start(t[:], wproj[m0:m0 + msz, :])
                wproj_sb.append(t)
            wout_sb = pw.tile([128, 4, DIM], dt.bfloat16, tag="wout")
            nc.sync.dma_start(wout_sb[:], wout[:].rearrange("(k p) c -> p k c", p=128))
            tapw_sb = pw.tile([128, 36], dt.float32, tag="tapw")
            nc.sync.dma_start(tapw_sb[:], tapw[:])
            cpb_sb = pw.tile([128, 4, IMG], dt.bfloat16, tag="cpb")
            nc.sync.dma_start(cpb_sb[:], cpb[:])
            ob_sb = pw.tile([1, DIM], dt.float32, tag="ob")
            nc.sync.dma_start(ob_sb[:], obp[:])
            obbc = pw.tile([128, DIM], dt.float32, tag="obbc")
            nc.gpsimd.partition_broadcast(obbc[:], ob_sb[:])

            qkvT_all, vnat_all, vc_all = {}, {}, {}

            # ======== phase 1: x^T + qk + v for BOTH batches (PE-dense) ========
            for b in range(BPC):
                xT = []
                for cb4 in range(4):
                    t = pxT.tile([128, N], dt.bfloat16, tag="xT")
                    c0 = cb4 * 128
                    nc.sync.dma_start_transpose(t[:, 0:1040], xs[b, 0:1040, c0:c0 + 128])
                    nc.sync.dma_start(
                        t[:, 1040:N], xs[b, 1040:N, c0:c0 + 128].rearrange("a b -> b a")
                    )
                    xT.append(t)

                # 4 qk tiles: t 0-1 q heads (4/tile @ 0,32,64,96), 2-3 k heads
                # k tiles padded to MP cols; pad region zeroed (never biased)
                qkvT = []
                for t4 in range(4):
                    is_k = t4 >= 2
                    t = pqk.tile([128, MP], dt.bfloat16, tag="qkvT")
                    if is_k:
                        nc.gpsimd.memset(t[:, N:MP], 0.0)
                    pss = [
                        psfl.tile([128, 512], dt.float32, tag="fl", name=f"qk{c}")
                        for c in range(2)
                    ] + [psot.tile([128, 20], dt.float32, tag="ot", name="qk2")]
                    for kc in range(4):
                        for c, (ci, cw) in enumerate(((0, 512), (512, 512), (1024, 20))):
                            nc.tensor.matmul(
                                pss[c][:, 0:cw] if c < 2 else pss[c][:, 0:20],
                                wqkv_sb[:, kc, t4 * 128:(t4 + 1) * 128],
                                xT[kc][:, ci:ci + cw],
                                start=(kc == 0), stop=(kc == 3),
                            )
                    for c, (ci, cw) in enumerate(((0, 512), (512, 512), (1024, 20))):
                        nc.vector.tensor_scalar(
                            t[:, ci:ci + cw],
                            pss[c][:, 0:cw] if c < 2 else pss[c][:, 0:20],
                            bqkv_sb[:, t4:t4 + 1], None, op0=alu.add,
                        )
                    qkvT.append(t)
                qkvT_all[b] = qkvT

                # v directly in (n, 8*65) layout with ones channels; m-tile 8
                # padded with zero rows (20 real)
                vnat = []
                for mt in range(NMT):
                    m0 = mt * 128
                    mreal = 128 if mt < 8 else 20
                    t = pvn.tile([128, 520], dt.bfloat16, tag="vnat")
                    if mreal < 128:
                        nc.gpsimd.memset(t[:], 0.0)
                    pss = [
                        psfl.tile([128, 260], dt.float32, tag="fl", name=f"vn{c}")
                        for c in range(2)
                    ]
                    for kc in range(4):
                        for c in range(2):
                            nc.tensor.matmul(
                                pss[c][0:mreal, 0:260],
                                xT[kc][:, m0:m0 + mreal],
                                wqkv_sb[:, kc, 512 + c * 260:512 + (c + 1) * 260],
                                start=(kc == 0), stop=(kc == 3),
                            )
                    for c in range(2):
                        nc.vector.tensor_tensor(
                            t[0:mreal, c * 260:(c + 1) * 260], pss[c][0:mreal, 0:260],
                            bvbc[0:mreal, c * 260:(c + 1) * 260], op=alu.add,
                        )
                    # stage all 8 heads' v in ONE DMA on the gpsimd queue
                    nc.gpsimd.dma_start(
                        v_dram[b].rearrange("h n d -> n h d")[m0:m0 + mreal],
                        t[0:mreal].rearrange("p (h dd) -> p h dd", h=8)[:, :, 0:D],
                    )
                    vnat.append(t)
                vnat_all[b] = vnat

            # ======== phase 2: attention, head pairs, big-exp per (h,mt) ========
            def emit_scores(b, pr):
                qkvT, vnat = qkvT_all[b], vnat_all[b]
                t4, half = pr // 2, pr % 2
                qTt, kTt = qkvT[t4], qkvT[2 + t4]
                rows = (64 * half, 64 * half + 32)
                heads = [t4 * 4 + half * 2, t4 * 4 + half * 2 + 1]
                ets = {}
                # scores + exp: one [128,1024] 2-bank tile + one exp per
                # (head, mt); the pair's score MMs run concurrently via
                # PE row tiling
                for mt in range(NMT):
                    m0 = mt * 128
                    scs = [
                        pssc.tile([128, 1024], dt.float32, tag="sc", name=f"sc{i}")
                        for i in range(2)
                    ]
                    for (ci, cw) in CHUNKS:
                        for i in range(2):
                            nc.tensor.matmul(
                                scs[i][:, ci:ci + cw],
                                kTt[rows[i]:rows[i] + 32, m0:m0 + 128],
                                qTt[rows[i]:rows[i] + 32, ci:ci + cw],
                                start=True, stop=True, tile_position=(rows[i], 0),
                            )
                    for i, hh in enumerate(heads):
                        et = pexp.tile([128, 1024], dt.bfloat16, tag="exp")
                        nc.scalar.activation(et[:], scs[i][:], act_exp)
                        ets[(hh, mt)] = et
                return heads, rows, qTt, kTt, ets

            def emit_otail(b, pr, heads, rows, qTt, kTt, ets, oT_sb):
                vnat = vnat_all[b]
                # oT accumulation per head, both n-chunks per m-tile so each
                # et tile is released right after its m-tile is consumed
                for hh in heads:
                    ots = [
                        psot.tile([D + 1, cw], dt.float32, tag="ot", name=f"ot{c}")
                        for c, (ci, cw) in enumerate(CHUNKS)
                    ]
                    for mt in range(NMT):
                        for c, (ci, cw) in enumerate(CHUNKS):
                            nc.tensor.matmul(
                                ots[c][:],
                                vnat[mt][:, hh * 65:hh * 65 + D + 1],
                                ets[(hh, mt)][:, ci:ci + cw],
                                start=(mt == 0), stop=(mt == NMT - 1),
                            )
                    for c, (ci, cw) in enumerate(CHUNKS):
                        nc.vector.tensor_copy(oT_sb[hh][:, ci:ci + cw], ots[c][:])
                # merged n-tail pass: [128, 9*20] accumulators, 1 exp/head
                tacc = [
                    psot.tile([128, NMT * TLW], dt.float32, tag="ot", name=f"tl{i}")
                    for i in range(2)
                ]
                for mt in range(NMT):
                    m0 = mt * 128
                    for i in range(2):
                        nc.tensor.matmul(
                            tacc[i][:, mt * TLW:(mt + 1) * TLW],
                            kTt[rows[i]:rows[i] + 32, m0:m0 + 128],
                            qTt[rows[i]:rows[i] + 32, TLO:TLO + TLW],
                            start=True, stop=True, tile_position=(rows[i], 0),
                        )
                for i, hh in enumerate(heads):
                    ett = pext.tile([128, NMT * TLW], dt.bfloat16, tag="expt")
                    nc.scalar.activation(ett[:], tacc[i][:], act_exp)
                    ot_t = psot.tile([D + 1, TLW], dt.float32, tag="ot")
                    for mt in range(NMT):
                        nc.tensor.matmul(
                            ot_t[:],
                            vnat[mt][:, hh * 65:hh * 65 + D + 1],
                            ett[:, mt * TLW:(mt + 1) * TLW],
                            start=(mt == 0), stop=(mt == NMT - 1),
                        )
                    nc.vector.tensor_copy(oT_sb[hh][:, TLO:TLO + TLW], ot_t[:])

            def emit_stage(b, heads, oT_sb):
                # transpose back, normalize, stage o
                for hh in heads:
                    for nt, (n0, nsz) in enumerate(NTT):
                        tp2 = psot.tile([nsz, D + 1], dt.bfloat16, tag="ot")
                        nc.tensor.transpose(
                            tp2[:], oT_sb[hh][:, n0:n0 + nsz], id_sb[0:D + 1, 0:D + 1]
                        )
                        rcp = psm.tile([nsz, 1], dt.float32, tag="rcp")
                        nc.vector.reciprocal(rcp[:], tp2[:, D:D + 1])
                        onrm = psm.tile([nsz, D], dt.bfloat16, tag="onrm")
                        nc.vector.tensor_scalar(
                            onrm[:], tp2[:, 0:D], rcp[:], None, op0=alu.mult
                        )
                        nc.sync.dma_start(
                            o_nat[b, n0:n0 + nsz, hh * D:(hh + 1) * D], onrm[:]
                        )

            def emit_conv(b, ct, vc_tiles):
                # conv for this pair's channel-tile (bf16 DVE ops)
                cin = pcin.tile([128, 1024], dt.bfloat16, tag="cin")
                for hl in range(2):
                    src = v_dram[b, 2 * ct + hl, 0:1024, :].rearrange(
                        "(c n2) d -> c (n2 d)", c=64
                    )
                    nc.scalar.dma_start(cin[hl * 64:(hl + 1) * 64, :], src)
                u = pcv.tile([128, 1024], dt.bfloat16, tag="u")
                nc.vector.tensor_scalar(
                    u[:], cin[:], 1.0 / 6.0, 0.5, op0=alu.mult, op1=alu.add
                )
                nc.vector.tensor_scalar(
                    u[:], u[:], 1.0, 0.0, op0=alu.min, op1=alu.max
                )
                hs = pcv.tile([128, 32, 32], dt.bfloat16, tag="hs")
                nc.vector.tensor_tensor(
                    hs[:].rearrange("p a b -> p (a b)"), u[:], cin[:], op=alu.mult,
                )
                pad = pcv.tile([128, 34, 34], dt.bfloat16, tag="pad")
                nc.gpsimd.memset(pad[:], 0.0)
                nc.vector.tensor_copy(pad[:, 1:33, 1:33], hs[:])
                acc = pcv.tile([128, 1024], dt.bfloat16, tag="acc")
                t2 = pcv.tile([128, 1024], dt.bfloat16, tag="t2")
                for tap in range(9):
                    dy, dx = tap // 3, tap % 3
                    view = pad[:, dy:dy + 32, dx:dx + 32]
                    wsl = tapw_sb[:, ct * 9 + tap:ct * 9 + tap + 1]
                    if tap == 0:
                        nc.vector.tensor_scalar(
                            acc[:].rearrange("p (a b) -> p a b", a=32),
                            view, wsl, None, op0=alu.mult,
                        )
                    else:
                        nc.vector.tensor_scalar(
                            t2[:].rearrange("p (a b) -> p a b", a=32),
                            view, wsl, None, op0=alu.mult,
                        )
                        nc.vector.tensor_tensor(acc[:], acc[:], t2[:], op=alu.add)
                vct = pvc.tile([128, 1024], dt.bfloat16, tag="vc")
                nc.vector.tensor_tensor(vct[:], acc[:], cpb_sb[:, ct, :], op=alu.add)
                vc_tiles.append(vct)

            for b in range(BPC):
                vc_tiles = []
                for pr in range(4):
                    heads, rows, qTt, kTt, ets = emit_scores(b, pr)
                    oT_sb = {}
                    for hh in heads:
                        oT_sb[hh] = poTs.tile(
                            [D + 1, N], dt.bfloat16, tag="oTs", name=f"oTs{hh}"
                        )
                    emit_otail(b, pr, heads, rows, qTt, kTt, ets, oT_sb)
                    emit_stage(b, heads, oT_sb)
                    emit_conv(b, pr, vc_tiles)
                vc_all[b] = vc_tiles

            # ======== phase 3: proj + out per batch ========
            for b in range(BPC):
                vc_tiles = vc_all[b]
                O2v = o_nat[b].rearrange("n c -> (n c)").rearrange("(r m) -> r m", m=N)
                o2t = []
                for mt, (m0, msz) in enumerate(NTT):
                    t = po2.tile([msz, DH], dt.bfloat16, tag=f"o2t{mt}")
                    if msz == 128:
                        nc.sync.dma_start_transpose(t[:], O2v[:, m0:m0 + msz])
                    else:
                        nc.sync.dma_start(
                            t[:], O2v[:, m0:m0 + msz].rearrange("a b -> b a")
                        )
                    o2t.append(t)
                xo_sb = [
                    pxo.tile([128, 1024], dt.bfloat16, tag="xo", name=f"xo{rt}")
                    for rt in range(4)
                ]
                for rt in range(4):
                    pss = [
                        psfl.tile([128, 512], dt.float32, tag="fl", name=f"xop{c}")
                        for c in range(2)
                    ]
                    for mt, (m0, msz) in enumerate(NTT):
                        for c, jc in enumerate((0, 512)):
                            nc.tensor.matmul(
                                pss[c][:],
                                o2t[mt][0:msz, rt * 128:(rt + 1) * 128],
                                wproj_sb[mt][0:msz, jc:jc + 512],
                                start=(mt == 0), stop=(mt == 8),
                            )
                    for c, jc in enumerate((0, 512)):
                        nc.vector.tensor_tensor(
                            xo_sb[rt][:, jc:jc + 512], pss[c][:],
                            vc_tiles[rt][:, jc:jc + 512], op=alu.add,
                        )

                for jt in range(8):
                    ps = psfl.tile([128, 512], dt.float32, tag="fl")
                    for rt in range(4):
                        nc.tensor.matmul(
                            ps[:],
                            xo_sb[rt][:, jt * 128:(jt + 1) * 128],
                            wout_sb[:, rt, :],
                            start=(rt == 0), stop=(rt == 3),
                        )
                    osb = posb.tile([128, DIM], dt.float32, tag="outsb")
                    nc.vector.tensor_tensor(osb[:], ps[:], obbc[:], op=alu.add)
                    nc.scalar.dma_start(out_ext[b, jt * 128:(jt + 1) * 128, :], osb[:])

    nc.compile()
    return nc


def _prep_weights(qkv_w, bn_gamma, bn_beta, bn_mean, bn_var,
                  conv_w, conv_b, proj_w, proj_b, out_w, out_b):
    s = bn_gamma / np.sqrt(bn_var + EPS)
    bias = bn_beta - bn_mean * s
    Wt = (qkv_w * s[:, None]).T.copy()
    bvec = bias.copy()
    scale = KD ** -0.5
    for hh in range(H):
        Wt[:, hh * 128:hh * 128 + KD] *= scale
        bvec[hh * 128:hh * 128 + KD] *= scale
    # scatter channels into 1032 layout (see _build head slicing):
    # cols 0:256 q tiles (4 heads/tile at 0,32,64,96), 256:512 k tiles,
    # 512:1032 v channels hh*65+j with a ones channel (zero weight, bias 1)
    Wn = np.zeros((DIM, 1032), Wt.dtype)
    bn = np.zeros(1032, bvec.dtype)
    for hh in range(H):
        qdst = (hh // 4) * 128 + (hh % 4) * 32
        kdst = 256 + (hh // 4) * 128 + (hh % 4) * 32
        vdst = 512 + hh * 65
        qsrc, ksrc, vsrc = hh * 128, hh * 128 + 32, hh * 128 + 64
        Wn[:, qdst:qdst + 32] = Wt[:, qsrc:qsrc + 32]
        bn[qdst:qdst + 32] = bvec[qsrc:qsrc + 32]
        Wn[:, kdst:kdst + 32] = Wt[:, ksrc:ksrc + 32]
        bn[kdst:kdst + 32] = bvec[ksrc:ksrc + 32]
        Wn[:, vdst:vdst + 64] = Wt[:, vsrc:vsrc + 64]
        bn[vdst:vdst + 64] = bvec[vsrc:vsrc + 64]
        bn[vdst + 64] = 1.0
    cpb_t = (conv_b.reshape(4, 128).T[:, :, None]
             + proj_b.reshape(1, 1, IMG))              # [128, 4, 1024]
    return {
        "wqkv": np.ascontiguousarray(Wn).astype(BF16),
        "bqkv": np.ascontiguousarray(bn[:512].reshape(4, 128).T).astype(np.float32),
        "bv": np.ascontiguousarray(bn[512:].reshape(1, 520)).astype(np.float32),
        "wproj": np.ascontiguousarray(proj_w.T).astype(BF16),
        "wout": np.ascontiguousarray(out_w.T).astype(BF16),
        "tapw": np.ascontiguousarray(
            conv_w[:, 0].reshape(4, 128, 9).transpose(1, 0, 2).reshape(128, 36)
        ).astype(np.float32),
        "cpb": np.ascontiguousarray(cpb_t).astype(BF16),
        "obp": out_b.reshape(1, DIM).astype(np.float32),
    }


def run(trace=False, tmpdir=None, **inputs):
    from concourse.bass_utils import run_bass_kernel_spmd

    if "nc" not in _cached:
        _cached["nc"] = _build()
    nc = _cached["nc"]

    w = _prep_weights(**{k: np.asarray(v) for k, v in inputs.items() if k != "x"})
    x = np.asarray(inputs["x"]).astype(BF16)
    in_maps = []
    for c in range(NCORES):
        m = dict(w)
        m["xs"] = np.ascontiguousarray(x[c * BPC:(c + 1) * BPC])
        in_maps.append(m)
    res = run_bass_kernel_spmd(
        nc, in_maps, core_ids=list(range(NCORES)), trace=trace, tmpdir=tmpdir
    )
    out = np.concatenate([np.asarray(r["out"]) for r in res.results], axis=0)
    return out.astype(np.float32), res.exec_time_ns


def kernel(**inputs):
    out, _ = run(trace=False, **inputs)
    return out


if __name__ == "__main__":
    print("building graph...")
    nc = _build()
    print("build OK:", len(nc.m.functions[0].allocations), "allocations")


# revision 22
# speedup vs baseline: 1.3801x; 1.0156x over previous
"""Trainium2 Bass kernel for nn_Attention_66546223284383.

Strategy: pure data-parallel over batch B=16 -> 2 batches per core x 8 cores.
Per core, per batch:
  qkvT = (BN-folded W)^T @ x^T           (h on partitions, n free)
  Attention m-dim padded 1044->1152 (zero k/v pad rows contribute
  exp(0)*0 = 0), so all 9 m-tiles are uniform 128 rows.
  q/k packed 4 heads per 128-partition tile at offsets {0,32,64,96};
  scores for 4 heads run CONCURRENTLY via PE row tiling
  (tile_position=(32i,0), K=32 each).
  n-chunk-major softmax: per (group, chunk<=512): per m-tile:
  4 row-tiled score MMs -> 4 psum banks -> exp on ACT -> 4 oT MMs
  ([v|1]^T @ exp, 65 rows: 64 o-dims + denominator) accumulating into
  4 more banks.  n-tail (1024:1044) handled in a merged pass: one
  [128, 9*20] accumulator + ONE exp per head.
  PE-transpose oT -> normalize (ACT Copy with per-partition reciprocal
  scale) -> stage o to DRAM (n, c) bf16.
  conv branch: v reflowed via DRAM to (channel, spatial), hardswish +
  9-tap depthwise conv on VectorE with per-partition tap weights.
  proj:     O2^T tiles read back via XBAR transpose DMA (handles the torch
            "raw reshape" (B,N,H,d)->(B,DH,N) as a flat re-chunk),
            xo = O2 @ proj_w^T + vc  (r on partitions, j free)
  out:      out = xo^T @ out_w^T + out_b -> (1024, 512) fp32.
All matmuls bf16 inputs with fp32 PSUM accumulation.
"""
import sys
import numpy as np

sys.path.insert(0, "/opt/trn_rl_repo")

import ml_dtypes  # noqa: E402

BF16 = ml_dtypes.bfloat16

KD, H, D, DH, DIM, IMG, S = 32, 8, 64, 512, 512, 1024, 32
N, B = 1044, 16
EPS = 1e-5
NCORES = 8
BPC = B // NCORES  # batches per core
MP = 1152          # padded attention m-dim (9 x 128)
NMT = 9            # m-tiles, all 128 rows
NTT = [(i * 128, 128) for i in range(8)] + [(1024, 20)]  # real-n tiles
CHUNKS = [(0, 512), (512, 512)]                          # big n chunks
TLO, TLW = 1024, 20                                      # n tail

_cached = {}


def _build():
    from concourse import bacc, tile
    import concourse.bass as bass
    import concourse.mybir as mybir
    from concourse.masks import make_identity

    dt = mybir.dt
    alu = mybir.AluOpType
    act_exp = mybir.ActivationFunctionType.Exp
    act_copy = mybir.ActivationFunctionType.Copy

    nc = bacc.Bacc(None, target_bir_lowering=False, debug=False)

    xs = nc.declare_dram_parameter("xs", [BPC, N, DIM], dt.bfloat16, isOutput=False)
    # wqkv cols: 0:256 = 2 q tiles (4 heads each at offsets 0/32/64/96),
    # 256:512 = 2 k tiles, 512:1032 = v channels hh*65+j (j==64 is a ones
    # channel: zero weights, bias 1 -> softmax denominator column)
    wqkv = nc.declare_dram_parameter("wqkv", [DIM, 1032], dt.bfloat16, isOutput=False)
    bqkv = nc.declare_dram_parameter("bqkv", [128, 4], dt.float32, isOutput=False)
    bv = nc.declare_dram_parameter("bv", [1, 520], dt.float32, isOutput=False)
    wproj = nc.declare_dram_parameter("wproj", [N, IMG], dt.bfloat16, isOutput=False)
    wout = nc.declare_dram_parameter("wout", [DH, DIM], dt.bfloat16, isOutput=False)
    tapw = nc.declare_dram_parameter("tapw", [128, 36], dt.float32, isOutput=False)
    cpb = nc.declare_dram_parameter("cpb", [128, 4, IMG], dt.bfloat16, isOutput=False)
    obp = nc.declare_dram_parameter("obp", [1, DIM], dt.float32, isOutput=False)
    out_ext = nc.declare_dram_parameter("out", [BPC, IMG, DIM], dt.float32, isOutput=True)

    o_nat = nc.dram_tensor("o_nat", [BPC, N, DH], dt.bfloat16)
    v_dram = nc.dram_tensor("v_dram", [BPC, H, N, D], dt.bfloat16)

    import contextlib
    with tile.TileContext(nc) as tc:
        with contextlib.ExitStack() as _st:
            ec = _st.enter_context
            pw = ec(tc.tile_pool(name="w", bufs=1))
            pxT = ec(tc.tile_pool(name="xT", bufs=7))
            pqk = ec(tc.tile_pool(name="qkvT", bufs=8))
            pvn = ec(tc.tile_pool(name="vnat", bufs=18))
            pexp = ec(tc.tile_pool(name="exp", bufs=13))
            pext = ec(tc.tile_pool(name="expt", bufs=2))
            poTs = ec(tc.tile_pool(name="oTs", bufs=5))
            psm = ec(tc.tile_pool(name="small", bufs=6))
            posb = ec(tc.tile_pool(name="osb", bufs=2))
            pcv = ec(tc.tile_pool(name="conv", bufs=1))
            pcin = ec(tc.tile_pool(name="cin", bufs=2))
            pvc = ec(tc.tile_pool(name="vc", bufs=8))
            po2 = ec(tc.tile_pool(name="o2t", bufs=1))
            pxo = ec(tc.tile_pool(name="xo", bufs=4))
            # PSUM budget (8 banks): sc 3x(128,1024)=6, ot 2x(65,512)=2
            # (qkv/vnat/proj/out ride the "ot" slots, chunk-serial)
            pssc = ec(tc.tile_pool(name="pssc", bufs=3, space=bass.MemorySpace.PSUM))
            psot = ec(tc.tile_pool(name="psot", bufs=2, space=bass.MemorySpace.PSUM))
            # ---- constants / weights ----
            id_sb = pw.tile([128, 128], dt.bfloat16, tag="id")
            make_identity(nc, id_sb[:])
            wqkv_sb = pw.tile([128, 4, 1032], dt.bfloat16, tag="wqkv")
            nc.sync.dma_start(wqkv_sb[:], wqkv[:].rearrange("(k p) h -> p k h", p=128))
            bqkv_sb = pw.tile([128, 4], dt.float32, tag="bqkv")
            nc.sync.dma_start(bqkv_sb[:], bqkv[:])
            bv_sb = pw.tile([1, 520], dt.float32, tag="bv")
            nc.sync.dma_start(bv_sb[:], bv[:])
            bvbc = pw.tile([128, 520], dt.float32, tag="bvbc")
            nc.gpsimd.partition_broadcast(bvbc[:], bv_sb[:])
            wproj_sb = []
            for mt, (m0, msz) in enumerate(NTT):
                t = pw.tile([msz, 1024], dt.bfloat16, tag=f"wproj{mt}")
                nc.sync.dma_start(t[:], wproj[m0:m0 + msz, :])
                wproj_sb.append(t)
            wout_sb = pw.tile([128, 4, DIM], dt.bfloat16, tag="wout")
            nc.sync.dma_start(wout_sb[:], wout[:].rearrange("(k p) c -> p k c", p=128))
            tapw_sb = pw.tile([128, 36], dt.float32, tag="tapw")
            nc.sync.dma_start(tapw_sb[:], tapw[:])
            cpb_sb = pw.tile([128, 4, IMG], dt.bfloat16, tag="cpb")
            nc.sync.dma_start(cpb_sb[:], cpb[:])
            ob_sb = pw.tile([1, DIM], dt.float32, tag="ob")
            nc.sync.dma_start(ob_sb[:], obp[:])
            obbc = pw.tile([128, DIM], dt.float32, tag="obbc")
            nc.gpsimd.partition_broadcast(obbc[:], ob_sb[:])

            qkvT_all, vnat_all, vc_all = {}, {}, {}

            # ======== phase 1: x^T + qk + v for BOTH batches (PE-dense) ========
            for b in range(BPC):
                xT = []
                for cb4 in range(4):
                    t = pxT.tile([128, N], dt.bfloat16, tag="xT")
                    c0 = cb4 * 128
                    nc.sync.dma_start_transpose(t[:, 0:1040], xs[b, 0:1040, c0:c0 + 128])
                    nc.sync.dma_start(
                        t[:, 1040:N], xs[b, 1040:N, c0:c0 + 128].rearrange("a b -> b a")
                    )
                    xT.append(t)

                # 4 qk tiles: t 0-1 q heads (4/tile @ 0,32,64,96), 2-3 k heads
                # k tiles padded to MP cols; pad region zeroed (never biased)
                qkvT = []
                for t4 in range(4):
                    is_k = t4 >= 2
                    t = pqk.tile([128, MP], dt.bfloat16, tag="qkvT")
                    if is_k:
                        nc.gpsimd.memset(t[:, N:MP], 0.0)
                    for (ci, cw) in ((0, 512), (512, 512), (1024, 20)):
                        ps = psot.tile([128, 512], dt.float32, tag="ot", name="qkp")
                        for kc in range(4):
                            nc.tensor.matmul(
                                ps[:, 0:cw],
                                wqkv_sb[:, kc, t4 * 128:(t4 + 1) * 128],
                                xT[kc][:, ci:ci + cw],
                                start=(kc == 0), stop=(kc == 3),
                            )
                        nc.vector.tensor_scalar(
                            t[:, ci:ci + cw], ps[:, 0:cw],
                            bqkv_sb[:, t4:t4 + 1], None, op0=alu.add,
                        )
                    qkvT.append(t)
                qkvT_all[b] = qkvT

                # v directly in (n, 8*65) layout with ones channels; m-tile 8
                # padded with zero rows (20 real)
                vnat = []
                for mt in range(NMT):
                    m0 = mt * 128
                    mreal = 128 if mt < 8 else 20
                    t = pvn.tile([128, 520], dt.bfloat16, tag="vnat")
                    if mreal < 128:
                        nc.gpsimd.memset(t[:], 0.0)
                    for c in range(2):
                        ps = psot.tile([128, 512], dt.float32, tag="ot", name="vnp")
                        for kc in range(4):
                            nc.tensor.matmul(
                                ps[0:mreal, 0:260],
                                xT[kc][:, m0:m0 + mreal],
                                wqkv_sb[:, kc, 512 + c * 260:512 + (c + 1) * 260],
                                start=(kc == 0), stop=(kc == 3),
                            )
                        nc.vector.tensor_tensor(
                            t[0:mreal, c * 260:(c + 1) * 260], ps[0:mreal, 0:260],
                            bvbc[0:mreal, c * 260:(c + 1) * 260], op=alu.add,
                        )
                    # stage all 8 heads' v in ONE DMA on the gpsimd queue
                    nc.gpsimd.dma_start(
                        v_dram[b].rearrange("h n d -> n h d")[m0:m0 + mreal],
                        t[0:mreal].rearrange("p (h dd) -> p h dd", h=8)[:, :, 0:D],
                    )
                    vnat.append(t)
                vnat_all[b] = vnat

            # ======== phase 2: attention, head pairs, big-exp per (h,mt) ========
            def emit_scores(b, pr):
                qkvT, vnat = qkvT_all[b], vnat_all[b]
                t4, half = pr // 2, pr % 2
                qTt, kTt = qkvT[t4], qkvT[2 + t4]
                rows = (64 * half, 64 * half + 32)
                heads = [t4 * 4 + half * 2, t4 * 4 + half * 2 + 1]
                ets = {}
                # scores + exp: one [128,1024] 2-bank tile + one exp per
                # (head, mt); the pair's score MMs run concurrently via
                # PE row tiling
                for mt in range(NMT):
                    m0 = mt * 128
                    scs = [
                        pssc.tile([128, 1024], dt.float32, tag="sc", name=f"sc{i}")
                        for i in range(2)
                    ]
                    for (ci, cw) in CHUNKS:
                        for i in range(2):
                            nc.tensor.matmul(
                                scs[i][:, ci:ci + cw],
                                kTt[rows[i]:rows[i] + 32, m0:m0 + 128],
                                qTt[rows[i]:rows[i] + 32, ci:ci + cw],
                                start=True, stop=True, tile_position=(rows[i], 0),
                            )
                    for i, hh in enumerate(heads):
                        et = pexp.tile([128, 1024], dt.bfloat16, tag="exp")
                        nc.scalar.activation(et[:], scs[i][:], act_exp)
                        ets[(hh, mt)] = et
                return heads, rows, qTt, kTt, ets

            def emit_otail(b, pr, heads, rows, qTt, kTt, ets, oT_sb):
                vnat = vnat_all[b]
                # oT accumulation per head, both n-chunks per m-tile so each
                # et tile is released right after its m-tile is consumed
                for hh in heads:
                    ots = [
                        psot.tile([D + 1, cw], dt.float32, tag="ot", name=f"ot{c}")
                        for c, (ci, cw) in enumerate(CHUNKS)
                    ]
                    for mt in range(NMT):
                        for c, (ci, cw) in enumerate(CHUNKS):
                            nc.tensor.matmul(
                                ots[c][:],
                                vnat[mt][:, hh * 65:hh * 65 + D + 1],
                                ets[(hh, mt)][:, ci:ci + cw],
                                start=(mt == 0), stop=(mt == NMT - 1),
                            )
                    for c, (ci, cw) in enumerate(CHUNKS):
                        nc.vector.tensor_copy(oT_sb[hh][:, ci:ci + cw], ots[c][:])
                # merged n-tail pass: [128, 9*20] accumulators, 1 exp/head
                tacc = [
                    psot.tile([128, NMT * TLW], dt.float32, tag="ot", name=f"tl{i}")
                    for i in range(2)
                ]
                for mt in range(NMT):
                    m0 = mt * 128
                    for i in range(2):
                        nc.tensor.matmul(
                            tacc[i][:, mt * TLW:(mt + 1) * TLW],
                            kTt[rows[i]:rows[i] + 32, m0:m0 + 128],
                            qTt[rows[i]:rows[i] + 32, TLO:TLO + TLW],
                            start=True, stop=True, tile_position=(rows[i], 0),
                        )
                for i, hh in enumerate(heads):
                    ett = pext.tile([128, NMT * TLW], dt.bfloat16, tag="expt")
                    nc.scalar.activation(ett[:], tacc[i][:], act_exp)
                    ot_t = psot.tile([D + 1, TLW], dt.float32, tag="ot")
                    for mt in range(NMT):
                        nc.tensor.matmul(
                            ot_t[:],
                            vnat[mt][:, hh * 65:hh * 65 + D + 1],
                            ett[:, mt * TLW:(mt + 1) * TLW],
                            start=(mt == 0), stop=(mt == NMT - 1),
                        )
                    nc.vector.tensor_copy(oT_sb[hh][:, TLO:TLO + TLW], ot_t[:])

            def emit_stage(b, heads, oT_sb):
                # transpose back, normalize, stage o
                for hh in heads:
                    for nt, (n0, nsz) in enumerate(NTT):
                        tp2 = psot.tile([nsz, D + 1], dt.bfloat16, tag="ot")
                        nc.tensor.transpose(
                            tp2[:], oT_sb[hh][:, n0:n0 + nsz], id_sb[0:D + 1, 0:D + 1]
                        )
                        rcp = psm.tile([nsz, 1], dt.float32, tag="rcp")
                        nc.vector.reciprocal(rcp[:], tp2[:, D:D + 1])
                        onrm = psm.tile([nsz, D], dt.bfloat16, tag="onrm")
                        nc.vector.tensor_scalar(
                            onrm[:], tp2[:, 0:D], rcp[:], None, op0=alu.mult
                        )
                        nc.sync.dma_start(
                            o_nat[b, n0:n0 + nsz, hh * D:(hh + 1) * D], onrm[:]
                        )

            def emit_conv(b, ct, vc_tiles):
                # conv for this pair's channel-tile (bf16 DVE ops)
                cin = pcin.tile([128, 1024], dt.bfloat16, tag="cin")
                for hl in range(2):
                    src = v_dram[b, 2 * ct + hl, 0:1024, :].rearrange(
                        "(c n2) d -> c (n2 d)", c=64
                    )
                    nc.scalar.dma_start(cin[hl * 64:(hl + 1) * 64, :], src)
                u = pcv.tile([128, 1024], dt.bfloat16, tag="u")
                nc.vector.tensor_scalar(
                    u[:], cin[:], 1.0 / 6.0, 0.5, op0=alu.mult, op1=alu.add
                )
                nc.vector.tensor_scalar(
                    u[:], u[:], 1.0, 0.0, op0=alu.min, op1=alu.max
                )
                hs = pcv.tile([128, 32, 32], dt.bfloat16, tag="hs")
                nc.vector.tensor_tensor(
                    hs[:].rearrange("p a b -> p (a b)"), u[:], cin[:], op=alu.mult,
                )
                pad = pcv.tile([128, 34, 34], dt.bfloat16, tag="pad")
                nc.gpsimd.memset(pad[:], 0.0)
                nc.vector.tensor_copy(pad[:, 1:33, 1:33], hs[:])
                acc = pcv.tile([128, 1024], dt.bfloat16, tag="acc")
                t2 = pcv.tile([128, 1024], dt.bfloat16, tag="t2")
                for tap in range(9):
                    dy, dx = tap // 3, tap % 3
                    view = pad[:, dy:dy + 32, dx:dx + 32]
                    wsl = tapw_sb[:, ct * 9 + tap:ct * 9 + tap + 1]
                    if tap == 0:
                        nc.vector.tensor_scalar(
                            acc[:].rearrange("p (a b) -> p a b", a=32),
                            view, wsl, None, op0=alu.mult,
                        )
                    else:
                        nc.vector.tensor_scalar(
                            t2[:].rearrange("p (a b) -> p a b", a=32),
                            view, wsl, None, op0=alu.mult,
                        )
                        nc.vector.tensor_tensor(acc[:], acc[:], t2[:], op=alu.add)
                vct = pvc.tile([128, 1024], dt.bfloat16, tag="vc")
                nc.vector.tensor_tensor(vct[:], acc[:], cpb_sb[:, ct, :], op=alu.add)
                vc_tiles.append(vct)

            for b in range(BPC):
                vc_tiles = []
                for pr in range(4):
                    heads, rows, qTt, kTt, ets = emit_scores(b, pr)
                    oT_sb = {}
                    for hh in heads:
                        oT_sb[hh] = poTs.tile(
                            [D + 1, N], dt.bfloat16, tag="oTs", name=f"oTs{hh}"
                        )
                    emit_otail(b, pr, heads, rows, qTt, kTt, ets, oT_sb)
                    emit_stage(b, heads, oT_sb)
                    emit_conv(b, pr, vc_tiles)
                vc_all[b] = vc_tiles

            # ======== phase 3: proj + out per batch ========
            for b in range(BPC):
                vc_tiles = vc_all[b]
                O2v = o_nat[b].rearrange("n c -> (n c)").rearrange("(r m) -> r m", m=N)
                o2t = []
                for mt, (m0, msz) in enumerate(NTT):
                    t = po2.tile([msz, DH], dt.bfloat16, tag=f"o2t{mt}")
                    if msz == 128:
                        nc.sync.dma_start_transpose(t[:], O2v[:, m0:m0 + msz])
                    else:
                        nc.sync.dma_start(
                            t[:], O2v[:, m0:m0 + msz].rearrange("a b -> b a")
                        )
                    o2t.append(t)
                xo_sb = [
                    pxo.tile([128, 1024], dt.bfloat16, tag="xo", name=f"xo{rt}")
                    for rt in range(4)
                ]
                for rt in range(4):
                    for c, jc in enumerate((0, 512)):
                        ps = psot.tile([128, 512], dt.float32, tag="ot", name="xop")
                        for mt, (m0, msz) in enumerate(NTT):
                            nc.tensor.matmul(
                                ps[:],
                                o2t[mt][0:msz, rt * 128:(rt + 1) * 128],
                                wproj_sb[mt][0:msz, jc:jc + 512],
                                start=(mt == 0), stop=(mt == 8),
                            )
                        nc.vector.tensor_tensor(
                            xo_sb[rt][:, jc:jc + 512], ps[:],
                            vc_tiles[rt][:, jc:jc + 512], op=alu.add,
                        )

                for jt in range(8):
                    ps = psot.tile([128, 512], dt.float32, tag="ot", name="outp")
                    for rt in range(4):
                        nc.tensor.matmul(
                            ps[:],
                            xo_sb[rt][:, jt * 128:(jt + 1) * 128],
                            wout_sb[:, rt, :],
                            start=(rt == 0), stop=(rt == 3),
                        )
                    osb = posb.tile([128, DIM], dt.float32, tag="outsb")
                    nc.vector.tensor_tensor(osb[:], ps[:], obbc[:], op=alu.add)
                    nc.scalar.dma_start(out_ext[b, jt * 128:(jt + 1) * 128, :], osb[:])

    nc.compile()
    return nc


def _prep_weights(qkv_w, bn_gamma, bn_beta, bn_mean, bn_var,
                  conv_w, conv_b, proj_w, proj_b, out_w, out_b):
    s = bn_gamma / np.sqrt(bn_var + EPS)
    bias = bn_beta - bn_mean * s
    Wt = (qkv_w * s[:, None]).T.copy()
    bvec = bias.copy()
    scale = KD ** -0.5
    for hh in range(H):
        Wt[:, hh * 128:hh * 128 + KD] *= scale
        bvec[hh * 128:hh * 128 + KD] *= scale
    # scatter channels into 1032 layout (see _build head slicing):
    # cols 0:256 q tiles (4 heads/tile at 0,32,64,96), 256:512 k tiles,
    # 512:1032 v channels hh*65+j with a ones channel (zero weight, bias 1)
    Wn = np.zeros((DIM, 1032), Wt.dtype)
    bn = np.zeros(1032, bvec.dtype)
    for hh in range(H):
        qdst = (hh // 4) * 128 + (hh % 4) * 32
        kdst = 256 + (hh // 4) * 128 + (hh % 4) * 32
        vdst = 512 + hh * 65
        qsrc, ksrc, vsrc = hh * 128, hh * 128 + 32, hh * 128 + 64
        Wn[:, qdst:qdst + 32] = Wt[:, qsrc:qsrc + 32]
        bn[qdst:qdst + 32] = bvec[qsrc:qsrc + 32]
        Wn[:, kdst:kdst + 32] = Wt[:, ksrc:ksrc + 32]
        bn[kdst:kdst + 32] = bvec[ksrc:ksrc + 32]
        Wn[:, vdst:vdst + 64] = Wt[:, vsrc:vsrc + 64]
        bn[vdst:vdst + 64] = bvec[vsrc:vsrc + 64]
        bn[vdst + 64] = 1.0
    cpb_t = (conv_b.reshape(4, 128).T[:, :, None]
             + proj_b.reshape(1, 1, IMG))              # [128, 4, 1024]
    return {
        "wqkv": np.ascontiguousarray(Wn).astype(BF16),
        "bqkv": np.ascontiguousarray(bn[:512].reshape(4, 128).T).astype(np.float32),
        "bv": np.ascontiguousarray(bn[512:].reshape(1, 520)).astype(np.float32),
        "wproj": np.ascontiguousarray(proj_w.T).astype(BF16),
        "wout": np.ascontiguousarray(out_w.T).astype(BF16),
        "tapw": np.ascontiguousarray(
            conv_w[:, 0].reshape(4, 128, 9).transpose(1, 0, 2).reshape(128, 36)
        ).astype(np.float32),
        "cpb": np.ascontiguousarray(cpb_t).astype(BF16),
        "obp": out_b.reshape(1, DIM).astype(np.float32),
    }


def run(trace=False, tmpdir=None, **inputs):
    from concourse.bass_utils import run_bass_kernel_spmd

    if "nc" not in _cached:
        _cached["nc"] = _build()
    nc = _cached["nc"]

    w = _prep_weights(**{k: np.asarray(v) for k, v in inputs.items() if k != "x"})
    x = np.asarray(inputs["x"]).astype(BF16)
    in_maps = []
    for c in range(NCORES):
        m = dict(w)
        m["xs"] = np.ascontiguousarray(x[c * BPC:(c + 1) * BPC])
        in_maps.append(m)
    res = run_bass_kernel_spmd(
        nc, in_maps, core_ids=list(range(NCORES)), trace=trace, tmpdir=tmpdir
    )
    out = np.concatenate([np.asarray(r["out"]) for r in res.results], axis=0)
    return out.astype(np.float32), res.exec_time_ns


def kernel(**inputs):
    out, _ = run(trace=False, **inputs)
    return out


if __name__ == "__main__":
    print("building graph...")
    nc = _build()
    print("build OK:", len(nc.m.functions[0].allocations), "allocations")


# revision 27
# speedup vs baseline: 1.4085x; 1.0206x over previous
"""Trainium2 Bass kernel for nn_Attention_66546223284383.

Strategy: pure data-parallel over batch B=16 -> 2 batches per core x 8 cores.
Per core, per batch:
  qkvT = (BN-folded W)^T @ x^T           (h on partitions, n free)
  Attention m-dim padded 1044->1152 (zero k/v pad rows contribute
  exp(0)*0 = 0), so all 9 m-tiles are uniform 128 rows.
  q/k packed 4 heads per 128-partition tile at offsets {0,32,64,96};
  scores for 4 heads run CONCURRENTLY via PE row tiling
  (tile_position=(32i,0), K=32 each).
  n-chunk-major softmax: per (group, chunk<=512): per m-tile:
  4 row-tiled score MMs -> 4 psum banks -> exp on ACT -> 4 oT MMs
  ([v|1]^T @ exp, 65 rows: 64 o-dims + denominator) accumulating into
  4 more banks.  n-tail (1024:1044) handled in a merged pass: one
  [128, 9*20] accumulator + ONE exp per head.
  PE-transpose oT -> normalize (ACT Copy with per-partition reciprocal
  scale) -> stage o to DRAM (n, c) bf16.
  conv branch: v reflowed via DRAM to (channel, spatial), hardswish +
  9-tap depthwise conv on VectorE with per-partition tap weights.
  proj:     O2^T tiles read back via XBAR transpose DMA (handles the torch
            "raw reshape" (B,N,H,d)->(B,DH,N) as a flat re-chunk),
            xo = O2 @ proj_w^T + vc  (r on partitions, j free)
  out:      out = xo^T @ out_w^T + out_b -> (1024, 512) fp32.
All matmuls bf16 inputs with fp32 PSUM accumulation.
"""
import sys
import numpy as np

sys.path.insert(0, "/opt/trn_rl_repo")

import ml_dtypes  # noqa: E402

BF16 = ml_dtypes.bfloat16

KD, H, D, DH, DIM, IMG, S = 32, 8, 64, 512, 512, 1024, 32
N, B = 1044, 16
EPS = 1e-5
NCORES = 8
BPC = B // NCORES  # batches per core
MP = 1152          # padded attention m-dim (9 x 128)
NMT = 9            # m-tiles, all 128 rows
NTT = [(i * 128, 128) for i in range(8)] + [(1024, 20)]  # real-n tiles
CHUNKS = [(0, 512), (512, 512)]                          # big n chunks
TLO, TLW = 1024, 20                                      # n tail

_cached = {}


def _build():
    from concourse import bacc, tile
    import concourse.bass as bass
    import concourse.mybir as mybir
    from concourse.masks import make_identity

    dt = mybir.dt
    alu = mybir.AluOpType
    act_exp = mybir.ActivationFunctionType.Exp
    act_copy = mybir.ActivationFunctionType.Copy

    nc = bacc.Bacc(None, target_bir_lowering=False, debug=False)

    xs = nc.declare_dram_parameter("xs", [BPC, N, DIM], dt.bfloat16, isOutput=False)
    # wqkv cols: 0:256 = 2 q tiles (4 heads each at offsets 0/32/64/96),
    # 256:512 = 2 k tiles, 512:1032 = v channels hh*65+j (j==64 is a ones
    # channel: zero weights, bias 1 -> softmax denominator column)
    wqkv = nc.declare_dram_parameter("wqkv", [DIM, 1032], dt.bfloat16, isOutput=False)
    bqkv = nc.declare_dram_parameter("bqkv", [128, 4], dt.float32, isOutput=False)
    bv = nc.declare_dram_parameter("bv", [1, 520], dt.float32, isOutput=False)
    wproj = nc.declare_dram_parameter("wproj", [N, IMG], dt.bfloat16, isOutput=False)
    wout = nc.declare_dram_parameter("wout", [DH, DIM], dt.bfloat16, isOutput=False)
    tapw = nc.declare_dram_parameter("tapw", [128, 36], dt.float32, isOutput=False)
    cpb = nc.declare_dram_parameter("cpb", [128, 4, IMG], dt.bfloat16, isOutput=False)
    obp = nc.declare_dram_parameter("obp", [1, DIM], dt.float32, isOutput=False)
    out_ext = nc.declare_dram_parameter("out", [BPC, IMG, DIM], dt.float32, isOutput=True)

    o_nat = nc.dram_tensor("o_nat", [BPC, N, DH], dt.bfloat16)
    v_dram = nc.dram_tensor("v_dram", [BPC, H, N, D], dt.bfloat16)

    import contextlib
    with tile.TileContext(nc) as tc:
        with contextlib.ExitStack() as _st:
            ec = _st.enter_context
            pw = ec(tc.tile_pool(name="w", bufs=1))
            pxT = ec(tc.tile_pool(name="xT", bufs=5))
            pqk = ec(tc.tile_pool(name="qkvT", bufs=8))
            pvn = ec(tc.tile_pool(name="vnat", bufs=18))
            pexp = ec(tc.tile_pool(name="exp", bufs=18))
            pext = ec(tc.tile_pool(name="expt", bufs=1))
            poTs = ec(tc.tile_pool(name="oTs", bufs=4))
            psm = ec(tc.tile_pool(name="small", bufs=6))
            posb = ec(tc.tile_pool(name="osb", bufs=2))
            pcv = ec(tc.tile_pool(name="conv", bufs=1))
            pcin = ec(tc.tile_pool(name="cin", bufs=2))
            pvc = ec(tc.tile_pool(name="vc", bufs=7))
            po2 = ec(tc.tile_pool(name="o2t", bufs=1))
            pxo = ec(tc.tile_pool(name="xo", bufs=4))
            # PSUM budget (8 banks): sc 3x(128,1024)=6, ot 2x(65,512)=2
            # (qkv/vnat/proj/out ride the "ot" slots, chunk-serial)
            pssc = ec(tc.tile_pool(name="pssc", bufs=3, space=bass.MemorySpace.PSUM))
            psot = ec(tc.tile_pool(name="psot", bufs=2, space=bass.MemorySpace.PSUM))
            # ---- constants / weights ----
            id_sb = pw.tile([128, 128], dt.bfloat16, tag="id")
            make_identity(nc, id_sb[:])
            wqkv_sb = pw.tile([128, 4, 1032], dt.bfloat16, tag="wqkv")
            nc.sync.dma_start(wqkv_sb[:], wqkv[:].rearrange("(k p) h -> p k h", p=128))
            bqkv_sb = pw.tile([128, 4], dt.float32, tag="bqkv")
            nc.sync.dma_start(bqkv_sb[:], bqkv[:])
            bv_sb = pw.tile([1, 520], dt.float32, tag="bv")
            nc.sync.dma_start(bv_sb[:], bv[:])
            bvbc = pw.tile([128, 520], dt.float32, tag="bvbc")
            nc.gpsimd.partition_broadcast(bvbc[:], bv_sb[:])
            wproj_sb = []
            for mt, (m0, msz) in enumerate(NTT):
                t = pw.tile([msz, 1024], dt.bfloat16, tag=f"wproj{mt}")
                nc.sync.dma_start(t[:], wproj[m0:m0 + msz, :])
                wproj_sb.append(t)
            wout_sb = pw.tile([128, 4, DIM], dt.bfloat16, tag="wout")
            nc.sync.dma_start(wout_sb[:], wout[:].rearrange("(k p) c -> p k c", p=128))
            tapw_sb = pw.tile([128, 36], dt.float32, tag="tapw")
            nc.sync.dma_start(tapw_sb[:], tapw[:])
            cpb_sb = pw.tile([128, 4, IMG], dt.bfloat16, tag="cpb")
            nc.sync.dma_start(cpb_sb[:], cpb[:])
            ob_sb = pw.tile([1, DIM], dt.float32, tag="ob")
            nc.sync.dma_start(ob_sb[:], obp[:])
            obbc = pw.tile([128, DIM], dt.float32, tag="obbc")
            nc.gpsimd.partition_broadcast(obbc[:], ob_sb[:])

            qkvT_all, vnat_all, vc_all = {}, {}, {}

            # ======== phase 1: x^T + qk + v for BOTH batches (PE-dense) ========
            for b in range(BPC):
                xT = []
                for cb4 in range(4):
                    t = pxT.tile([128, N], dt.bfloat16, tag="xT")
                    c0 = cb4 * 128
                    nc.sync.dma_start_transpose(t[:, 0:1040], xs[b, 0:1040, c0:c0 + 128])
                    nc.sync.dma_start(
                        t[:, 1040:N], xs[b, 1040:N, c0:c0 + 128].rearrange("a b -> b a")
                    )
                    xT.append(t)

                # 4 qk tiles: t 0-1 q heads (4/tile @ 0,32,64,96), 2-3 k heads
                # k tiles padded to MP cols; pad region zeroed (never biased)
                qkvT = []
                for t4 in range(4):
                    is_k = t4 >= 2
                    t = pqk.tile([128, MP], dt.bfloat16, tag="qkvT")
                    if is_k:
                        nc.gpsimd.memset(t[:, N:MP], 0.0)
                    for (ci, cw) in ((0, 512), (512, 512), (1024, 20)):
                        ps = psot.tile([128, 512], dt.float32, tag="ot", name="qkp")
                        for kc in range(4):
                            nc.tensor.matmul(
                                ps[:, 0:cw],
                                wqkv_sb[:, kc, t4 * 128:(t4 + 1) * 128],
                                xT[kc][:, ci:ci + cw],
                                start=(kc == 0), stop=(kc == 3),
                            )
                        nc.vector.tensor_scalar(
                            t[:, ci:ci + cw], ps[:, 0:cw],
                            bqkv_sb[:, t4:t4 + 1], None, op0=alu.add,
                        )
                    qkvT.append(t)
                qkvT_all[b] = qkvT

                # v directly in (n, 8*65) layout with ones channels; m-tile 8
                # padded with zero rows (20 real)
                vnat = []
                for mt in range(NMT):
                    m0 = mt * 128
                    mreal = 128 if mt < 8 else 20
                    t = pvn.tile([128, 520], dt.bfloat16, tag="vnat")
                    if mreal < 128:
                        nc.gpsimd.memset(t[:], 0.0)
                    for c in range(2):
                        ps = psot.tile([128, 512], dt.float32, tag="ot", name="vnp")
                        for kc in range(4):
                            nc.tensor.matmul(
                                ps[0:mreal, 0:260],
                                xT[kc][:, m0:m0 + mreal],
                                wqkv_sb[:, kc, 512 + c * 260:512 + (c + 1) * 260],
                                start=(kc == 0), stop=(kc == 3),
                            )
                        nc.vector.tensor_tensor(
                            t[0:mreal, c * 260:(c + 1) * 260], ps[0:mreal, 0:260],
                            bvbc[0:mreal, c * 260:(c + 1) * 260], op=alu.add,
                        )
                    # stage all 8 heads' v in ONE DMA on the gpsimd queue
                    nc.gpsimd.dma_start(
                        v_dram[b].rearrange("h n d -> n h d")[m0:m0 + mreal],
                        t[0:mreal].rearrange("p (h dd) -> p h dd", h=8)[:, :, 0:D],
                    )
                    vnat.append(t)
                vnat_all[b] = vnat

            # ======== phase 2: attention, head pairs, big-exp per (h,mt) ========
            def emit_scores(b, pr):
                qkvT, vnat = qkvT_all[b], vnat_all[b]
                t4, half = pr // 2, pr % 2
                qTt, kTt = qkvT[t4], qkvT[2 + t4]
                rows = (64 * half, 64 * half + 32)
                heads = [t4 * 4 + half * 2, t4 * 4 + half * 2 + 1]
                ets = {}
                # scores + exp: one [128,1024] 2-bank tile + one exp per
                # (head, mt); the pair's score MMs run concurrently via
                # PE row tiling
                for mt in range(NMT):
                    m0 = mt * 128
                    scs = [
                        pssc.tile([128, 1024], dt.float32, tag="sc", name=f"sc{i}")
                        for i in range(2)
                    ]
                    for (ci, cw) in CHUNKS:
                        for i in range(2):
                            nc.tensor.matmul(
                                scs[i][:, ci:ci + cw],
                                kTt[rows[i]:rows[i] + 32, m0:m0 + 128],
                                qTt[rows[i]:rows[i] + 32, ci:ci + cw],
                                start=True, stop=True, tile_position=(rows[i], 0),
                            )
                    for i, hh in enumerate(heads):
                        et = pexp.tile([128, 1024], dt.bfloat16, tag="exp")
                        nc.scalar.activation(et[:], scs[i][:], act_exp)
                        ets[(i, mt)] = et
                return heads, rows, qTt, kTt, ets

            def emit_otail(b, pr, heads, rows, qTt, kTt, ets, oT_sb):
                vnat = vnat_all[b]
                # oT accumulation per head, both n-chunks per m-tile; et tiles
                # hold (head A | head B) halves and release after B's pass
                for i, hh in enumerate(heads):
                    ots = [
                        psot.tile([D + 1, cw], dt.float32, tag="ot", name=f"ot{c}")
                        for c, (ci, cw) in enumerate(CHUNKS)
                    ]
                    for mt in range(NMT):
                        for c, (ci, cw) in enumerate(CHUNKS):
                            nc.tensor.matmul(
                                ots[c][:],
                                vnat[mt][:, hh * 65:hh * 65 + D + 1],
                                ets[(i, mt)][:, ci:ci + cw],
                                start=(mt == 0), stop=(mt == NMT - 1),
                            )
                    for c, (ci, cw) in enumerate(CHUNKS):
                        nc.vector.tensor_copy(oT_sb[hh][:, ci:ci + cw], ots[c][:])
                # merged n-tail pass: [128, 9*20] accumulators, 1 exp/head
                tacc = [
                    psot.tile([128, NMT * TLW], dt.float32, tag="ot", name=f"tl{i}")
                    for i in range(2)
                ]
                for mt in range(NMT):
                    m0 = mt * 128
                    for i in range(2):
                        nc.tensor.matmul(
                            tacc[i][:, mt * TLW:(mt + 1) * TLW],
                            kTt[rows[i]:rows[i] + 32, m0:m0 + 128],
                            qTt[rows[i]:rows[i] + 32, TLO:TLO + TLW],
                            start=True, stop=True, tile_position=(rows[i], 0),
                        )
                for i, hh in enumerate(heads):
                    ett = pext.tile([128, NMT * TLW], dt.bfloat16, tag="expt")
                    nc.scalar.activation(ett[:], tacc[i][:], act_exp)
                    ot_t = psot.tile([D + 1, TLW], dt.float32, tag="ot")
                    for mt in range(NMT):
                        nc.tensor.matmul(
                            ot_t[:],
                            vnat[mt][:, hh * 65:hh * 65 + D + 1],
                            ett[:, mt * TLW:(mt + 1) * TLW],
                            start=(mt == 0), stop=(mt == NMT - 1),
                        )
                    nc.vector.tensor_copy(oT_sb[hh][:, TLO:TLO + TLW], ot_t[:])

            def emit_stage(b, heads, oT_sb):
                # transpose back, normalize, stage o
                for hh in heads:
                    for nt, (n0, nsz) in enumerate(NTT):
                        tp2 = psot.tile([nsz, D + 1], dt.bfloat16, tag="ot")
                        nc.tensor.transpose(
                            tp2[:], oT_sb[hh][:, n0:n0 + nsz], id_sb[0:D + 1, 0:D + 1]
                        )
                        rcp = psm.tile([nsz, 1], dt.float32, tag="rcp")
                        nc.vector.reciprocal(rcp[:], tp2[:, D:D + 1])
                        onrm = psm.tile([nsz, D], dt.bfloat16, tag="onrm")
                        nc.vector.tensor_scalar(
                            onrm[:], tp2[:, 0:D], rcp[:], None, op0=alu.mult
                        )
                        nc.sync.dma_start(
                            o_nat[b, n0:n0 + nsz, hh * D:(hh + 1) * D], onrm[:]
                        )

            def emit_conv(b, ct, vc_tiles):
                # conv for this pair's channel-tile (bf16 DVE ops)
                cin = pcin.tile([128, 1024], dt.bfloat16, tag="cin")
                for hl in range(2):
                    src = v_dram[b, 2 * ct + hl, 0:1024, :].rearrange(
                        "(c n2) d -> c (n2 d)", c=64
                    )
                    nc.scalar.dma_start(cin[hl * 64:(hl + 1) * 64, :], src)
                u = pcv.tile([128, 1024], dt.bfloat16, tag="u")
                nc.vector.tensor_scalar(
                    u[:], cin[:], 1.0 / 6.0, 0.5, op0=alu.mult, op1=alu.add
                )
                nc.vector.tensor_scalar(
                    u[:], u[:], 1.0, 0.0, op0=alu.min, op1=alu.max
                )
                hs = pcv.tile([128, 32, 32], dt.bfloat16, tag="hs")
                nc.vector.tensor_tensor(
                    hs[:].rearrange("p a b -> p (a b)"), u[:], cin[:], op=alu.mult,
                )
                pad = pcv.tile([128, 34, 34], dt.bfloat16, tag="pad")
                nc.gpsimd.memset(pad[:], 0.0)
                nc.vector.tensor_copy(pad[:, 1:33, 1:33], hs[:])
                acc = pcv.tile([128, 1024], dt.bfloat16, tag="acc")
                t2 = pcv.tile([128, 1024], dt.bfloat16, tag="t2")
                for tap in range(9):
                    dy, dx = tap // 3, tap % 3
                    view = pad[:, dy:dy + 32, dx:dx + 32]
                    wsl = tapw_sb[:, ct * 9 + tap:ct * 9 + tap + 1]
                    if tap == 0:
                        nc.vector.tensor_scalar(
                            acc[:].rearrange("p (a b) -> p a b", a=32),
                            view, wsl, None, op0=alu.mult,
                        )
                    else:
                        nc.vector.tensor_scalar(
                            t2[:].rearrange("p (a b) -> p a b", a=32),
                            view, wsl, None, op0=alu.mult,
                        )
                        nc.vector.tensor_tensor(acc[:], acc[:], t2[:], op=alu.add)
                vct = pvc.tile([128, 1024], dt.bfloat16, tag="vc")
                nc.vector.tensor_tensor(vct[:], acc[:], cpb_sb[:, ct, :], op=alu.add)
                vc_tiles.append(vct)

            for b in range(BPC):
                vc_tiles = []
                for pr in range(4):
                    heads, rows, qTt, kTt, ets = emit_scores(b, pr)
                    oT_sb = {}
                    for hh in heads:
                        oT_sb[hh] = poTs.tile(
                            [D + 1, N], dt.bfloat16, tag="oTs", name=f"oTs{hh}"
                        )
                    emit_otail(b, pr, heads, rows, qTt, kTt, ets, oT_sb)
                    emit_stage(b, heads, oT_sb)
                    emit_conv(b, pr, vc_tiles)
                vc_all[b] = vc_tiles

            # ======== phase 3: proj + out per batch ========
            for b in range(BPC):
                vc_tiles = vc_all[b]
                O2v = o_nat[b].rearrange("n c -> (n c)").rearrange("(r m) -> r m", m=N)
                o2t = []
                for mt, (m0, msz) in enumerate(NTT):
                    t = po2.tile([msz, DH], dt.bfloat16, tag=f"o2t{mt}")
                    if msz == 128:
                        nc.sync.dma_start_transpose(t[:], O2v[:, m0:m0 + msz])
                    else:
                        nc.sync.dma_start(
                            t[:], O2v[:, m0:m0 + msz].rearrange("a b -> b a")
                        )
                    o2t.append(t)
                xo_sb = [
                    pxo.tile([128, 1024], dt.bfloat16, tag="xo", name=f"xo{rt}")
                    for rt in range(4)
                ]
                for rt in range(4):
                    for c, jc in enumerate((0, 512)):
                        ps = psot.tile([128, 512], dt.float32, tag="ot", name="xop")
                        for mt, (m0, msz) in enumerate(NTT):
                            nc.tensor.matmul(
                                ps[:],
                                o2t[mt][0:msz, rt * 128:(rt + 1) * 128],
                                wproj_sb[mt][0:msz, jc:jc + 512],
                                start=(mt == 0), stop=(mt == 8),
                            )
                        nc.vector.tensor_tensor(
                            xo_sb[rt][:, jc:jc + 512], ps[:],
                            vc_tiles[rt][:, jc:jc + 512], op=alu.add,
                        )

                for jt in range(8):
                    ps = psot.tile([128, 512], dt.float32, tag="ot", name="outp")
                    for rt in range(4):
                        nc.tensor.matmul(
                            ps[:],
                            xo_sb[rt][:, jt * 128:(jt + 1) * 128],
                            wout_sb[:, rt, :],
                            start=(rt == 0), stop=(rt == 3),
                        )
                    osb = posb.tile([128, DIM], dt.float32, tag="outsb")
                    nc.vector.tensor_tensor(osb[:], ps[:], obbc[:], op=alu.add)
                    nc.scalar.dma_start(out_ext[b, jt * 128:(jt + 1) * 128, :], osb[:])

    nc.compile()
    return nc


def _prep_weights(qkv_w, bn_gamma, bn_beta, bn_mean, bn_var,
                  conv_w, conv_b, proj_w, proj_b, out_w, out_b):
    s = bn_gamma / np.sqrt(bn_var + EPS)
    bias = bn_beta - bn_mean * s
    Wt = (qkv_w * s[:, None]).T.copy()
    bvec = bias.copy()
    scale = KD ** -0.5
    for hh in range(H):
        Wt[:, hh * 128:hh * 128 + KD] *= scale
        bvec[hh * 128:hh * 128 + KD] *= scale
    # scatter channels into 1032 layout (see _build head slicing):
    # cols 0:256 q tiles (4 heads/tile at 0,32,64,96), 256:512 k tiles,
    # 512:1032 v channels hh*65+j with a ones channel (zero weight, bias 1)
    Wn = np.zeros((DIM, 1032), Wt.dtype)
    bn = np.zeros(1032, bvec.dtype)
    for hh in range(H):
        qdst = (hh // 4) * 128 + (hh % 4) * 32
        kdst = 256 + (hh // 4) * 128 + (hh % 4) * 32
        vdst = 512 + hh * 65
        qsrc, ksrc, vsrc = hh * 128, hh * 128 + 32, hh * 128 + 64
        Wn[:, qdst:qdst + 32] = Wt[:, qsrc:qsrc + 32]
        bn[qdst:qdst + 32] = bvec[qsrc:qsrc + 32]
        Wn[:, kdst:kdst + 32] = Wt[:, ksrc:ksrc + 32]
        bn[kdst:kdst + 32] = bvec[ksrc:ksrc + 32]
        Wn[:, vdst:vdst + 64] = Wt[:, vsrc:vsrc + 64]
        bn[vdst:vdst + 64] = bvec[vsrc:vsrc + 64]
        bn[vdst + 64] = 1.0
    cpb_t = (conv_b.reshape(4, 128).T[:, :, None]
             + proj_b.reshape(1, 1, IMG))              # [128, 4, 1024]
    return {
        "wqkv": np.ascontiguousarray(Wn).astype(BF16),
        "bqkv": np.ascontiguousarray(bn[:512].reshape(4, 128).T).astype(np.float32),
        "bv": np.ascontiguousarray(bn[512:].reshape(1, 520)).astype(np.float32),
        "wproj": np.ascontiguousarray(proj_w.T).astype(BF16),
        "wout": np.ascontiguousarray(out_w.T).astype(BF16),
        "tapw": np.ascontiguousarray(
            conv_w[:, 0].reshape(4, 128, 9).transpose(1, 0, 2).reshape(128, 36)
        ).astype(np.float32),
        "cpb": np.ascontiguousarray(cpb_t).astype(BF16),
        "obp": out_b.reshape(1, DIM).astype(np.float32),
    }


def run(trace=False, tmpdir=None, **inputs):
    from concourse.bass_utils import run_bass_kernel_spmd

    if "nc" not in _cached:
        _cached["nc"] = _build()
    nc = _cached["nc"]

    w = _prep_weights(**{k: np.asarray(v) for k, v in inputs.items() if k != "x"})
    x = np.asarray(inputs["x"]).astype(BF16)
    in_maps = []
    for c in range(NCORES):
        m = dict(w)
        m["xs"] = np.ascontiguousarray(x[c * BPC:(c + 1) * BPC])
        in_maps.append(m)
    res = run_bass_kernel_spmd(
        nc, in_maps, core_ids=list(range(NCORES)), trace=trace, tmpdir=tmpdir
    )
    out = np.concatenate([np.asarray(r["out"]) for r in res.results], axis=0)
    return out.astype(np.float32), res.exec_time_ns


def kernel(**inputs):
    out, _ = run(trace=False, **inputs)
    return out


if __name__ == "__main__":
    print("building graph...")
    nc = _build()
    print("build OK:", len(nc.m.functions[0].allocations), "allocations")


# revision 28
# speedup vs baseline: 1.4418x; 1.0236x over previous
"""Trainium2 Bass kernel for nn_Attention_66546223284383.

Strategy: pure data-parallel over batch B=16 -> 2 batches per core x 8 cores.
Per core, per batch:
  qkvT = (BN-folded W)^T @ x^T           (h on partitions, n free)
  Attention m-dim padded 1044->1152 (zero k/v pad rows contribute
  exp(0)*0 = 0), so all 9 m-tiles are uniform 128 rows.
  q/k packed 4 heads per 128-partition tile at offsets {0,32,64,96};
  scores for 4 heads run CONCURRENTLY via PE row tiling
  (tile_position=(32i,0), K=32 each).
  n-chunk-major softmax: per (group, chunk<=512): per m-tile:
  4 row-tiled score MMs -> 4 psum banks -> exp on ACT -> 4 oT MMs
  ([v|1]^T @ exp, 65 rows: 64 o-dims + denominator) accumulating into
  4 more banks.  n-tail (1024:1044) handled in a merged pass: one
  [128, 9*20] accumulator + ONE exp per head.
  PE-transpose oT -> normalize (ACT Copy with per-partition reciprocal
  scale) -> stage o to DRAM (n, c) bf16.
  conv branch: v reflowed via DRAM to (channel, spatial), hardswish +
  9-tap depthwise conv on VectorE with per-partition tap weights.
  proj:     O2^T tiles read back via XBAR transpose DMA (handles the torch
            "raw reshape" (B,N,H,d)->(B,DH,N) as a flat re-chunk),
            xo = O2 @ proj_w^T + vc  (r on partitions, j free)
  out:      out = xo^T @ out_w^T + out_b -> (1024, 512) fp32.
All matmuls bf16 inputs with fp32 PSUM accumulation.
"""
import sys
import numpy as np

sys.path.insert(0, "/opt/trn_rl_repo")

import ml_dtypes  # noqa: E402

BF16 = ml_dtypes.bfloat16

KD, H, D, DH, DIM, IMG, S = 32, 8, 64, 512, 512, 1024, 32
N, B = 1044, 16
EPS = 1e-5
NCORES = 8
BPC = B // NCORES  # batches per core
MP = 1152          # padded attention m-dim (9 x 128)
NMT = 9            # m-tiles, all 128 rows
NTT = [(i * 128, 128) for i in range(8)] + [(1024, 20)]  # real-n tiles
CHUNKS = [(0, 512), (512, 512)]                          # big n chunks
TLO, TLW = 1024, 20                                      # n tail

_cached = {}


def _build():
    from concourse import bacc, tile
    import concourse.bass as bass
    import concourse.mybir as mybir
    from concourse.masks import make_identity

    dt = mybir.dt
    alu = mybir.AluOpType
    act_exp = mybir.ActivationFunctionType.Exp
    act_copy = mybir.ActivationFunctionType.Copy

    nc = bacc.Bacc(None, target_bir_lowering=False, debug=False)

    xs = nc.declare_dram_parameter("xs", [BPC, N, DIM], dt.bfloat16, isOutput=False)
    # wqkv cols: 0:256 = 2 q tiles (4 heads each at offsets 0/32/64/96),
    # 256:512 = 2 k tiles, 512:1032 = v channels hh*65+j (j==64 is a ones
    # channel: zero weights, bias 1 -> softmax denominator column)
    wqkv = nc.declare_dram_parameter("wqkv", [DIM, 1032], dt.bfloat16, isOutput=False)
    bqkv = nc.declare_dram_parameter("bqkv", [128, 4], dt.float32, isOutput=False)
    bv = nc.declare_dram_parameter("bv", [1, 520], dt.float32, isOutput=False)
    wproj = nc.declare_dram_parameter("wproj", [N, IMG], dt.bfloat16, isOutput=False)
    wout = nc.declare_dram_parameter("wout", [DH, DIM], dt.bfloat16, isOutput=False)
    tapw = nc.declare_dram_parameter("tapw", [128, 36], dt.float32, isOutput=False)
    cpb = nc.declare_dram_parameter("cpb", [128, 4, IMG], dt.bfloat16, isOutput=False)
    obp = nc.declare_dram_parameter("obp", [1, DIM], dt.float32, isOutput=False)
    out_ext = nc.declare_dram_parameter("out", [BPC, IMG, DIM], dt.float32, isOutput=True)

    o_nat = nc.dram_tensor("o_nat", [BPC, N, DH], dt.bfloat16)
    v_dram = nc.dram_tensor("v_dram", [BPC, H, N, D], dt.bfloat16)

    import contextlib
    with tile.TileContext(nc) as tc:
        with contextlib.ExitStack() as _st:
            ec = _st.enter_context
            pw = ec(tc.tile_pool(name="w", bufs=1))
            pxT = ec(tc.tile_pool(name="xT", bufs=5))
            pqk = ec(tc.tile_pool(name="qkvT", bufs=8))
            pvn = ec(tc.tile_pool(name="vnat", bufs=18))
            pexp = ec(tc.tile_pool(name="exp", bufs=18))
            pext = ec(tc.tile_pool(name="expt", bufs=1))
            poTs = ec(tc.tile_pool(name="oTs", bufs=4))
            psm = ec(tc.tile_pool(name="small", bufs=6))
            posb = ec(tc.tile_pool(name="osb", bufs=2))
            pcv = ec(tc.tile_pool(name="conv", bufs=1))
            pcin = ec(tc.tile_pool(name="cin", bufs=2))
            pvc = ec(tc.tile_pool(name="vc", bufs=7))
            po2 = ec(tc.tile_pool(name="o2t", bufs=1))
            pxo = ec(tc.tile_pool(name="xo", bufs=4))
            # PSUM budget (8 banks): sc 3x(128,1024)=6, ot 2x(65,512)=2
            # (qkv/vnat/proj/out ride the "ot" slots, chunk-serial)
            pssc = ec(tc.tile_pool(name="pssc", bufs=3, space=bass.MemorySpace.PSUM))
            psot = ec(tc.tile_pool(name="psot", bufs=2, space=bass.MemorySpace.PSUM))
            # ---- constants / weights ----
            id_sb = pw.tile([128, 128], dt.bfloat16, tag="id")
            make_identity(nc, id_sb[:])
            wqkv_sb = pw.tile([128, 4, 1032], dt.bfloat16, tag="wqkv")
            nc.sync.dma_start(wqkv_sb[:], wqkv[:].rearrange("(k p) h -> p k h", p=128))
            bqkv_sb = pw.tile([128, 4], dt.float32, tag="bqkv")
            nc.sync.dma_start(bqkv_sb[:], bqkv[:])
            bv_sb = pw.tile([1, 520], dt.float32, tag="bv")
            nc.sync.dma_start(bv_sb[:], bv[:])
            bvbc = pw.tile([128, 520], dt.float32, tag="bvbc")
            nc.gpsimd.partition_broadcast(bvbc[:], bv_sb[:])
            wproj_sb = []
            for mt, (m0, msz) in enumerate(NTT):
                t = pw.tile([msz, 1024], dt.bfloat16, tag=f"wproj{mt}")
                nc.sync.dma_start(t[:], wproj[m0:m0 + msz, :])
                wproj_sb.append(t)
            wout_sb = pw.tile([128, 4, DIM], dt.bfloat16, tag="wout")
            nc.sync.dma_start(wout_sb[:], wout[:].rearrange("(k p) c -> p k c", p=128))
            tapw_sb = pw.tile([128, 36], dt.float32, tag="tapw")
            nc.sync.dma_start(tapw_sb[:], tapw[:])
            cpb_sb = pw.tile([128, 4, IMG], dt.bfloat16, tag="cpb")
            nc.sync.dma_start(cpb_sb[:], cpb[:])
            ob_sb = pw.tile([1, DIM], dt.float32, tag="ob")
            nc.sync.dma_start(ob_sb[:], obp[:])
            obbc = pw.tile([128, DIM], dt.float32, tag="obbc")
            nc.gpsimd.partition_broadcast(obbc[:], ob_sb[:])

            qkvT_all, vnat_all, vc_all = {}, {}, {}

            # ======== phase 1: x^T + qk + v for BOTH batches (PE-dense) ========
            for b in range(BPC):
                xT = []
                for cb4 in range(4):
                    t = pxT.tile([128, N], dt.bfloat16, tag="xT")
                    c0 = cb4 * 128
                    nc.sync.dma_start_transpose(t[:, 0:1040], xs[b, 0:1040, c0:c0 + 128])
                    nc.sync.dma_start(
                        t[:, 1040:N], xs[b, 1040:N, c0:c0 + 128].rearrange("a b -> b a")
                    )
                    xT.append(t)

                # 4 qk tiles: t 0-1 q heads (4/tile @ 0,32,64,96), 2-3 k heads
                # k tiles padded to MP cols; pad region zeroed (never biased)
                qkvT = []
                for t4 in range(4):
                    is_k = t4 >= 2
                    t = pqk.tile([128, MP], dt.bfloat16, tag="qkvT")
                    if is_k:
                        nc.gpsimd.memset(t[:, N:MP], 0.0)
                    for (ci, cw) in ((0, 512), (512, 512), (1024, 20)):
                        ps = psot.tile([128, 512], dt.float32, tag="ot", name="qkp")
                        for kc in range(4):
                            nc.tensor.matmul(
                                ps[:, 0:cw],
                                wqkv_sb[:, kc, t4 * 128:(t4 + 1) * 128],
                                xT[kc][:, ci:ci + cw],
                                start=(kc == 0), stop=(kc == 3),
                            )
                        nc.vector.tensor_scalar(
                            t[:, ci:ci + cw], ps[:, 0:cw],
                            bqkv_sb[:, t4:t4 + 1], None, op0=alu.add,
                        )
                    qkvT.append(t)
                qkvT_all[b] = qkvT

                # v directly in (n, 8*65) layout with ones channels; m-tile 8
                # padded with zero rows (20 real)
                vnat = []
                for mt in range(NMT):
                    m0 = mt * 128
                    mreal = 128 if mt < 8 else 20
                    t = pvn.tile([128, 520], dt.bfloat16, tag="vnat")
                    if mreal < 128:
                        nc.gpsimd.memset(t[:], 0.0)
                    for c in range(2):
                        ps = psot.tile([128, 512], dt.float32, tag="ot", name="vnp")
                        for kc in range(4):
                            nc.tensor.matmul(
                                ps[0:mreal, 0:260],
                                xT[kc][:, m0:m0 + mreal],
                                wqkv_sb[:, kc, 512 + c * 260:512 + (c + 1) * 260],
                                start=(kc == 0), stop=(kc == 3),
                            )
                        nc.vector.tensor_tensor(
                            t[0:mreal, c * 260:(c + 1) * 260], ps[0:mreal, 0:260],
                            bvbc[0:mreal, c * 260:(c + 1) * 260], op=alu.add,
                        )
                    # stage all 8 heads' v in ONE DMA on the gpsimd queue
                    nc.gpsimd.dma_start(
                        v_dram[b].rearrange("h n d -> n h d")[m0:m0 + mreal],
                        t[0:mreal].rearrange("p (h dd) -> p h dd", h=8)[:, :, 0:D],
                    )
                    vnat.append(t)
                vnat_all[b] = vnat

            # ======== phase 2: attention, head pairs, big-exp per (h,mt) ========
            def emit_scores(b, pr):
                qkvT, vnat = qkvT_all[b], vnat_all[b]
                t4, half = pr // 2, pr % 2
                qTt, kTt = qkvT[t4], qkvT[2 + t4]
                rows = (64 * half, 64 * half + 32)
                heads = [t4 * 4 + half * 2, t4 * 4 + half * 2 + 1]
                ets = {}
                # scores + exp: one [128,1024] 2-bank tile + one exp per
                # (head, mt); the pair's score MMs run concurrently via
                # PE row tiling
                for mt in range(NMT):
                    m0 = mt * 128
                    for c, (ci, cw) in enumerate(CHUNKS):
                        sc = pssc.tile([128, 1024], dt.float32, tag="sc")
                        for i in range(2):
                            nc.tensor.matmul(
                                sc[:, i * 512:i * 512 + cw],
                                kTt[rows[i]:rows[i] + 32, m0:m0 + 128],
                                qTt[rows[i]:rows[i] + 32, ci:ci + cw],
                                start=True, stop=True, tile_position=(rows[i], 0),
                            )
                        et = pexp.tile([128, 1024], dt.bfloat16, tag="exp")
                        nc.scalar.activation(et[:], sc[:], act_exp)
                        ets[(c, mt)] = et
                return heads, rows, qTt, kTt, ets

            def emit_otail(b, pr, heads, rows, qTt, kTt, ets, oT_sb):
                vnat = vnat_all[b]
                # oT accumulation per head, both n-chunks per m-tile; et tiles
                # hold (head A | head B) halves and release after B's pass
                for i, hh in enumerate(heads):
                    ots = [
                        psot.tile([D + 1, cw], dt.float32, tag="ot", name=f"ot{c}")
                        for c, (ci, cw) in enumerate(CHUNKS)
                    ]
                    for mt in range(NMT):
                        for c, (ci, cw) in enumerate(CHUNKS):
                            nc.tensor.matmul(
                                ots[c][:],
                                vnat[mt][:, hh * 65:hh * 65 + D + 1],
                                ets[(c, mt)][:, i * 512:i * 512 + cw],
                                start=(mt == 0), stop=(mt == NMT - 1),
                            )
                    for c, (ci, cw) in enumerate(CHUNKS):
                        nc.vector.tensor_copy(oT_sb[hh][:, ci:ci + cw], ots[c][:])
                # merged n-tail pass: [128, 9*20] accumulators, 1 exp/head
                tacc = [
                    psot.tile([128, NMT * TLW], dt.float32, tag="ot", name=f"tl{i}")
                    for i in range(2)
                ]
                for mt in range(NMT):
                    m0 = mt * 128
                    for i in range(2):
                        nc.tensor.matmul(
                            tacc[i][:, mt * TLW:(mt + 1) * TLW],
                            kTt[rows[i]:rows[i] + 32, m0:m0 + 128],
                            qTt[rows[i]:rows[i] + 32, TLO:TLO + TLW],
                            start=True, stop=True, tile_position=(rows[i], 0),
                        )
                for i, hh in enumerate(heads):
                    ett = pext.tile([128, NMT * TLW], dt.bfloat16, tag="expt")
                    nc.scalar.activation(ett[:], tacc[i][:], act_exp)
                    ot_t = psot.tile([D + 1, TLW], dt.float32, tag="ot")
                    for mt in range(NMT):
                        nc.tensor.matmul(
                            ot_t[:],
                            vnat[mt][:, hh * 65:hh * 65 + D + 1],
                            ett[:, mt * TLW:(mt + 1) * TLW],
                            start=(mt == 0), stop=(mt == NMT - 1),
                        )
                    nc.vector.tensor_copy(oT_sb[hh][:, TLO:TLO + TLW], ot_t[:])

            def emit_stage(b, heads, oT_sb):
                # transpose back, normalize, stage o
                for hh in heads:
                    for nt, (n0, nsz) in enumerate(NTT):
                        tp2 = psot.tile([nsz, D + 1], dt.bfloat16, tag="ot")
                        nc.tensor.transpose(
                            tp2[:], oT_sb[hh][:, n0:n0 + nsz], id_sb[0:D + 1, 0:D + 1]
                        )
                        rcp = psm.tile([nsz, 1], dt.float32, tag="rcp")
                        nc.vector.reciprocal(rcp[:], tp2[:, D:D + 1])
                        onrm = psm.tile([nsz, D], dt.bfloat16, tag="onrm")
                        nc.vector.tensor_scalar(
                            onrm[:], tp2[:, 0:D], rcp[:], None, op0=alu.mult
                        )
                        nc.sync.dma_start(
                            o_nat[b, n0:n0 + nsz, hh * D:(hh + 1) * D], onrm[:]
                        )

            def emit_conv(b, ct, vc_tiles):
                # conv for this pair's channel-tile (bf16 DVE ops)
                cin = pcin.tile([128, 1024], dt.bfloat16, tag="cin")
                for hl in range(2):
                    src = v_dram[b, 2 * ct + hl, 0:1024, :].rearrange(
                        "(c n2) d -> c (n2 d)", c=64
                    )
                    nc.scalar.dma_start(cin[hl * 64:(hl + 1) * 64, :], src)
                u = pcv.tile([128, 1024], dt.bfloat16, tag="u")
                nc.vector.tensor_scalar(
                    u[:], cin[:], 1.0 / 6.0, 0.5, op0=alu.mult, op1=alu.add
                )
                nc.vector.tensor_scalar(
                    u[:], u[:], 1.0, 0.0, op0=alu.min, op1=alu.max
                )
                hs = pcv.tile([128, 32, 32], dt.bfloat16, tag="hs")
                nc.vector.tensor_tensor(
                    hs[:].rearrange("p a b -> p (a b)"), u[:], cin[:], op=alu.mult,
                )
                pad = pcv.tile([128, 34, 34], dt.bfloat16, tag="pad")
                nc.gpsimd.memset(pad[:], 0.0)
                nc.vector.tensor_copy(pad[:, 1:33, 1:33], hs[:])
                acc = pcv.tile([128, 1024], dt.bfloat16, tag="acc")
                t2 = pcv.tile([128, 1024], dt.bfloat16, tag="t2")
                for tap in range(9):
                    dy, dx = tap // 3, tap % 3
                    view = pad[:, dy:dy + 32, dx:dx + 32]
                    wsl = tapw_sb[:, ct * 9 + tap:ct * 9 + tap + 1]
                    if tap == 0:
                        nc.vector.tensor_scalar(
                            acc[:].rearrange("p (a b) -> p a b", a=32),
                            view, wsl, None, op0=alu.mult,
                        )
                    else:
                        nc.vector.tensor_scalar(
                            t2[:].rearrange("p (a b) -> p a b", a=32),
                            view, wsl, None, op0=alu.mult,
                        )
                        nc.vector.tensor_tensor(acc[:], acc[:], t2[:], op=alu.add)
                vct = pvc.tile([128, 1024], dt.bfloat16, tag="vc")
                nc.vector.tensor_tensor(vct[:], acc[:], cpb_sb[:, ct, :], op=alu.add)
                vc_tiles.append(vct)

            for b in range(BPC):
                vc_tiles = []
                for pr in range(4):
                    heads, rows, qTt, kTt, ets = emit_scores(b, pr)
                    oT_sb = {}
                    for hh in heads:
                        oT_sb[hh] = poTs.tile(
                            [D + 1, N], dt.bfloat16, tag="oTs", name=f"oTs{hh}"
                        )
                    emit_otail(b, pr, heads, rows, qTt, kTt, ets, oT_sb)
                    emit_stage(b, heads, oT_sb)
                    emit_conv(b, pr, vc_tiles)
                vc_all[b] = vc_tiles

            # ======== phase 3: proj + out per batch ========
            for b in range(BPC):
                vc_tiles = vc_all[b]
                O2v = o_nat[b].rearrange("n c -> (n c)").rearrange("(r m) -> r m", m=N)
                o2t = []
                for mt, (m0, msz) in enumerate(NTT):
                    t = po2.tile([msz, DH], dt.bfloat16, tag=f"o2t{mt}")
                    if msz == 128:
                        nc.sync.dma_start_transpose(t[:], O2v[:, m0:m0 + msz])
                    else:
                        nc.sync.dma_start(
                            t[:], O2v[:, m0:m0 + msz].rearrange("a b -> b a")
                        )
                    o2t.append(t)
                xo_sb = [
                    pxo.tile([128, 1024], dt.bfloat16, tag="xo", name=f"xo{rt}")
                    for rt in range(4)
                ]
                for rt in range(4):
                    for c, jc in enumerate((0, 512)):
                        ps = psot.tile([128, 512], dt.float32, tag="ot", name="xop")
                        for mt, (m0, msz) in enumerate(NTT):
                            nc.tensor.matmul(
                                ps[:],
                                o2t[mt][0:msz, rt * 128:(rt + 1) * 128],
                                wproj_sb[mt][0:msz, jc:jc + 512],
                                start=(mt == 0), stop=(mt == 8),
                            )
                        nc.vector.tensor_tensor(
                            xo_sb[rt][:, jc:jc + 512], ps[:],
                            vc_tiles[rt][:, jc:jc + 512], op=alu.add,
                        )

                for jt in range(8):
                    ps = psot.tile([128, 512], dt.float32, tag="ot", name="outp")
                    for rt in range(4):
                        nc.tensor.matmul(
                            ps[:],
                            xo_sb[rt][:, jt * 128:(jt + 1) * 128],
                            wout_sb[:, rt, :],
                            start=(rt == 0), stop=(rt == 3),
                        )
                    osb = posb.tile([128, DIM], dt.float32, tag="outsb")
                    nc.vector.tensor_tensor(osb[:], ps[:], obbc[:], op=alu.add)
                    nc.scalar.dma_start(out_ext[b, jt * 128:(jt + 1) * 128, :], osb[:])

    nc.compile()
    return nc


def _prep_weights(qkv_w, bn_gamma, bn_beta, bn_mean, bn_var,
                  conv_w, conv_b, proj_w, proj_b, out_w, out_b):
    s = bn_gamma / np.sqrt(bn_var + EPS)
    bias = bn_beta - bn_mean * s
    Wt = (qkv_w * s[:, None]).T.copy()
    bvec = bias.copy()
    scale = KD ** -0.5
    for hh in range(H):
        Wt[:, hh * 128:hh * 128 + KD] *= scale
        bvec[hh * 128:hh * 128 + KD] *= scale
    # scatter channels into 1032 layout (see _build head slicing):
    # cols 0:256 q tiles (4 heads/tile at 0,32,64,96), 256:512 k tiles,
    # 512:1032 v channels hh*65+j with a ones channel (zero weight, bias 1)
    Wn = np.zeros((DIM, 1032), Wt.dtype)
    bn = np.zeros(1032, bvec.dtype)
    for hh in range(H):
        qdst = (hh // 4) * 128 + (hh % 4) * 32
        kdst = 256 + (hh // 4) * 128 + (hh % 4) * 32
        vdst = 512 + hh * 65
        qsrc, ksrc, vsrc = hh * 128, hh * 128 + 32, hh * 128 + 64
        Wn[:, qdst:qdst + 32] = Wt[:, qsrc:qsrc + 32]
        bn[qdst:qdst + 32] = bvec[qsrc:qsrc + 32]
        Wn[:, kdst:kdst + 32] = Wt[:, ksrc:ksrc + 32]
        bn[kdst:kdst + 32] = bvec[ksrc:ksrc + 32]
        Wn[:, vdst:vdst + 64] = Wt[:, vsrc:vsrc + 64]
        bn[vdst:vdst + 64] = bvec[vsrc:vsrc + 64]
        bn[vdst + 64] = 1.0
    cpb_t = (conv_b.reshape(4, 128).T[:, :, None]
             + proj_b.reshape(1, 1, IMG))              # [128, 4, 1024]
    return {
        "wqkv": np.ascontiguousarray(Wn).astype(BF16),
        "bqkv": np.ascontiguousarray(bn[:512].reshape(4, 128).T).astype(np.float32),
        "bv": np.ascontiguousarray(bn[512:].reshape(1, 520)).astype(np.float32),
        "wproj": np.ascontiguousarray(proj_w.T).astype(BF16),
        "wout": np.ascontiguousarray(out_w.T).astype(BF16),
        "tapw": np.ascontiguousarray(
            conv_w[:, 0].reshape(4, 128, 9).transpose(1, 0, 2).reshape(128, 36)
        ).astype(np.float32),
        "cpb": np.ascontiguousarray(cpb_t).astype(BF16),
        "obp": out_b.reshape(1, DIM).astype(np.float32),
    }


def run(trace=False, tmpdir=None, **inputs):
    from concourse.bass_utils import run_bass_kernel_spmd

    if "nc" not in _cached:
        _cached["nc"] = _build()
    nc = _cached["nc"]

    w = _prep_weights(**{k: np.asarray(v) for k, v in inputs.items() if k != "x"})
    x = np.asarray(inputs["x"]).astype(BF16)
    in_maps = []
    for c in range(NCORES):
        m = dict(w)
        m["xs"] = np.ascontiguousarray(x[c * BPC:(c + 1) * BPC])
        in_maps.append(m)
    res = run_bass_kernel_spmd(
        nc, in_maps, core_ids=list(range(NCORES)), trace=trace, tmpdir=tmpdir
    )
    out = np.concatenate([np.asarray(r["out"]) for r in res.results], axis=0)
    return out.astype(np.float32), res.exec_time_ns


def kernel(**inputs):
    out, _ = run(trace=False, **inputs)
    return out


if __name__ == "__main__":
    print("building graph...")
    nc = _build()
    print("build OK:", len(nc.m.functions[0].allocations), "allocations")


# revision 29
# speedup vs baseline: 1.4621x; 1.0141x over previous
"""Trainium2 Bass kernel for nn_Attention_66546223284383.

Strategy: pure data-parallel over batch B=16 -> 2 batches per core x 8 cores.
Per core, per batch:
  qkvT = (BN-folded W)^T @ x^T           (h on partitions, n free)
  Attention m-dim padded 1044->1152 (zero k/v pad rows contribute
  exp(0)*0 = 0), so all 9 m-tiles are uniform 128 rows.
  q/k packed 4 heads per 128-partition tile at offsets {0,32,64,96};
  scores for 4 heads run CONCURRENTLY via PE row tiling
  (tile_position=(32i,0), K=32 each).
  n-chunk-major softmax: per (group, chunk<=512): per m-tile:
  4 row-tiled score MMs -> 4 psum banks -> exp on ACT -> 4 oT MMs
  ([v|1]^T @ exp, 65 rows: 64 o-dims + denominator) accumulating into
  4 more banks.  n-tail (1024:1044) handled in a merged pass: one
  [128, 9*20] accumulator + ONE exp per head.
  PE-transpose oT -> normalize (ACT Copy with per-partition reciprocal
  scale) -> stage o to DRAM (n, c) bf16.
  conv branch: v reflowed via DRAM to (channel, spatial), hardswish +
  9-tap depthwise conv on VectorE with per-partition tap weights.
  proj:     O2^T tiles read back via XBAR transpose DMA (handles the torch
            "raw reshape" (B,N,H,d)->(B,DH,N) as a flat re-chunk),
            xo = O2 @ proj_w^T + vc  (r on partitions, j free)
  out:      out = xo^T @ out_w^T + out_b -> (1024, 512) fp32.
All matmuls bf16 inputs with fp32 PSUM accumulation.
"""
import sys
import numpy as np

sys.path.insert(0, "/opt/trn_rl_repo")

import ml_dtypes  # noqa: E402

BF16 = ml_dtypes.bfloat16

KD, H, D, DH, DIM, IMG, S = 32, 8, 64, 512, 512, 1024, 32
N, B = 1044, 16
EPS = 1e-5
NCORES = 8
BPC = B // NCORES  # batches per core
MP = 1152          # padded attention m-dim (9 x 128)
NMT = 9            # m-tiles, all 128 rows
NTT = [(i * 128, 128) for i in range(8)] + [(1024, 20)]  # real-n tiles
CHUNKS = [(0, 512), (512, 512)]                          # big n chunks
TLO, TLW = 1024, 20                                      # n tail

_cached = {}


def _build():
    from concourse import bacc, tile
    import concourse.bass as bass
    import concourse.mybir as mybir
    from concourse.masks import make_identity

    dt = mybir.dt
    alu = mybir.AluOpType
    act_exp = mybir.ActivationFunctionType.Exp
    act_copy = mybir.ActivationFunctionType.Copy

    nc = bacc.Bacc(None, target_bir_lowering=False, debug=False)

    xs = nc.declare_dram_parameter("xs", [BPC, N, DIM], dt.bfloat16, isOutput=False)
    # wqkv cols: 0:256 = 2 q tiles (4 heads each at offsets 0/32/64/96),
    # 256:512 = 2 k tiles, 512:1032 = v channels hh*65+j (j==64 is a ones
    # channel: zero weights, bias 1 -> softmax denominator column)
    wqkv = nc.declare_dram_parameter("wqkv", [DIM, 1032], dt.bfloat16, isOutput=False)
    bqkv = nc.declare_dram_parameter("bqkv", [128, 4], dt.float32, isOutput=False)
    bv = nc.declare_dram_parameter("bv", [1, 520], dt.float32, isOutput=False)
    wproj = nc.declare_dram_parameter("wproj", [N, IMG], dt.bfloat16, isOutput=False)
    wout = nc.declare_dram_parameter("wout", [DH, DIM], dt.bfloat16, isOutput=False)
    tapw = nc.declare_dram_parameter("tapw", [128, 36], dt.float32, isOutput=False)
    cpb = nc.declare_dram_parameter("cpb", [128, 4, IMG], dt.bfloat16, isOutput=False)
    obp = nc.declare_dram_parameter("obp", [1, DIM], dt.float32, isOutput=False)
    out_ext = nc.declare_dram_parameter("out", [BPC, IMG, DIM], dt.float32, isOutput=True)

    o_nat = nc.dram_tensor("o_nat", [BPC, N, DH], dt.bfloat16)
    v_dram = nc.dram_tensor("v_dram", [BPC, H, N, D], dt.bfloat16)

    import contextlib
    with tile.TileContext(nc) as tc:
        with contextlib.ExitStack() as _st:
            ec = _st.enter_context
            pw = ec(tc.tile_pool(name="w", bufs=1))
            pxT = ec(tc.tile_pool(name="xT", bufs=5))
            pqk = ec(tc.tile_pool(name="qkvT", bufs=8))
            pvn = ec(tc.tile_pool(name="vnat", bufs=18))
            pexp = ec(tc.tile_pool(name="exp", bufs=18))
            pext = ec(tc.tile_pool(name="expt", bufs=1))
            poTs = ec(tc.tile_pool(name="oTs", bufs=4))
            psm = ec(tc.tile_pool(name="small", bufs=6))
            posb = ec(tc.tile_pool(name="osb", bufs=2))
            pcv = ec(tc.tile_pool(name="conv", bufs=1))
            pcin = ec(tc.tile_pool(name="cin", bufs=2))
            pvc = ec(tc.tile_pool(name="vc", bufs=7))
            po2 = ec(tc.tile_pool(name="o2t", bufs=1))
            pxo = ec(tc.tile_pool(name="xo", bufs=4))
            # PSUM budget (8 banks): sc 2x(128,1024)=4, ot 2x(65,512)=2,
            # fl 2x(128,512)=2 (fillers: qkv/vnat/proj/out run in PE gaps)
            pssc = ec(tc.tile_pool(name="pssc", bufs=2, space=bass.MemorySpace.PSUM))
            psfl = ec(tc.tile_pool(name="psfl", bufs=2, space=bass.MemorySpace.PSUM))
            psot = ec(tc.tile_pool(name="psot", bufs=2, space=bass.MemorySpace.PSUM))
            # ---- constants / weights ----
            id_sb = pw.tile([128, 128], dt.bfloat16, tag="id")
            make_identity(nc, id_sb[:])
            wqkv_sb = pw.tile([128, 4, 1032], dt.bfloat16, tag="wqkv")
            nc.sync.dma_start(wqkv_sb[:], wqkv[:].rearrange("(k p) h -> p k h", p=128))
            bqkv_sb = pw.tile([128, 4], dt.float32, tag="bqkv")
            nc.sync.dma_start(bqkv_sb[:], bqkv[:])
            bv_sb = pw.tile([1, 520], dt.float32, tag="bv")
            nc.sync.dma_start(bv_sb[:], bv[:])
            bvbc = pw.tile([128, 520], dt.float32, tag="bvbc")
            nc.gpsimd.partition_broadcast(bvbc[:], bv_sb[:])
            wproj_sb = []
            for mt, (m0, msz) in enumerate(NTT):
                t = pw.tile([msz, 1024], dt.bfloat16, tag=f"wproj{mt}")
                nc.sync.dma_start(t[:], wproj[m0:m0 + msz, :])
                wproj_sb.append(t)
            wout_sb = pw.tile([128, 4, DIM], dt.bfloat16, tag="wout")
            nc.sync.dma_start(wout_sb[:], wout[:].rearrange("(k p) c -> p k c", p=128))
            tapw_sb = pw.tile([128, 36], dt.float32, tag="tapw")
            nc.sync.dma_start(tapw_sb[:], tapw[:])
            cpb_sb = pw.tile([128, 4, IMG], dt.bfloat16, tag="cpb")
            nc.sync.dma_start(cpb_sb[:], cpb[:])
            ob_sb = pw.tile([1, DIM], dt.float32, tag="ob")
            nc.sync.dma_start(ob_sb[:], obp[:])
            obbc = pw.tile([128, DIM], dt.float32, tag="obbc")
            nc.gpsimd.partition_broadcast(obbc[:], ob_sb[:])

            qkvT_all, vnat_all, vc_all = {}, {}, {}

            # ======== phase 1: x^T + qk + v for BOTH batches (PE-dense) ========
            for b in range(BPC):
                xT = []
                for cb4 in range(4):
                    t = pxT.tile([128, N], dt.bfloat16, tag="xT")
                    c0 = cb4 * 128
                    nc.sync.dma_start_transpose(t[:, 0:1040], xs[b, 0:1040, c0:c0 + 128])
                    nc.sync.dma_start(
                        t[:, 1040:N], xs[b, 1040:N, c0:c0 + 128].rearrange("a b -> b a")
                    )
                    xT.append(t)

                # 4 qk tiles: t 0-1 q heads (4/tile @ 0,32,64,96), 2-3 k heads
                # k tiles padded to MP cols; pad region zeroed (never biased)
                qkvT = []
                for t4 in range(4):
                    is_k = t4 >= 2
                    t = pqk.tile([128, MP], dt.bfloat16, tag="qkvT")
                    if is_k:
                        nc.gpsimd.memset(t[:, N:MP], 0.0)
                    for (ci, cw) in ((0, 512), (512, 512), (1024, 20)):
                        ps = psfl.tile([128, 512], dt.float32, tag="fl", name="qkp")
                        for kc in range(4):
                            nc.tensor.matmul(
                                ps[:, 0:cw],
                                wqkv_sb[:, kc, t4 * 128:(t4 + 1) * 128],
                                xT[kc][:, ci:ci + cw],
                                start=(kc == 0), stop=(kc == 3),
                            )
                        nc.vector.tensor_scalar(
                            t[:, ci:ci + cw], ps[:, 0:cw],
                            bqkv_sb[:, t4:t4 + 1], None, op0=alu.add,
                        )
                    qkvT.append(t)
                qkvT_all[b] = qkvT

                # v directly in (n, 8*65) layout with ones channels; m-tile 8
                # padded with zero rows (20 real)
                vnat = []
                for mt in range(NMT):
                    m0 = mt * 128
                    mreal = 128 if mt < 8 else 20
                    t = pvn.tile([128, 520], dt.bfloat16, tag="vnat")
                    if mreal < 128:
                        nc.gpsimd.memset(t[:], 0.0)
                    for c in range(2):
                        ps = psfl.tile([128, 512], dt.float32, tag="fl", name="vnp")
                        for kc in range(4):
                            nc.tensor.matmul(
                                ps[0:mreal, 0:260],
                                xT[kc][:, m0:m0 + mreal],
                                wqkv_sb[:, kc, 512 + c * 260:512 + (c + 1) * 260],
                                start=(kc == 0), stop=(kc == 3),
                            )
                        nc.vector.tensor_tensor(
                            t[0:mreal, c * 260:(c + 1) * 260], ps[0:mreal, 0:260],
                            bvbc[0:mreal, c * 260:(c + 1) * 260], op=alu.add,
                        )
                    # stage all 8 heads' v in ONE DMA on the gpsimd queue
                    nc.gpsimd.dma_start(
                        v_dram[b].rearrange("h n d -> n h d")[m0:m0 + mreal],
                        t[0:mreal].rearrange("p (h dd) -> p h dd", h=8)[:, :, 0:D],
                    )
                    vnat.append(t)
                vnat_all[b] = vnat

            # ======== phase 2: attention, head pairs, big-exp per (h,mt) ========
            def emit_scores(b, pr):
                qkvT, vnat = qkvT_all[b], vnat_all[b]
                t4, half = pr // 2, pr % 2
                qTt, kTt = qkvT[t4], qkvT[2 + t4]
                rows = (64 * half, 64 * half + 32)
                heads = [t4 * 4 + half * 2, t4 * 4 + half * 2 + 1]
                ets = {}
                # scores + exp: one [128,1024] 2-bank tile + one exp per
                # (head, mt); the pair's score MMs run concurrently via
                # PE row tiling
                for mt in range(NMT):
                    m0 = mt * 128
                    for c, (ci, cw) in enumerate(CHUNKS):
                        sc = pssc.tile([128, 1024], dt.float32, tag="sc")
                        for i in range(2):
                            nc.tensor.matmul(
                                sc[:, i * 512:i * 512 + cw],
                                kTt[rows[i]:rows[i] + 32, m0:m0 + 128],
                                qTt[rows[i]:rows[i] + 32, ci:ci + cw],
                                start=True, stop=True, tile_position=(rows[i], 0),
                            )
                        et = pexp.tile([128, 1024], dt.bfloat16, tag="exp")
                        nc.scalar.activation(et[:], sc[:], act_exp)
                        ets[(c, mt)] = et
                return heads, rows, qTt, kTt, ets

            def emit_otail(b, pr, heads, rows, qTt, kTt, ets, oT_sb):
                vnat = vnat_all[b]
                # oT accumulation per head, both n-chunks per m-tile; et tiles
                # hold (head A | head B) halves and release after B's pass
                for i, hh in enumerate(heads):
                    ots = [
                        psot.tile([D + 1, cw], dt.float32, tag="ot", name=f"ot{c}")
                        for c, (ci, cw) in enumerate(CHUNKS)
                    ]
                    for mt in range(NMT):
                        for c, (ci, cw) in enumerate(CHUNKS):
                            nc.tensor.matmul(
                                ots[c][:],
                                vnat[mt][:, hh * 65:hh * 65 + D + 1],
                                ets[(c, mt)][:, i * 512:i * 512 + cw],
                                start=(mt == 0), stop=(mt == NMT - 1),
                            )
                    for c, (ci, cw) in enumerate(CHUNKS):
                        nc.vector.tensor_copy(oT_sb[hh][:, ci:ci + cw], ots[c][:])
                # merged n-tail pass: [128, 9*20] accumulators, 1 exp/head
                tacc = [
                    psot.tile([128, NMT * TLW], dt.float32, tag="ot", name=f"tl{i}")
                    for i in range(2)
                ]
                for mt in range(NMT):
                    m0 = mt * 128
                    for i in range(2):
                        nc.tensor.matmul(
                            tacc[i][:, mt * TLW:(mt + 1) * TLW],
                            kTt[rows[i]:rows[i] + 32, m0:m0 + 128],
                            qTt[rows[i]:rows[i] + 32, TLO:TLO + TLW],
                            start=True, stop=True, tile_position=(rows[i], 0),
                        )
                for i, hh in enumerate(heads):
                    ett = pext.tile([128, NMT * TLW], dt.bfloat16, tag="expt")
                    nc.scalar.activation(ett[:], tacc[i][:], act_exp)
                    ot_t = psot.tile([D + 1, TLW], dt.float32, tag="ot")
                    for mt in range(NMT):
                        nc.tensor.matmul(
                            ot_t[:],
                            vnat[mt][:, hh * 65:hh * 65 + D + 1],
                            ett[:, mt * TLW:(mt + 1) * TLW],
                            start=(mt == 0), stop=(mt == NMT - 1),
                        )
                    nc.vector.tensor_copy(oT_sb[hh][:, TLO:TLO + TLW], ot_t[:])

            def emit_stage(b, heads, oT_sb):
                # transpose back, normalize, stage o
                for hh in heads:
                    for nt, (n0, nsz) in enumerate(NTT):
                        tp2 = psot.tile([nsz, D + 1], dt.bfloat16, tag="ot")
                        nc.tensor.transpose(
                            tp2[:], oT_sb[hh][:, n0:n0 + nsz], id_sb[0:D + 1, 0:D + 1]
                        )
                        rcp = psm.tile([nsz, 1], dt.float32, tag="rcp")
                        nc.vector.reciprocal(rcp[:], tp2[:, D:D + 1])
                        onrm = psm.tile([nsz, D], dt.bfloat16, tag="onrm")
                        nc.vector.tensor_scalar(
                            onrm[:], tp2[:, 0:D], rcp[:], None, op0=alu.mult
                        )
                        nc.sync.dma_start(
                            o_nat[b, n0:n0 + nsz, hh * D:(hh + 1) * D], onrm[:]
                        )

            def emit_conv(b, ct, vc_tiles):
                # conv for this pair's channel-tile (bf16 DVE ops)
                cin = pcin.tile([128, 1024], dt.bfloat16, tag="cin")
                for hl in range(2):
                    src = v_dram[b, 2 * ct + hl, 0:1024, :].rearrange(
                        "(c n2) d -> c (n2 d)", c=64
                    )
                    nc.scalar.dma_start(cin[hl * 64:(hl + 1) * 64, :], src)
                u = pcv.tile([128, 1024], dt.bfloat16, tag="u")
                nc.vector.tensor_scalar(
                    u[:], cin[:], 1.0 / 6.0, 0.5, op0=alu.mult, op1=alu.add
                )
                nc.vector.tensor_scalar(
                    u[:], u[:], 1.0, 0.0, op0=alu.min, op1=alu.max
                )
                hs = pcv.tile([128, 32, 32], dt.bfloat16, tag="hs")
                nc.vector.tensor_tensor(
                    hs[:].rearrange("p a b -> p (a b)"), u[:], cin[:], op=alu.mult,
                )
                pad = pcv.tile([128, 34, 34], dt.bfloat16, tag="pad")
                nc.gpsimd.memset(pad[:], 0.0)
                nc.vector.tensor_copy(pad[:, 1:33, 1:33], hs[:])
                acc = pcv.tile([128, 1024], dt.bfloat16, tag="acc")
                t2 = pcv.tile([128, 1024], dt.bfloat16, tag="t2")
                for tap in range(9):
                    dy, dx = tap // 3, tap % 3
                    view = pad[:, dy:dy + 32, dx:dx + 32]
                    wsl = tapw_sb[:, ct * 9 + tap:ct * 9 + tap + 1]
                    if tap == 0:
                        nc.vector.tensor_scalar(
                            acc[:].rearrange("p (a b) -> p a b", a=32),
                            view, wsl, None, op0=alu.mult,
                        )
                    else:
                        nc.vector.tensor_scalar(
                            t2[:].rearrange("p (a b) -> p a b", a=32),
                            view, wsl, None, op0=alu.mult,
                        )
                        nc.vector.tensor_tensor(acc[:], acc[:], t2[:], op=alu.add)
                vct = pvc.tile([128, 1024], dt.bfloat16, tag="vc")
                nc.vector.tensor_tensor(vct[:], acc[:], cpb_sb[:, ct, :], op=alu.add)
                vc_tiles.append(vct)

            for b in range(BPC):
                vc_tiles = []
                for pr in range(4):
                    heads, rows, qTt, kTt, ets = emit_scores(b, pr)
                    oT_sb = {}
                    for hh in heads:
                        oT_sb[hh] = poTs.tile(
                            [D + 1, N], dt.bfloat16, tag="oTs", name=f"oTs{hh}"
                        )
                    emit_otail(b, pr, heads, rows, qTt, kTt, ets, oT_sb)
                    emit_stage(b, heads, oT_sb)
                    emit_conv(b, pr, vc_tiles)
                vc_all[b] = vc_tiles

            # ======== phase 3: proj + out per batch ========
            for b in range(BPC):
                vc_tiles = vc_all[b]
                O2v = o_nat[b].rearrange("n c -> (n c)").rearrange("(r m) -> r m", m=N)
                o2t = []
                for mt, (m0, msz) in enumerate(NTT):
                    t = po2.tile([msz, DH], dt.bfloat16, tag=f"o2t{mt}")
                    if msz == 128:
                        nc.sync.dma_start_transpose(t[:], O2v[:, m0:m0 + msz])
                    else:
                        nc.sync.dma_start(
                            t[:], O2v[:, m0:m0 + msz].rearrange("a b -> b a")
                        )
                    o2t.append(t)
                xo_sb = [
                    pxo.tile([128, 1024], dt.bfloat16, tag="xo", name=f"xo{rt}")
                    for rt in range(4)
                ]
                for rt in range(4):
                    for c, jc in enumerate((0, 512)):
                        ps = psfl.tile([128, 512], dt.float32, tag="fl", name="xop")
                        for mt, (m0, msz) in enumerate(NTT):
                            nc.tensor.matmul(
                                ps[:],
                                o2t[mt][0:msz, rt * 128:(rt + 1) * 128],
                                wproj_sb[mt][0:msz, jc:jc + 512],
                                start=(mt == 0), stop=(mt == 8),
                            )
                        nc.vector.tensor_tensor(
                            xo_sb[rt][:, jc:jc + 512], ps[:],
                            vc_tiles[rt][:, jc:jc + 512], op=alu.add,
                        )

                for jt in range(8):
                    ps = psfl.tile([128, 512], dt.float32, tag="fl", name="outp")
                    for rt in range(4):
                        nc.tensor.matmul(
                            ps[:],
                            xo_sb[rt][:, jt * 128:(jt + 1) * 128],
                            wout_sb[:, rt, :],
                            start=(rt == 0), stop=(rt == 3),
                        )
                    osb = posb.tile([128, DIM], dt.float32, tag="outsb")
                    nc.vector.tensor_tensor(osb[:], ps[:], obbc[:], op=alu.add)
                    nc.scalar.dma_start(out_ext[b, jt * 128:(jt + 1) * 128, :], osb[:])

    nc.compile()
    return nc


def _prep_weights(qkv_w, bn_gamma, bn_beta, bn_mean, bn_var,
                  conv_w, conv_b, proj_w, proj_b, out_w, out_b):
    s = bn_gamma / np.sqrt(bn_var + EPS)
    bias = bn_beta - bn_mean * s
    Wt = (qkv_w * s[:, None]).T.copy()
    bvec = bias.copy()
    scale = KD ** -0.5
    for hh in range(H):
        Wt[:, hh * 128:hh * 128 + KD] *= scale
        bvec[hh * 128:hh * 128 + KD] *= scale
    # scatter channels into 1032 layout (see _build head slicing):
    # cols 0:256 q tiles (4 heads/tile at 0,32,64,96), 256:512 k tiles,
    # 512:1032 v channels hh*65+j with a ones channel (zero weight, bias 1)
    Wn = np.zeros((DIM, 1032), Wt.dtype)
    bn = np.zeros(1032, bvec.dtype)
    for hh in range(H):
        qdst = (hh // 4) * 128 + (hh % 4) * 32
        kdst = 256 + (hh // 4) * 128 + (hh % 4) * 32
        vdst = 512 + hh * 65
        qsrc, ksrc, vsrc = hh * 128, hh * 128 + 32, hh * 128 + 64
        Wn[:, qdst:qdst + 32] = Wt[:, qsrc:qsrc + 32]
        bn[qdst:qdst + 32] = bvec[qsrc:qsrc + 32]
        Wn[:, kdst:kdst + 32] = Wt[:, ksrc:ksrc + 32]
        bn[kdst:kdst + 32] = bvec[ksrc:ksrc + 32]
        Wn[:, vdst:vdst + 64] = Wt[:, vsrc:vsrc + 64]
        bn[vdst:vdst + 64] = bvec[vsrc:vsrc + 64]
        bn[vdst + 64] = 1.0
    cpb_t = (conv_b.reshape(4, 128).T[:, :, None]
             + proj_b.reshape(1, 1, IMG))              # [128, 4, 1024]
    return {
        "wqkv": np.ascontiguousarray(Wn).astype(BF16),
        "bqkv": np.ascontiguousarray(bn[:512].reshape(4, 128).T).astype(np.float32),
        "bv": np.ascontiguousarray(bn[512:].reshape(1, 520)).astype(np.float32),
        "wproj": np.ascontiguousarray(proj_w.T).astype(BF16),
        "wout": np.ascontiguousarray(out_w.T).astype(BF16),
        "tapw": np.ascontiguousarray(
            conv_w[:, 0].reshape(4, 128, 9).transpose(1, 0, 2).reshape(128, 36)
        ).astype(np.float32),
        "cpb": np.ascontiguousarray(cpb_t).astype(BF16),
        "obp": out_b.reshape(1, DIM).astype(np.float32),
    }


def run(trace=False, tmpdir=None, **inputs):
    from concourse.bass_utils import run_bass_kernel_spmd

    if "nc" not in _cached:
        _cached["nc"] = _build()
    nc = _cached["nc"]

    w = _prep_weights(**{k: np.asarray(v) for k, v in inputs.items() if k != "x"})
    x = np.asarray(inputs["x"]).astype(BF16)
    in_maps = []
    for c in range(NCORES):
        m = dict(w)
        m["xs"] = np.ascontiguousarray(x[c * BPC:(c + 1) * BPC])
        in_maps.append(m)
    res = run_bass_kernel_spmd(
        nc, in_maps, core_ids=list(range(NCORES)), trace=trace, tmpdir=tmpdir
    )
    out = np.concatenate([np.asarray(r["out"]) for r in res.results], axis=0)
    return out.astype(np.float32), res.exec_time_ns


def kernel(**inputs):
    out, _ = run(trace=False, **inputs)
    return out


if __name__ == "__main__":
    print("building graph...")
    nc = _build()
    print("build OK:", len(nc.m.functions[0].allocations), "allocations")


# revision 32
# speedup vs baseline: 1.4986x; 1.0249x over previous
"""Trainium2 Bass kernel for nn_Attention_66546223284383.

Strategy: pure data-parallel over batch B=16 -> 2 batches per core x 8 cores.
Per core, per batch:
  qkvT = (BN-folded W)^T @ x^T           (h on partitions, n free)
  Attention m-dim padded 1044->1152 (zero k/v pad rows contribute
  exp(0)*0 = 0), so all 9 m-tiles are uniform 128 rows.
  q/k packed 4 heads per 128-partition tile at offsets {0,32,64,96};
  scores for 4 heads run CONCURRENTLY via PE row tiling
  (tile_position=(32i,0), K=32 each).
  n-chunk-major softmax: per (group, chunk<=512): per m-tile:
  4 row-tiled score MMs -> 4 psum banks -> exp on ACT -> 4 oT MMs
  ([v|1]^T @ exp, 65 rows: 64 o-dims + denominator) accumulating into
  4 more banks.  n-tail (1024:1044) handled in a merged pass: one
  [128, 9*20] accumulator + ONE exp per head.
  PE-transpose oT -> normalize (ACT Copy with per-partition reciprocal
  scale) -> stage o to DRAM (n, c) bf16.
  conv branch: v reflowed via DRAM to (channel, spatial), hardswish +
  9-tap depthwise conv on VectorE with per-partition tap weights.
  proj:     O2^T tiles read back via XBAR transpose DMA (handles the torch
            "raw reshape" (B,N,H,d)->(B,DH,N) as a flat re-chunk),
            xo = O2 @ proj_w^T + vc  (r on partitions, j free)
  out:      out = xo^T @ out_w^T + out_b -> (1024, 512) fp32.
All matmuls bf16 inputs with fp32 PSUM accumulation.
"""
import sys
import numpy as np

sys.path.insert(0, "/opt/trn_rl_repo")

import ml_dtypes  # noqa: E402

BF16 = ml_dtypes.bfloat16

KD, H, D, DH, DIM, IMG, S = 32, 8, 64, 512, 512, 1024, 32
N, B = 1044, 16
EPS = 1e-5
NCORES = 8
BPC = B // NCORES  # batches per core
MP = 1152          # padded attention m-dim (9 x 128)
NMT = 9            # m-tiles, all 128 rows
NTT = [(i * 128, 128) for i in range(8)] + [(1024, 20)]  # real-n tiles
CHUNKS = [(0, 512), (512, 512)]                          # big n chunks
TLO, TLW = 1024, 20                                      # n tail

_cached = {}


def _build():
    from concourse import bacc, tile
    import concourse.bass as bass
    import concourse.mybir as mybir
    from concourse.masks import make_identity

    dt = mybir.dt
    alu = mybir.AluOpType
    act_exp = mybir.ActivationFunctionType.Exp
    act_copy = mybir.ActivationFunctionType.Copy

    nc = bacc.Bacc(None, target_bir_lowering=False, debug=False)

    xs = nc.declare_dram_parameter("xs", [BPC, N, DIM], dt.bfloat16, isOutput=False)
    # wqkv cols: 0:256 = 2 q tiles (4 heads each at offsets 0/32/64/96),
    # 256:512 = 2 k tiles, 512:1032 = v channels hh*65+j (j==64 is a ones
    # channel: zero weights, bias 1 -> softmax denominator column)
    wqkv = nc.declare_dram_parameter("wqkv", [DIM, 1032], dt.bfloat16, isOutput=False)
    bqkv = nc.declare_dram_parameter("bqkv", [128, 4], dt.float32, isOutput=False)
    bv = nc.declare_dram_parameter("bv", [1, 520], dt.float32, isOutput=False)
    wproj = nc.declare_dram_parameter("wproj", [N, IMG], dt.bfloat16, isOutput=False)
    wout = nc.declare_dram_parameter("wout", [DH, DIM], dt.bfloat16, isOutput=False)
    tapw = nc.declare_dram_parameter("tapw", [128, 36], dt.float32, isOutput=False)
    cpb = nc.declare_dram_parameter("cpb", [128, 4, IMG], dt.bfloat16, isOutput=False)
    obp = nc.declare_dram_parameter("obp", [1, DIM], dt.float32, isOutput=False)
    out_ext = nc.declare_dram_parameter("out", [BPC, IMG, DIM], dt.float32, isOutput=True)

    o_nat = nc.dram_tensor("o_nat", [BPC, N, DH], dt.bfloat16)
    v_dram = nc.dram_tensor("v_dram", [BPC, H, N, D], dt.bfloat16)

    import contextlib
    with tile.TileContext(nc) as tc:
        with contextlib.ExitStack() as _st:
            ec = _st.enter_context
            pw = ec(tc.tile_pool(name="w", bufs=1))
            pxT = ec(tc.tile_pool(name="xT", bufs=5))
            pqk = ec(tc.tile_pool(name="qkvT", bufs=8))
            pvn = ec(tc.tile_pool(name="vnat", bufs=18))
            pexp = ec(tc.tile_pool(name="exp", bufs=18))
            pext = ec(tc.tile_pool(name="expt", bufs=1))
            poTs = ec(tc.tile_pool(name="oTs", bufs=4))
            psm = ec(tc.tile_pool(name="small", bufs=6))
            posb = ec(tc.tile_pool(name="osb", bufs=2))
            pcv = ec(tc.tile_pool(name="conv", bufs=1))
            pcin = ec(tc.tile_pool(name="cin", bufs=2))
            pvc = ec(tc.tile_pool(name="vc", bufs=7))
            po2 = ec(tc.tile_pool(name="o2t", bufs=1))
            pxo = ec(tc.tile_pool(name="xo", bufs=4))
            # PSUM budget (8 banks): sc 2x(128,1024)=4, ot 2x(65,512)=2,
            # fl 2x(128,512)=2 (fillers: qkv/vnat/proj/out run in PE gaps)
            pssc = ec(tc.tile_pool(name="pssc", bufs=2, space=bass.MemorySpace.PSUM))
            psfl = ec(tc.tile_pool(name="psfl", bufs=2, space=bass.MemorySpace.PSUM))
            psot = ec(tc.tile_pool(name="psot", bufs=2, space=bass.MemorySpace.PSUM))
            # ---- constants / weights ----
            id_sb = pw.tile([128, 128], dt.bfloat16, tag="id")
            make_identity(nc, id_sb[:])
            wqkv_sb = pw.tile([128, 4, 1032], dt.bfloat16, tag="wqkv")
            nc.sync.dma_start(wqkv_sb[:], wqkv[:].rearrange("(k p) h -> p k h", p=128))
            bqkv_sb = pw.tile([128, 4], dt.float32, tag="bqkv")
            nc.sync.dma_start(bqkv_sb[:], bqkv[:])
            bv_sb = pw.tile([1, 520], dt.float32, tag="bv")
            nc.sync.dma_start(bv_sb[:], bv[:])
            bvbc = pw.tile([128, 520], dt.float32, tag="bvbc")
            nc.gpsimd.partition_broadcast(bvbc[:], bv_sb[:])
            wproj_sb = []
            for mt, (m0, msz) in enumerate(NTT):
                t = pw.tile([msz, 1024], dt.bfloat16, tag=f"wproj{mt}")
                nc.sync.dma_start(t[:], wproj[m0:m0 + msz, :])
                wproj_sb.append(t)
            wout_sb = pw.tile([128, 4, DIM], dt.bfloat16, tag="wout")
            nc.sync.dma_start(wout_sb[:], wout[:].rearrange("(k p) c -> p k c", p=128))
            tapw_sb = pw.tile([128, 36], dt.float32, tag="tapw")
            nc.sync.dma_start(tapw_sb[:], tapw[:])
            cpb_sb = pw.tile([128, 4, IMG], dt.bfloat16, tag="cpb")
            nc.sync.dma_start(cpb_sb[:], cpb[:])
            ob_sb = pw.tile([1, DIM], dt.float32, tag="ob")
            nc.sync.dma_start(ob_sb[:], obp[:])
            obbc = pw.tile([128, DIM], dt.float32, tag="obbc")
            nc.gpsimd.partition_broadcast(obbc[:], ob_sb[:])

            qkvT_all, vnat_all, vc_all = {}, {}, {}

            # ======== phase 1: x^T + qk + v per batch ========
            def emit_phase1(b):
                xT = []
                for cb4 in range(4):
                    t = pxT.tile([128, N], dt.bfloat16, tag="xT")
                    c0 = cb4 * 128
                    nc.sync.dma_start_transpose(t[:, 0:1040], xs[b, 0:1040, c0:c0 + 128])
                    nc.sync.dma_start(
                        t[:, 1040:N], xs[b, 1040:N, c0:c0 + 128].rearrange("a b -> b a")
                    )
                    xT.append(t)

                # 4 qk tiles: t 0-1 q heads (4/tile @ 0,32,64,96), 2-3 k heads
                # k tiles padded to MP cols; pad region zeroed (never biased)
                qkvT = []
                for t4 in range(4):
                    is_k = t4 >= 2
                    t = pqk.tile([128, MP], dt.bfloat16, tag="qkvT")
                    if is_k:
                        nc.gpsimd.memset(t[:, N:MP], 0.0)
                    for (ci, cw) in ((0, 512), (512, 512), (1024, 20)):
                        ps = psfl.tile([128, 512], dt.float32, tag="fl", name="qkp")
                        for kc in range(4):
                            nc.tensor.matmul(
                                ps[:, 0:cw],
                                wqkv_sb[:, kc, t4 * 128:(t4 + 1) * 128],
                                xT[kc][:, ci:ci + cw],
                                start=(kc == 0), stop=(kc == 3),
                            )
                        nc.vector.tensor_scalar(
                            t[:, ci:ci + cw], ps[:, 0:cw],
                            bqkv_sb[:, t4:t4 + 1], None, op0=alu.add,
                        )
                    qkvT.append(t)
                qkvT_all[b] = qkvT

                # v directly in (n, 8*65) layout with ones channels; m-tile 8
                # padded with zero rows (20 real)
                vnat = []
                for mt in range(NMT):
                    m0 = mt * 128
                    mreal = 128 if mt < 8 else 20
                    t = pvn.tile([128, 520], dt.bfloat16, tag="vnat")
                    if mreal < 128:
                        nc.gpsimd.memset(t[:], 0.0)
                    for c in range(2):
                        ps = psfl.tile([128, 512], dt.float32, tag="fl", name="vnp")
                        for kc in range(4):
                            nc.tensor.matmul(
                                ps[0:mreal, 0:260],
                                xT[kc][:, m0:m0 + mreal],
                                wqkv_sb[:, kc, 512 + c * 260:512 + (c + 1) * 260],
                                start=(kc == 0), stop=(kc == 3),
                            )
                        nc.vector.tensor_tensor(
                            t[0:mreal, c * 260:(c + 1) * 260], ps[0:mreal, 0:260],
                            bvbc[0:mreal, c * 260:(c + 1) * 260], op=alu.add,
                        )
                    # stage all 8 heads' v in ONE DMA on the gpsimd queue
                    nc.gpsimd.dma_start(
                        v_dram[b].rearrange("h n d -> n h d")[m0:m0 + mreal],
                        t[0:mreal].rearrange("p (h dd) -> p h dd", h=8)[:, :, 0:D],
                    )
                    vnat.append(t)
                vnat_all[b] = vnat

            # ======== phase 2: attention, head pairs, big-exp per (h,mt) ========
            def emit_scores(b, pr):
                qkvT, vnat = qkvT_all[b], vnat_all[b]
                t4, half = pr // 2, pr % 2
                qTt, kTt = qkvT[t4], qkvT[2 + t4]
                rows = (64 * half, 64 * half + 32)
                heads = [t4 * 4 + half * 2, t4 * 4 + half * 2 + 1]
                ets = {}
                # scores + exp: one [128,1024] 2-bank tile + one exp per
                # (head, mt); the pair's score MMs run concurrently via
                # PE row tiling
                for mt in range(NMT):
                    m0 = mt * 128
                    for c, (ci, cw) in enumerate(CHUNKS):
                        sc = pssc.tile([128, 1024], dt.float32, tag="sc")
                        for i in range(2):
                            nc.tensor.matmul(
                                sc[:, i * 512:i * 512 + cw],
                                kTt[rows[i]:rows[i] + 32, m0:m0 + 128],
                                qTt[rows[i]:rows[i] + 32, ci:ci + cw],
                                start=True, stop=True, tile_position=(rows[i], 0),
                            )
                        et = pexp.tile([128, 1024], dt.bfloat16, tag="exp")
                        nc.scalar.activation(et[:], sc[:], act_exp)
                        ets[(c, mt)] = et
                return heads, rows, qTt, kTt, ets

            def emit_otail(b, pr, heads, rows, qTt, kTt, ets, oT_sb):
                vnat = vnat_all[b]
                # oT accumulation per head, both n-chunks per m-tile; et tiles
                # hold (head A | head B) halves and release after B's pass
                for i, hh in enumerate(heads):
                    ots = [
                        psot.tile([D + 1, cw], dt.float32, tag="ot", name=f"ot{c}")
                        for c, (ci, cw) in enumerate(CHUNKS)
                    ]
                    for mt in range(NMT):
                        for c, (ci, cw) in enumerate(CHUNKS):
                            nc.tensor.matmul(
                                ots[c][:],
                                vnat[mt][:, hh * 65:hh * 65 + D + 1],
                                ets[(c, mt)][:, i * 512:i * 512 + cw],
                                start=(mt == 0), stop=(mt == NMT - 1),
                            )
                    for c, (ci, cw) in enumerate(CHUNKS):
                        nc.vector.tensor_copy(oT_sb[hh][:, ci:ci + cw], ots[c][:])
                # merged n-tail pass: [128, 9*20] accumulators, 1 exp/head
                tacc = [
                    psot.tile([128, NMT * TLW], dt.float32, tag="ot", name=f"tl{i}")
                    for i in range(2)
                ]
                for mt in range(NMT):
                    m0 = mt * 128
                    for i in range(2):
                        nc.tensor.matmul(
                            tacc[i][:, mt * TLW:(mt + 1) * TLW],
                            kTt[rows[i]:rows[i] + 32, m0:m0 + 128],
                            qTt[rows[i]:rows[i] + 32, TLO:TLO + TLW],
                            start=True, stop=True, tile_position=(rows[i], 0),
                        )
                for i, hh in enumerate(heads):
                    ett = pext.tile([128, NMT * TLW], dt.bfloat16, tag="expt")
                    nc.scalar.activation(ett[:], tacc[i][:], act_exp)
                    ot_t = psot.tile([D + 1, TLW], dt.float32, tag="ot")
                    for mt in range(NMT):
                        nc.tensor.matmul(
                            ot_t[:],
                            vnat[mt][:, hh * 65:hh * 65 + D + 1],
                            ett[:, mt * TLW:(mt + 1) * TLW],
                            start=(mt == 0), stop=(mt == NMT - 1),
                        )
                    nc.vector.tensor_copy(oT_sb[hh][:, TLO:TLO + TLW], ot_t[:])

            def emit_stage(b, heads, oT_sb):
                # transpose back, normalize, stage o
                for hh in heads:
                    for nt, (n0, nsz) in enumerate(NTT):
                        tp2 = psot.tile([nsz, D + 1], dt.bfloat16, tag="ot")
                        nc.tensor.transpose(
                            tp2[:], oT_sb[hh][:, n0:n0 + nsz], id_sb[0:D + 1, 0:D + 1]
                        )
                        rcp = psm.tile([nsz, 1], dt.float32, tag="rcp")
                        nc.vector.reciprocal(rcp[:], tp2[:, D:D + 1])
                        onrm = psm.tile([nsz, D], dt.bfloat16, tag="onrm")
                        nc.vector.tensor_scalar(
                            onrm[:], tp2[:, 0:D], rcp[:], None, op0=alu.mult
                        )
                        nc.sync.dma_start(
                            o_nat[b, n0:n0 + nsz, hh * D:(hh + 1) * D], onrm[:]
                        )

            def emit_conv(b, ct, vc_tiles):
                # conv for this pair's channel-tile (bf16 DVE ops)
                cin = pcin.tile([128, 1024], dt.bfloat16, tag="cin")
                for hl in range(2):
                    src = v_dram[b, 2 * ct + hl, 0:1024, :].rearrange(
                        "(c n2) d -> c (n2 d)", c=64
                    )
                    nc.scalar.dma_start(cin[hl * 64:(hl + 1) * 64, :], src)
                u = pcv.tile([128, 1024], dt.bfloat16, tag="u")
                nc.vector.tensor_scalar(
                    u[:], cin[:], 1.0 / 6.0, 0.5, op0=alu.mult, op1=alu.add
                )
                nc.vector.tensor_scalar(
                    u[:], u[:], 1.0, 0.0, op0=alu.min, op1=alu.max
                )
                hs = pcv.tile([128, 32, 32], dt.bfloat16, tag="hs")
                nc.vector.tensor_tensor(
                    hs[:].rearrange("p a b -> p (a b)"), u[:], cin[:], op=alu.mult,
                )
                pad = pcv.tile([128, 34, 34], dt.bfloat16, tag="pad")
                nc.gpsimd.memset(pad[:], 0.0)
                nc.vector.tensor_copy(pad[:, 1:33, 1:33], hs[:])
                acc = pcv.tile([128, 1024], dt.bfloat16, tag="acc")
                t2 = pcv.tile([128, 1024], dt.bfloat16, tag="t2")
                for tap in range(9):
                    dy, dx = tap // 3, tap % 3
                    view = pad[:, dy:dy + 32, dx:dx + 32]
                    wsl = tapw_sb[:, ct * 9 + tap:ct * 9 + tap + 1]
                    if tap == 0:
                        nc.vector.tensor_scalar(
                            acc[:].rearrange("p (a b) -> p a b", a=32),
                            view, wsl, None, op0=alu.mult,
                        )
                    else:
                        nc.vector.tensor_scalar(
                            t2[:].rearrange("p (a b) -> p a b", a=32),
                            view, wsl, None, op0=alu.mult,
                        )
                        nc.vector.tensor_tensor(acc[:], acc[:], t2[:], op=alu.add)
                vct = pvc.tile([128, 1024], dt.bfloat16, tag="vc")
                nc.vector.tensor_tensor(vct[:], acc[:], cpb_sb[:, ct, :], op=alu.add)
                vc_tiles.append(vct)

            def emit_attn(b):
                vc_tiles = []
                for pr in range(4):
                    heads, rows, qTt, kTt, ets = emit_scores(b, pr)
                    oT_sb = {}
                    for hh in heads:
                        oT_sb[hh] = poTs.tile(
                            [D + 1, N], dt.bfloat16, tag="oTs", name=f"oTs{hh}"
                        )
                    emit_otail(b, pr, heads, rows, qTt, kTt, ets, oT_sb)
                    emit_stage(b, heads, oT_sb)
                    emit_conv(b, pr, vc_tiles)
                vc_all[b] = vc_tiles

            # ======== phase 3: proj + out per batch ========
            def emit_phase3(b):
                vc_tiles = vc_all[b]
                O2v = o_nat[b].rearrange("n c -> (n c)").rearrange("(r m) -> r m", m=N)
                o2t = []
                for mt, (m0, msz) in enumerate(NTT):
                    t = po2.tile([msz, DH], dt.bfloat16, tag=f"o2t{mt}")
                    if msz == 128:
                        nc.sync.dma_start_transpose(t[:], O2v[:, m0:m0 + msz])
                    else:
                        nc.sync.dma_start(
                            t[:], O2v[:, m0:m0 + msz].rearrange("a b -> b a")
                        )
                    o2t.append(t)
                xo_sb = [
                    pxo.tile([128, 1024], dt.bfloat16, tag="xo", name=f"xo{rt}")
                    for rt in range(4)
                ]
                for rt in range(4):
                    for c, jc in enumerate((0, 512)):
                        ps = psfl.tile([128, 512], dt.float32, tag="fl", name="xop")
                        for mt, (m0, msz) in enumerate(NTT):
                            nc.tensor.matmul(
                                ps[:],
                                o2t[mt][0:msz, rt * 128:(rt + 1) * 128],
                                wproj_sb[mt][0:msz, jc:jc + 512],
                                start=(mt == 0), stop=(mt == 8),
                            )
                        nc.vector.tensor_tensor(
                            xo_sb[rt][:, jc:jc + 512], ps[:],
                            vc_tiles[rt][:, jc:jc + 512], op=alu.add,
                        )

                for jt in range(8):
                    ps = psfl.tile([128, 512], dt.float32, tag="fl", name="outp")
                    for rt in range(4):
                        nc.tensor.matmul(
                            ps[:],
                            xo_sb[rt][:, jt * 128:(jt + 1) * 128],
                            wout_sb[:, rt, :],
                            start=(rt == 0), stop=(rt == 3),
                        )
                    osb = posb.tile([128, DIM], dt.float32, tag="outsb")
                    nc.vector.tensor_tensor(osb[:], ps[:], obbc[:], op=alu.add)
                    nc.scalar.dma_start(out_ext[b, jt * 128:(jt + 1) * 128, :], osb[:])

            # Emission order sets scheduler priority: each attention phase is
            # followed (in priority) by independent PE work that fills its
            # ACT-bound gaps: ph1(b1) fills attn(b0), ph3(b0) fills attn(b1).
            emit_phase1(0)
            emit_attn(0)
            emit_phase1(1)
            emit_attn(1)
            emit_phase3(0)
            emit_phase3(1)

    nc.compile()
    return nc


def _prep_weights(qkv_w, bn_gamma, bn_beta, bn_mean, bn_var,
                  conv_w, conv_b, proj_w, proj_b, out_w, out_b):
    s = bn_gamma / np.sqrt(bn_var + EPS)
    bias = bn_beta - bn_mean * s
    Wt = (qkv_w * s[:, None]).T.copy()
    bvec = bias.copy()
    scale = KD ** -0.5
    for hh in range(H):
        Wt[:, hh * 128:hh * 128 + KD] *= scale
        bvec[hh * 128:hh * 128 + KD] *= scale
    # scatter channels into 1032 layout (see _build head slicing):
    # cols 0:256 q tiles (4 heads/tile at 0,32,64,96), 256:512 k tiles,
    # 512:1032 v channels hh*65+j with a ones channel (zero weight, bias 1)
    Wn = np.zeros((DIM, 1032), Wt.dtype)
    bn = np.zeros(1032, bvec.dtype)
    for hh in range(H):
        qdst = (hh // 4) * 128 + (hh % 4) * 32
        kdst = 256 + (hh // 4) * 128 + (hh % 4) * 32
        vdst = 512 + hh * 65
        qsrc, ksrc, vsrc = hh * 128, hh * 128 + 32, hh * 128 + 64
        Wn[:, qdst:qdst + 32] = Wt[:, qsrc:qsrc + 32]
        bn[qdst:qdst + 32] = bvec[qsrc:qsrc + 32]
        Wn[:, kdst:kdst + 32] = Wt[:, ksrc:ksrc + 32]
        bn[kdst:kdst + 32] = bvec[ksrc:ksrc + 32]
        Wn[:, vdst:vdst + 64] = Wt[:, vsrc:vsrc + 64]
        bn[vdst:vdst + 64] = bvec[vsrc:vsrc + 64]
        bn[vdst + 64] = 1.0
    cpb_t = (conv_b.reshape(4, 128).T[:, :, None]
             + proj_b.reshape(1, 1, IMG))              # [128, 4, 1024]
    return {
        "wqkv": np.ascontiguousarray(Wn).astype(BF16),
        "bqkv": np.ascontiguousarray(bn[:512].reshape(4, 128).T).astype(np.float32),
        "bv": np.ascontiguousarray(bn[512:].reshape(1, 520)).astype(np.float32),
        "wproj": np.ascontiguousarray(proj_w.T).astype(BF16),
        "wout": np.ascontiguousarray(out_w.T).astype(BF16),
        "tapw": np.ascontiguousarray(
            conv_w[:, 0].reshape(4, 128, 9).transpose(1, 0, 2).reshape(128, 36)
        ).astype(np.float32),
        "cpb": np.ascontiguousarray(cpb_t).astype(BF16),
        "obp": out_b.reshape(1, DIM).astype(np.float32),
    }


def run(trace=False, tmpdir=None, **inputs):
    from concourse.bass_utils import run_bass_kernel_spmd

    if "nc" not in _cached:
        _cached["nc"] = _build()
    nc = _cached["nc"]

    w = _prep_weights(**{k: np.asarray(v) for k, v in inputs.items() if k != "x"})
    x = np.asarray(inputs["x"]).astype(BF16)
    in_maps = []
    for c in range(NCORES):
        m = dict(w)
        m["xs"] = np.ascontiguousarray(x[c * BPC:(c + 1) * BPC])
        in_maps.append(m)
    res = run_bass_kernel_spmd(
        nc, in_maps, core_ids=list(range(NCORES)), trace=trace, tmpdir=tmpdir
    )
    out = np.concatenate([np.asarray(r["out"]) for r in res.results], axis=0)
    return out.astype(np.float32), res.exec_time_ns


def kernel(**inputs):
    out, _ = run(trace=False, **inputs)
    return out


if __name__ == "__main__":
    print("building graph...")
    nc = _build()
    print("build OK:", len(nc.m.functions[0].allocations), "allocations")


# revision 34
# speedup vs baseline: 1.5124x; 1.0092x over previous
"""Trainium2 Bass kernel for nn_Attention_66546223284383.

Strategy: pure data-parallel over batch B=16 -> 2 batches per core x 8 cores.
Per core, per batch:
  qkvT = (BN-folded W)^T @ x^T           (h on partitions, n free)
  Attention m-dim padded 1044->1152 (zero k/v pad rows contribute
  exp(0)*0 = 0), so all 9 m-tiles are uniform 128 rows.
  q/k packed 4 heads per 128-partition tile at offsets {0,32,64,96};
  scores for 4 heads run CONCURRENTLY via PE row tiling
  (tile_position=(32i,0), K=32 each).
  n-chunk-major softmax: per (group, chunk<=512): per m-tile:
  4 row-tiled score MMs -> 4 psum banks -> exp on ACT -> 4 oT MMs
  ([v|1]^T @ exp, 65 rows: 64 o-dims + denominator) accumulating into
  4 more banks.  n-tail (1024:1044) handled in a merged pass: one
  [128, 9*20] accumulator + ONE exp per head.
  PE-transpose oT -> normalize (ACT Copy with per-partition reciprocal
  scale) -> stage o to DRAM (n, c) bf16.
  conv branch: v reflowed via DRAM to (channel, spatial), hardswish +
  9-tap depthwise conv on VectorE with per-partition tap weights.
  proj:     O2^T tiles read back via XBAR transpose DMA (handles the torch
            "raw reshape" (B,N,H,d)->(B,DH,N) as a flat re-chunk),
            xo = O2 @ proj_w^T + vc  (r on partitions, j free)
  out:      out = xo^T @ out_w^T + out_b -> (1024, 512) fp32.
All matmuls bf16 inputs with fp32 PSUM accumulation.
"""
import sys
import numpy as np

sys.path.insert(0, "/opt/trn_rl_repo")

import ml_dtypes  # noqa: E402

BF16 = ml_dtypes.bfloat16

KD, H, D, DH, DIM, IMG, S = 32, 8, 64, 512, 512, 1024, 32
N, B = 1044, 16
EPS = 1e-5
NCORES = 8
BPC = B // NCORES  # batches per core
MP = 1152          # padded attention m-dim (9 x 128)
NMT = 9            # m-tiles, all 128 rows
NTT = [(i * 128, 128) for i in range(8)] + [(1024, 20)]  # real-n tiles
CHUNKS = [(0, 512), (512, 512)]                          # big n chunks
TLO, TLW = 1024, 20                                      # n tail

_cached = {}


def _build():
    from concourse import bacc, tile
    import concourse.bass as bass
    import concourse.mybir as mybir
    from concourse.masks import make_identity

    dt = mybir.dt
    alu = mybir.AluOpType
    act_exp = mybir.ActivationFunctionType.Exp
    act_copy = mybir.ActivationFunctionType.Copy

    nc = bacc.Bacc(None, target_bir_lowering=False, debug=False)

    xs = nc.declare_dram_parameter("xs", [BPC, N, DIM], dt.bfloat16, isOutput=False)
    # wqkv cols: 0:256 = 2 q tiles (4 heads each at offsets 0/32/64/96),
    # 256:512 = 2 k tiles, 512:1032 = v channels hh*65+j (j==64 is a ones
    # channel: zero weights, bias 1 -> softmax denominator column)
    wqkv = nc.declare_dram_parameter("wqkv", [DIM, 1032], dt.bfloat16, isOutput=False)
    bqkv = nc.declare_dram_parameter("bqkv", [128, 4], dt.float32, isOutput=False)
    bv = nc.declare_dram_parameter("bv", [1, 520], dt.float32, isOutput=False)
    wproj = nc.declare_dram_parameter("wproj", [N, IMG], dt.bfloat16, isOutput=False)
    wout = nc.declare_dram_parameter("wout", [DH, DIM], dt.bfloat16, isOutput=False)
    tapw = nc.declare_dram_parameter("tapw", [128, 36], dt.float32, isOutput=False)
    cpb = nc.declare_dram_parameter("cpb", [128, 4, IMG], dt.bfloat16, isOutput=False)
    obp = nc.declare_dram_parameter("obp", [1, DIM], dt.float32, isOutput=False)
    out_ext = nc.declare_dram_parameter("out", [BPC, IMG, DIM], dt.float32, isOutput=True)

    o_nat = nc.dram_tensor("o_nat", [BPC, N, DH], dt.bfloat16)
    v_dram = nc.dram_tensor("v_dram", [BPC, H, N, D], dt.bfloat16)

    import contextlib
    with tile.TileContext(nc) as tc:
        with contextlib.ExitStack() as _st:
            ec = _st.enter_context
            pw = ec(tc.tile_pool(name="w", bufs=1))
            pxT = ec(tc.tile_pool(name="xT", bufs=5))
            pqk = ec(tc.tile_pool(name="qkvT", bufs=8))
            pvn = ec(tc.tile_pool(name="vnat", bufs=18))
            pexp = ec(tc.tile_pool(name="exp", bufs=18))
            pext = ec(tc.tile_pool(name="expt", bufs=1))
            poTs = ec(tc.tile_pool(name="oTs", bufs=4))
            psm = ec(tc.tile_pool(name="small", bufs=6))
            posb = ec(tc.tile_pool(name="osb", bufs=2))
            pcv = ec(tc.tile_pool(name="conv", bufs=1))
            pcin = ec(tc.tile_pool(name="cin", bufs=2))
            pvc = ec(tc.tile_pool(name="vc", bufs=7))
            po2 = ec(tc.tile_pool(name="o2t", bufs=1))
            pxo = ec(tc.tile_pool(name="xo", bufs=4))
            # PSUM budget (8 banks): sc 2x(128,1024)=4, ot 2x(65,512)=2,
            # fl 2x(128,512)=2 (fillers: qkv/vnat/proj/out run in PE gaps)
            pssc = ec(tc.tile_pool(name="pssc", bufs=2, space=bass.MemorySpace.PSUM))
            psfl = ec(tc.tile_pool(name="psfl", bufs=2, space=bass.MemorySpace.PSUM))
            psot = ec(tc.tile_pool(name="psot", bufs=2, space=bass.MemorySpace.PSUM))
            # ---- constants / weights ----
            id_sb = pw.tile([128, 128], dt.bfloat16, tag="id")
            make_identity(nc, id_sb[:])
            wqkv_sb = pw.tile([128, 4, 1032], dt.bfloat16, tag="wqkv")
            nc.sync.dma_start(wqkv_sb[:], wqkv[:].rearrange("(k p) h -> p k h", p=128))
            bqkv_sb = pw.tile([128, 4], dt.float32, tag="bqkv")
            nc.sync.dma_start(bqkv_sb[:], bqkv[:])
            bv_sb = pw.tile([1, 520], dt.float32, tag="bv")
            nc.sync.dma_start(bv_sb[:], bv[:])
            bvbc = pw.tile([128, 520], dt.float32, tag="bvbc")
            nc.gpsimd.partition_broadcast(bvbc[:], bv_sb[:])
            wproj_sb = []
            for mt, (m0, msz) in enumerate(NTT):
                t = pw.tile([msz, 1024], dt.bfloat16, tag=f"wproj{mt}")
                nc.sync.dma_start(t[:], wproj[m0:m0 + msz, :])
                wproj_sb.append(t)
            wout_sb = pw.tile([128, 4, DIM], dt.bfloat16, tag="wout")
            nc.sync.dma_start(wout_sb[:], wout[:].rearrange("(k p) c -> p k c", p=128))
            tapw_sb = pw.tile([128, 36], dt.float32, tag="tapw")
            nc.sync.dma_start(tapw_sb[:], tapw[:])
            cpb_sb = pw.tile([128, 4, IMG], dt.bfloat16, tag="cpb")
            nc.sync.dma_start(cpb_sb[:], cpb[:])
            ob_sb = pw.tile([1, DIM], dt.float32, tag="ob")
            nc.sync.dma_start(ob_sb[:], obp[:])
            obbc = pw.tile([128, DIM], dt.float32, tag="obbc")
            nc.gpsimd.partition_broadcast(obbc[:], ob_sb[:])

            qkvT_all, vnat_all, vc_all = {}, {}, {}

            # ======== phase 1: x^T + qk + v per batch ========
            def emit_phase1(b):
                xT = []
                qs = [nc.sync, nc.scalar]
                for cb4 in range(4):
                    t = pxT.tile([128, N], dt.bfloat16, tag="xT")
                    c0 = cb4 * 128
                    qs[cb4 % 2].dma_start_transpose(t[:, 0:1040], xs[b, 0:1040, c0:c0 + 128])
                    nc.sync.dma_start(
                        t[:, 1040:N], xs[b, 1040:N, c0:c0 + 128].rearrange("a b -> b a")
                    )
                    xT.append(t)

                # 4 qk tiles: t 0-1 q heads (4/tile @ 0,32,64,96), 2-3 k heads
                # k tiles padded to MP cols; pad region zeroed (never biased)
                qkvT = []
                for t4 in range(4):
                    is_k = t4 >= 2
                    t = pqk.tile([128, MP], dt.bfloat16, tag="qkvT")
                    if is_k:
                        nc.gpsimd.memset(t[:, N:MP], 0.0)
                    for (ci, cw) in ((0, 512), (512, 512), (1024, 20)):
                        ps = psfl.tile([128, 512], dt.float32, tag="fl", name="qkp")
                        for kc in range(4):
                            nc.tensor.matmul(
                                ps[:, 0:cw],
                                wqkv_sb[:, kc, t4 * 128:(t4 + 1) * 128],
                                xT[kc][:, ci:ci + cw],
                                start=(kc == 0), stop=(kc == 3),
                            )
                        nc.vector.tensor_scalar(
                            t[:, ci:ci + cw], ps[:, 0:cw],
                            bqkv_sb[:, t4:t4 + 1], None, op0=alu.add,
                        )
                    qkvT.append(t)
                qkvT_all[b] = qkvT

                # v directly in (n, 8*65) layout with ones channels; m-tile 8
                # padded with zero rows (20 real)
                vnat = []
                for mt in range(NMT):
                    m0 = mt * 128
                    mreal = 128 if mt < 8 else 20
                    t = pvn.tile([128, 520], dt.bfloat16, tag="vnat")
                    if mreal < 128:
                        nc.gpsimd.memset(t[:], 0.0)
                    for c in range(2):
                        ps = psfl.tile([128, 512], dt.float32, tag="fl", name="vnp")
                        for kc in range(4):
                            nc.tensor.matmul(
                                ps[0:mreal, 0:260],
                                xT[kc][:, m0:m0 + mreal],
                                wqkv_sb[:, kc, 512 + c * 260:512 + (c + 1) * 260],
                                start=(kc == 0), stop=(kc == 3),
                            )
                        nc.vector.tensor_tensor(
                            t[0:mreal, c * 260:(c + 1) * 260], ps[0:mreal, 0:260],
                            bvbc[0:mreal, c * 260:(c + 1) * 260], op=alu.add,
                        )
                    # stage all 8 heads' v in ONE DMA on the gpsimd queue
                    nc.gpsimd.dma_start(
                        v_dram[b].rearrange("h n d -> n h d")[m0:m0 + mreal],
                        t[0:mreal].rearrange("p (h dd) -> p h dd", h=8)[:, :, 0:D],
                    )
                    vnat.append(t)
                vnat_all[b] = vnat

            # ======== phase 2: attention, head pairs, big-exp per (h,mt) ========
            def emit_scores(b, pr):
                qkvT, vnat = qkvT_all[b], vnat_all[b]
                t4, half = pr // 2, pr % 2
                qTt, kTt = qkvT[t4], qkvT[2 + t4]
                rows = (64 * half, 64 * half + 32)
                heads = [t4 * 4 + half * 2, t4 * 4 + half * 2 + 1]
                ets = {}
                # scores + exp: one [128,1024] 2-bank tile + one exp per
                # (head, mt); the pair's score MMs run concurrently via
                # PE row tiling
                for mt in range(NMT):
                    m0 = mt * 128
                    for c, (ci, cw) in enumerate(CHUNKS):
                        sc = pssc.tile([128, 1024], dt.float32, tag="sc")
                        for i in range(2):
                            nc.tensor.matmul(
                                sc[:, i * 512:i * 512 + cw],
                                kTt[rows[i]:rows[i] + 32, m0:m0 + 128],
                                qTt[rows[i]:rows[i] + 32, ci:ci + cw],
                                start=True, stop=True, tile_position=(rows[i], 0),
                            )
                        et = pexp.tile([128, 1024], dt.bfloat16, tag="exp")
                        nc.scalar.activation(et[:], sc[:], act_exp)
                        ets[(c, mt)] = et
                return heads, rows, qTt, kTt, ets

            def emit_otail(b, pr, heads, rows, qTt, kTt, ets, oT_sb):
                vnat = vnat_all[b]
                # oT accumulation per head, both n-chunks per m-tile; et tiles
                # hold (head A | head B) halves and release after B's pass
                for i, hh in enumerate(heads):
                    ots = [
                        psot.tile([D + 1, cw], dt.float32, tag="ot", name=f"ot{c}")
                        for c, (ci, cw) in enumerate(CHUNKS)
                    ]
                    for mt in range(NMT):
                        for c, (ci, cw) in enumerate(CHUNKS):
                            nc.tensor.matmul(
                                ots[c][:],
                                vnat[mt][:, hh * 65:hh * 65 + D + 1],
                                ets[(c, mt)][:, i * 512:i * 512 + cw],
                                start=(mt == 0), stop=(mt == NMT - 1),
                            )
                    for c, (ci, cw) in enumerate(CHUNKS):
                        nc.vector.tensor_copy(oT_sb[hh][:, ci:ci + cw], ots[c][:])
                # merged n-tail pass: [128, 9*20] accumulators, 1 exp/head
                tacc = [
                    psot.tile([128, NMT * TLW], dt.float32, tag="ot", name=f"tl{i}")
                    for i in range(2)
                ]
                for mt in range(NMT):
                    m0 = mt * 128
                    for i in range(2):
                        nc.tensor.matmul(
                            tacc[i][:, mt * TLW:(mt + 1) * TLW],
                            kTt[rows[i]:rows[i] + 32, m0:m0 + 128],
                            qTt[rows[i]:rows[i] + 32, TLO:TLO + TLW],
                            start=True, stop=True, tile_position=(rows[i], 0),
                        )
                for i, hh in enumerate(heads):
                    ett = pext.tile([128, NMT * TLW], dt.bfloat16, tag="expt")
                    nc.scalar.activation(ett[:], tacc[i][:], act_exp)
                    ot_t = psot.tile([D + 1, TLW], dt.float32, tag="ot")
                    for mt in range(NMT):
                        nc.tensor.matmul(
                            ot_t[:],
                            vnat[mt][:, hh * 65:hh * 65 + D + 1],
                            ett[:, mt * TLW:(mt + 1) * TLW],
                            start=(mt == 0), stop=(mt == NMT - 1),
                        )
                    nc.vector.tensor_copy(oT_sb[hh][:, TLO:TLO + TLW], ot_t[:])

            def emit_stage(b, heads, oT_sb):
                # transpose back, normalize, stage o
                for hh in heads:
                    for nt, (n0, nsz) in enumerate(NTT):
                        tp2 = psot.tile([nsz, D + 1], dt.bfloat16, tag="ot")
                        nc.tensor.transpose(
                            tp2[:], oT_sb[hh][:, n0:n0 + nsz], id_sb[0:D + 1, 0:D + 1]
                        )
                        rcp = psm.tile([nsz, 1], dt.float32, tag="rcp")
                        nc.vector.reciprocal(rcp[:], tp2[:, D:D + 1])
                        onrm = psm.tile([nsz, D], dt.bfloat16, tag="onrm")
                        nc.vector.tensor_scalar(
                            onrm[:], tp2[:, 0:D], rcp[:], None, op0=alu.mult
                        )
                        nc.gpsimd.dma_start(
                            o_nat[b, n0:n0 + nsz, hh * D:(hh + 1) * D], onrm[:]
                        )

            def emit_conv(b, ct, vc_tiles):
                # conv for this pair's channel-tile (bf16 DVE ops)
                cin = pcin.tile([128, 1024], dt.bfloat16, tag="cin")
                for hl in range(2):
                    src = v_dram[b, 2 * ct + hl, 0:1024, :].rearrange(
                        "(c n2) d -> c (n2 d)", c=64
                    )
                    nc.scalar.dma_start(cin[hl * 64:(hl + 1) * 64, :], src)
                u = pcv.tile([128, 1024], dt.bfloat16, tag="u")
                nc.vector.tensor_scalar(
                    u[:], cin[:], 1.0 / 6.0, 0.5, op0=alu.mult, op1=alu.add
                )
                nc.vector.tensor_scalar(
                    u[:], u[:], 1.0, 0.0, op0=alu.min, op1=alu.max
                )
                hs = pcv.tile([128, 32, 32], dt.bfloat16, tag="hs")
                nc.vector.tensor_tensor(
                    hs[:].rearrange("p a b -> p (a b)"), u[:], cin[:], op=alu.mult,
                )
                pad = pcv.tile([128, 34, 34], dt.bfloat16, tag="pad")
                nc.gpsimd.memset(pad[:], 0.0)
                nc.vector.tensor_copy(pad[:, 1:33, 1:33], hs[:])
                acc = pcv.tile([128, 1024], dt.bfloat16, tag="acc")
                t2 = pcv.tile([128, 1024], dt.bfloat16, tag="t2")
                for tap in range(9):
                    dy, dx = tap // 3, tap % 3
                    view = pad[:, dy:dy + 32, dx:dx + 32]
                    wsl = tapw_sb[:, ct * 9 + tap:ct * 9 + tap + 1]
                    if tap == 0:
                        nc.vector.tensor_scalar(
                            acc[:].rearrange("p (a b) -> p a b", a=32),
                            view, wsl, None, op0=alu.mult,
                        )
                    else:
                        nc.vector.tensor_scalar(
                            t2[:].rearrange("p (a b) -> p a b", a=32),
                            view, wsl, None, op0=alu.mult,
                        )
                        nc.vector.tensor_tensor(acc[:], acc[:], t2[:], op=alu.add)
                vct = pvc.tile([128, 1024], dt.bfloat16, tag="vc")
                nc.vector.tensor_tensor(vct[:], acc[:], cpb_sb[:, ct, :], op=alu.add)
                vc_tiles.append(vct)

            def emit_attn(b):
                vc_tiles = []
                for pr in range(4):
                    heads, rows, qTt, kTt, ets = emit_scores(b, pr)
                    oT_sb = {}
                    for hh in heads:
                        oT_sb[hh] = poTs.tile(
                            [D + 1, N], dt.bfloat16, tag="oTs", name=f"oTs{hh}"
                        )
                    emit_otail(b, pr, heads, rows, qTt, kTt, ets, oT_sb)
                    emit_stage(b, heads, oT_sb)
                    emit_conv(b, pr, vc_tiles)
                vc_all[b] = vc_tiles

            # ======== phase 3: proj + out per batch ========
            def emit_phase3(b):
                vc_tiles = vc_all[b]
                O2v = o_nat[b].rearrange("n c -> (n c)").rearrange("(r m) -> r m", m=N)
                o2t = []
                qs3 = [nc.sync, nc.scalar]
                for mt, (m0, msz) in enumerate(NTT):
                    t = po2.tile([msz, DH], dt.bfloat16, tag=f"o2t{mt}")
                    if msz == 128:
                        qs3[mt % 2].dma_start_transpose(t[:], O2v[:, m0:m0 + msz])
                    else:
                        nc.sync.dma_start(
                            t[:], O2v[:, m0:m0 + msz].rearrange("a b -> b a")
                        )
                    o2t.append(t)
                xo_sb = [
                    pxo.tile([128, 1024], dt.bfloat16, tag="xo", name=f"xo{rt}")
                    for rt in range(4)
                ]
                for rt in range(4):
                    for c, jc in enumerate((0, 512)):
                        ps = psfl.tile([128, 512], dt.float32, tag="fl", name="xop")
                        for mt, (m0, msz) in enumerate(NTT):
                            nc.tensor.matmul(
                                ps[:],
                                o2t[mt][0:msz, rt * 128:(rt + 1) * 128],
                                wproj_sb[mt][0:msz, jc:jc + 512],
                                start=(mt == 0), stop=(mt == 8),
                            )
                        nc.vector.tensor_tensor(
                            xo_sb[rt][:, jc:jc + 512], ps[:],
                            vc_tiles[rt][:, jc:jc + 512], op=alu.add,
                        )

                for jt in range(8):
                    ps = psfl.tile([128, 512], dt.float32, tag="fl", name="outp")
                    for rt in range(4):
                        nc.tensor.matmul(
                            ps[:],
                            xo_sb[rt][:, jt * 128:(jt + 1) * 128],
                            wout_sb[:, rt, :],
                            start=(rt == 0), stop=(rt == 3),
                        )
                    osb = posb.tile([128, DIM], dt.float32, tag="outsb")
                    nc.vector.tensor_tensor(osb[:], ps[:], obbc[:], op=alu.add)
                    nc.scalar.dma_start(out_ext[b, jt * 128:(jt + 1) * 128, :], osb[:])

            # Emission order sets scheduler priority: each attention phase is
            # followed (in priority) by independent PE work that fills its
            # ACT-bound gaps: ph1(b1) fills attn(b0), ph3(b0) fills attn(b1).
            emit_phase1(0)
            emit_attn(0)
            emit_phase1(1)
            emit_attn(1)
            emit_phase3(0)
            emit_phase3(1)

    nc.compile()
    return nc


def _prep_weights(qkv_w, bn_gamma, bn_beta, bn_mean, bn_var,
                  conv_w, conv_b, proj_w, proj_b, out_w, out_b):
    s = bn_gamma / np.sqrt(bn_var + EPS)
    bias = bn_beta - bn_mean * s
    Wt = (qkv_w * s[:, None]).T.copy()
    bvec = bias.copy()
    scale = KD ** -0.5
    for hh in range(H):
        Wt[:, hh * 128:hh * 128 + KD] *= scale
        bvec[hh * 128:hh * 128 + KD] *= scale
    # scatter channels into 1032 layout (see _build head slicing):
    # cols 0:256 q tiles (4 heads/tile at 0,32,64,96), 256:512 k tiles,
    # 512:1032 v channels hh*65+j with a ones channel (zero weight, bias 1)
    Wn = np.zeros((DIM, 1032), Wt.dtype)
    bn = np.zeros(1032, bvec.dtype)
    for hh in range(H):
        qdst = (hh // 4) * 128 + (hh % 4) * 32
        kdst = 256 + (hh // 4) * 128 + (hh % 4) * 32
        vdst = 512 + hh * 65
        qsrc, ksrc, vsrc = hh * 128, hh * 128 + 32, hh * 128 + 64
        Wn[:, qdst:qdst + 32] = Wt[:, qsrc:qsrc + 32]
        bn[qdst:qdst + 32] = bvec[qsrc:qsrc + 32]
        Wn[:, kdst:kdst + 32] = Wt[:, ksrc:ksrc + 32]
        bn[kdst:kdst + 32] = bvec[ksrc:ksrc + 32]
        Wn[:, vdst:vdst + 64] = Wt[:, vsrc:vsrc + 64]
        bn[vdst:vdst + 64] = bvec[vsrc:vsrc + 64]
        bn[vdst + 64] = 1.0
    cpb_t = (conv_b.reshape(4, 128).T[:, :, None]
             + proj_b.reshape(1, 1, IMG))              # [128, 4, 1024]
    return {
        "wqkv": np.ascontiguousarray(Wn).astype(BF16),
        "bqkv": np.ascontiguousarray(bn[:512].reshape(4, 128).T).astype(np.float32),
        "bv": np.ascontiguousarray(bn[512:].reshape(1, 520)).astype(np.float32),
        "wproj": np.ascontiguousarray(proj_w.T).astype(BF16),
        "wout": np.ascontiguousarray(out_w.T).astype(BF16),
        "tapw": np.ascontiguousarray(
            conv_w[:, 0].reshape(4, 128, 9).transpose(1, 0, 2).reshape(128, 36)
        ).astype(np.float32),
        "cpb": np.ascontiguousarray(cpb_t).astype(BF16),
        "obp": out_b.reshape(1, DIM).astype(np.float32),
    }


def run(trace=False, tmpdir=None, **inputs):
    from concourse.bass_utils import run_bass_kernel_spmd

    if "nc" not in _cached:
        _cached["nc"] = _build()
    nc = _cached["nc"]

    w = _prep_weights(**{k: np.asarray(v) for k, v in inputs.items() if k != "x"})
    x = np.asarray(inputs["x"]).astype(BF16)
    in_maps = []
    for c in range(NCORES):
        m = dict(w)
        m["xs"] = np.ascontiguousarray(x[c * BPC:(c + 1) * BPC])
        in_maps.append(m)
    res = run_bass_kernel_spmd(
        nc, in_maps, core_ids=list(range(NCORES)), trace=trace, tmpdir=tmpdir
    )
    out = np.concatenate([np.asarray(r["out"]) for r in res.results], axis=0)
    return out.astype(np.float32), res.exec_time_ns


def kernel(**inputs):
    out, _ = run(trace=False, **inputs)
    return out


if __name__ == "__main__":
    print("building graph...")
    nc = _build()
    print("build OK:", len(nc.m.functions[0].allocations), "allocations")


# revision 36
# speedup vs baseline: 1.5307x; 1.0121x over previous
"""Trainium2 Bass kernel for nn_Attention_66546223284383.

Strategy: pure data-parallel over batch B=16 -> 2 batches per core x 8 cores.
Per core, per batch:
  qkvT = (BN-folded W)^T @ x^T           (h on partitions, n free)
  Attention m-dim padded 1044->1152 (zero k/v pad rows contribute
  exp(0)*0 = 0), so all 9 m-tiles are uniform 128 rows.
  q/k packed 4 heads per 128-partition tile at offsets {0,32,64,96};
  scores for 4 heads run CONCURRENTLY via PE row tiling
  (tile_position=(32i,0), K=32 each).
  n-chunk-major softmax: per (group, chunk<=512): per m-tile:
  4 row-tiled score MMs -> 4 psum banks -> exp on ACT -> 4 oT MMs
  ([v|1]^T @ exp, 65 rows: 64 o-dims + denominator) accumulating into
  4 more banks.  n-tail (1024:1044) handled in a merged pass: one
  [128, 9*20] accumulator + ONE exp per head.
  PE-transpose oT -> normalize (ACT Copy with per-partition reciprocal
  scale) -> stage o to DRAM (n, c) bf16.
  conv branch: v reflowed via DRAM to (channel, spatial), hardswish +
  9-tap depthwise conv on VectorE with per-partition tap weights.
  proj:     O2^T tiles read back via XBAR transpose DMA (handles the torch
            "raw reshape" (B,N,H,d)->(B,DH,N) as a flat re-chunk),
            xo = O2 @ proj_w^T + vc  (r on partitions, j free)
  out:      out = xo^T @ out_w^T + out_b -> (1024, 512) fp32.
All matmuls bf16 inputs with fp32 PSUM accumulation.
"""
import sys
import numpy as np

sys.path.insert(0, "/opt/trn_rl_repo")

import ml_dtypes  # noqa: E402

BF16 = ml_dtypes.bfloat16

KD, H, D, DH, DIM, IMG, S = 32, 8, 64, 512, 512, 1024, 32
N, B = 1044, 16
EPS = 1e-5
NCORES = 8
BPC = B // NCORES  # batches per core
MP = 1152          # padded attention m-dim (9 x 128)
NMT = 9            # m-tiles, all 128 rows
NTT = [(i * 128, 128) for i in range(8)] + [(1024, 20)]  # real-n tiles
CHUNKS = [(0, 512), (512, 512)]                          # big n chunks
TLO, TLW = 1024, 20                                      # n tail

_cached = {}


def _build():
    from concourse import bacc, tile
    import concourse.bass as bass
    import concourse.mybir as mybir
    from concourse.masks import make_identity

    dt = mybir.dt
    alu = mybir.AluOpType
    act_exp = mybir.ActivationFunctionType.Exp
    act_copy = mybir.ActivationFunctionType.Copy

    nc = bacc.Bacc(None, target_bir_lowering=False, debug=False)

    xs = nc.declare_dram_parameter("xs", [BPC, N, DIM], dt.bfloat16, isOutput=False)
    # wqkv cols: 0:256 = 2 q tiles (4 heads each at offsets 0/32/64/96),
    # 256:512 = 2 k tiles, 512:1032 = v channels hh*65+j (j==64 is a ones
    # channel: zero weights, bias 1 -> softmax denominator column)
    wqkv = nc.declare_dram_parameter("wqkv", [DIM, 1032], dt.bfloat16, isOutput=False)
    bqkv = nc.declare_dram_parameter("bqkv", [128, 4], dt.float32, isOutput=False)
    bv = nc.declare_dram_parameter("bv", [1, 520], dt.float32, isOutput=False)
    wproj = nc.declare_dram_parameter("wproj", [N, IMG], dt.bfloat16, isOutput=False)
    wout = nc.declare_dram_parameter("wout", [DH, DIM], dt.bfloat16, isOutput=False)
    tapw = nc.declare_dram_parameter("tapw", [128, 36], dt.float32, isOutput=False)
    cpb = nc.declare_dram_parameter("cpb", [128, 4, IMG], dt.bfloat16, isOutput=False)
    obp = nc.declare_dram_parameter("obp", [1, DIM], dt.float32, isOutput=False)
    out_ext = nc.declare_dram_parameter("out", [BPC, IMG, DIM], dt.float32, isOutput=True)

    o_nat = nc.dram_tensor("o_nat", [BPC, N, DH], dt.bfloat16)
    v_dram = nc.dram_tensor("v_dram", [BPC, H, N, D], dt.bfloat16)

    import contextlib
    with tile.TileContext(nc) as tc:
        with contextlib.ExitStack() as _st:
            ec = _st.enter_context
            pw = ec(tc.tile_pool(name="w", bufs=1))
            pxT = ec(tc.tile_pool(name="xT", bufs=5))
            pqk = ec(tc.tile_pool(name="qkvT", bufs=8))
            pvn = ec(tc.tile_pool(name="vnat", bufs=18))
            pexp = ec(tc.tile_pool(name="exp", bufs=18))
            pext = ec(tc.tile_pool(name="expt", bufs=1))
            poTs = ec(tc.tile_pool(name="oTs", bufs=4))
            psm = ec(tc.tile_pool(name="small", bufs=6))
            posb = ec(tc.tile_pool(name="osb", bufs=2))
            pcv = ec(tc.tile_pool(name="conv", bufs=1))
            pcin = ec(tc.tile_pool(name="cin", bufs=2))
            pvc = ec(tc.tile_pool(name="vc", bufs=7))
            po2 = ec(tc.tile_pool(name="o2t", bufs=1))
            pxo = ec(tc.tile_pool(name="xo", bufs=4))
            # PSUM budget (8 banks): sc 2x(128,1024)=4, ot 2x(65,512)=2,
            # fl 2x(128,512)=2 (fillers: qkv/vnat/proj/out run in PE gaps)
            pssc = ec(tc.tile_pool(name="pssc", bufs=2, space=bass.MemorySpace.PSUM))
            psfl = ec(tc.tile_pool(name="psfl", bufs=2, space=bass.MemorySpace.PSUM))
            psot = ec(tc.tile_pool(name="psot", bufs=2, space=bass.MemorySpace.PSUM))
            # ---- constants / weights ----
            id_sb = pw.tile([128, 128], dt.bfloat16, tag="id")
            make_identity(nc, id_sb[:])
            wqkv_sb = pw.tile([128, 4, 1032], dt.bfloat16, tag="wqkv")
            nc.sync.dma_start(wqkv_sb[:], wqkv[:].rearrange("(k p) h -> p k h", p=128))
            bqkv_sb = pw.tile([128, 4], dt.float32, tag="bqkv")
            nc.sync.dma_start(bqkv_sb[:], bqkv[:])
            bv_sb = pw.tile([1, 520], dt.float32, tag="bv")
            nc.sync.dma_start(bv_sb[:], bv[:])
            bvbc = pw.tile([128, 520], dt.float32, tag="bvbc")
            nc.gpsimd.partition_broadcast(bvbc[:], bv_sb[:])
            wproj_sb = []
            for mt, (m0, msz) in enumerate(NTT):
                t = pw.tile([msz, 1024], dt.bfloat16, tag=f"wproj{mt}")
                nc.sync.dma_start(t[:], wproj[m0:m0 + msz, :])
                wproj_sb.append(t)
            wout_sb = pw.tile([128, 4, DIM], dt.bfloat16, tag="wout")
            nc.sync.dma_start(wout_sb[:], wout[:].rearrange("(k p) c -> p k c", p=128))
            tapw_sb = pw.tile([128, 36], dt.float32, tag="tapw")
            nc.sync.dma_start(tapw_sb[:], tapw[:])
            cpb_sb = pw.tile([128, 4, IMG], dt.bfloat16, tag="cpb")
            nc.sync.dma_start(cpb_sb[:], cpb[:])
            ob_sb = pw.tile([1, DIM], dt.float32, tag="ob")
            nc.sync.dma_start(ob_sb[:], obp[:])
            obbc = pw.tile([128, DIM], dt.float32, tag="obbc")
            nc.gpsimd.partition_broadcast(obbc[:], ob_sb[:])

            qkvT_all, vnat_all, vc_all = {}, {}, {}

            # ======== phase 1: x^T + qk + v per batch ========
            def emit_phase1(b):
                xT = []
                qs = [nc.sync, nc.scalar]
                for cb4 in range(4):
                    t = pxT.tile([128, N], dt.bfloat16, tag="xT")
                    c0 = cb4 * 128
                    qs[cb4 % 2].dma_start_transpose(t[:, 0:1040], xs[b, 0:1040, c0:c0 + 128])
                    nc.sync.dma_start(
                        t[:, 1040:N], xs[b, 1040:N, c0:c0 + 128].rearrange("a b -> b a")
                    )
                    xT.append(t)

                # 4 qk tiles: t 0-1 q heads (4/tile @ 0,32,64,96), 2-3 k heads
                # k tiles padded to MP cols; pad region zeroed (never biased)
                qkvT = []
                for t4 in range(4):
                    is_k = t4 >= 2
                    t = pqk.tile([128, MP], dt.bfloat16, tag="qkvT")
                    if is_k:
                        nc.gpsimd.memset(t[:, N:MP], 0.0)
                    for (ci, cw) in ((0, 512), (512, 512), (1024, 20)):
                        ps = psfl.tile([128, 512], dt.float32, tag="fl", name="qkp")
                        for kc in range(4):
                            nc.tensor.matmul(
                                ps[:, 0:cw],
                                wqkv_sb[:, kc, t4 * 128:(t4 + 1) * 128],
                                xT[kc][:, ci:ci + cw],
                                start=(kc == 0), stop=(kc == 3),
                            )
                        nc.vector.tensor_scalar(
                            t[:, ci:ci + cw], ps[:, 0:cw],
                            bqkv_sb[:, t4:t4 + 1], None, op0=alu.add,
                        )
                    qkvT.append(t)
                qkvT_all[b] = qkvT

                # v directly in (n, 8*65) layout with ones channels; m-tile 8
                # padded with zero rows (20 real)
                vnat = []
                for mt in range(NMT):
                    m0 = mt * 128
                    mreal = 128 if mt < 8 else 20
                    t = pvn.tile([128, 520], dt.bfloat16, tag="vnat")
                    if mreal < 128:
                        nc.gpsimd.memset(t[:], 0.0)
                    for c in range(2):
                        ps = psfl.tile([128, 512], dt.float32, tag="fl", name="vnp")
                        for kc in range(4):
                            nc.tensor.matmul(
                                ps[0:mreal, 0:260],
                                xT[kc][:, m0:m0 + mreal],
                                wqkv_sb[:, kc, 512 + c * 260:512 + (c + 1) * 260],
                                start=(kc == 0), stop=(kc == 3),
                            )
                        nc.vector.tensor_tensor(
                            t[0:mreal, c * 260:(c + 1) * 260], ps[0:mreal, 0:260],
                            bvbc[0:mreal, c * 260:(c + 1) * 260], op=alu.add,
                        )
                    # stage all 8 heads' v in ONE DMA on the gpsimd queue
                    nc.gpsimd.dma_start(
                        v_dram[b].rearrange("h n d -> n h d")[m0:m0 + mreal],
                        t[0:mreal].rearrange("p (h dd) -> p h dd", h=8)[:, :, 0:D],
                    )
                    vnat.append(t)
                vnat_all[b] = vnat

            # ======== phase 2: attention, head pairs, big-exp per (h,mt) ========
            def emit_scores(b, pr):
                qkvT, vnat = qkvT_all[b], vnat_all[b]
                t4, half = pr // 2, pr % 2
                qTt, kTt = qkvT[t4], qkvT[2 + t4]
                rows = (64 * half, 64 * half + 32)
                heads = [t4 * 4 + half * 2, t4 * 4 + half * 2 + 1]
                ets = {}
                # scores + exp: one [128,1024] 2-bank tile + one exp per
                # (head, mt); the pair's score MMs run concurrently via
                # PE row tiling
                for mt in range(NMT):
                    m0 = mt * 128
                    for c, (ci, cw) in enumerate(CHUNKS):
                        sc = pssc.tile([128, 1024], dt.float32, tag="sc")
                        for i in range(2):
                            nc.tensor.matmul(
                                sc[:, i * 512:i * 512 + cw],
                                kTt[rows[i]:rows[i] + 32, m0:m0 + 128],
                                qTt[rows[i]:rows[i] + 32, ci:ci + cw],
                                start=True, stop=True, tile_position=(rows[i], 0),
                            )
                        et = pexp.tile([128, 1024], dt.bfloat16, tag="exp")
                        nc.scalar.activation(et[:], sc[:], act_exp)
                        ets[(c, mt)] = et
                return heads, rows, qTt, kTt, ets

            def emit_otail(b, pr, heads, rows, qTt, kTt, ets, oT_sb):
                vnat = vnat_all[b]
                # oT accumulation per head, both n-chunks per m-tile; et tiles
                # hold (head A | head B) halves and release after B's pass
                for i, hh in enumerate(heads):
                    ots = [
                        psot.tile([D + 1, cw], dt.float32, tag="ot", name=f"ot{c}")
                        for c, (ci, cw) in enumerate(CHUNKS)
                    ]
                    for mt in range(NMT):
                        for c, (ci, cw) in enumerate(CHUNKS):
                            nc.tensor.matmul(
                                ots[c][:],
                                vnat[mt][:, hh * 65:hh * 65 + D + 1],
                                ets[(c, mt)][:, i * 512:i * 512 + cw],
                                start=(mt == 0), stop=(mt == NMT - 1),
                            )
                    for c, (ci, cw) in enumerate(CHUNKS):
                        nc.vector.tensor_copy(oT_sb[hh][:, ci:ci + cw], ots[c][:])
                # merged n-tail pass: [128, 9*20] accumulators, 1 exp/head
                tacc = [
                    psot.tile([128, NMT * TLW], dt.float32, tag="ot", name=f"tl{i}")
                    for i in range(2)
                ]
                for mt in range(NMT):
                    m0 = mt * 128
                    for i in range(2):
                        nc.tensor.matmul(
                            tacc[i][:, mt * TLW:(mt + 1) * TLW],
                            kTt[rows[i]:rows[i] + 32, m0:m0 + 128],
                            qTt[rows[i]:rows[i] + 32, TLO:TLO + TLW],
                            start=True, stop=True, tile_position=(rows[i], 0),
                        )
                for i, hh in enumerate(heads):
                    ett = pext.tile([128, NMT * TLW], dt.bfloat16, tag="expt")
                    nc.scalar.activation(ett[:], tacc[i][:], act_exp)
                    ot_t = psot.tile([D + 1, TLW], dt.float32, tag="ot")
                    for mt in range(NMT):
                        nc.tensor.matmul(
                            ot_t[:],
                            vnat[mt][:, hh * 65:hh * 65 + D + 1],
                            ett[:, mt * TLW:(mt + 1) * TLW],
                            start=(mt == 0), stop=(mt == NMT - 1),
                        )
                    nc.vector.tensor_copy(oT_sb[hh][:, TLO:TLO + TLW], ot_t[:])

            def emit_stage(b, heads, oT_sb):
                # transpose back, normalize into a shared pair tile, stage o
                # with one DMA per (pair, n-tile), alternating queues
                for nt, (n0, nsz) in enumerate(NTT):
                    onrm = psm.tile([nsz, 2 * D], dt.bfloat16, tag="onrm")
                    for i, hh in enumerate(heads):
                        tp2 = psot.tile([nsz, D + 1], dt.bfloat16, tag="ot")
                        nc.tensor.transpose(
                            tp2[:], oT_sb[hh][:, n0:n0 + nsz], id_sb[0:D + 1, 0:D + 1]
                        )
                        rcp = psm.tile([nsz, 1], dt.float32, tag="rcp")
                        nc.vector.reciprocal(rcp[:], tp2[:, D:D + 1])
                        nc.vector.tensor_scalar(
                            onrm[:, i * D:(i + 1) * D], tp2[:, 0:D], rcp[:], None,
                            op0=alu.mult,
                        )
                    nc.sync.dma_start(
                        o_nat[b, n0:n0 + nsz, heads[0] * D:(heads[0] + 2) * D], onrm[:]
                    )

            def emit_conv(b, ct, vc_tiles):
                # conv for this pair's channel-tile (bf16 DVE ops)
                cin = pcin.tile([128, 1024], dt.bfloat16, tag="cin")
                for hl in range(2):
                    src = v_dram[b, 2 * ct + hl, 0:1024, :].rearrange(
                        "(c n2) d -> c (n2 d)", c=64
                    )
                    nc.scalar.dma_start(cin[hl * 64:(hl + 1) * 64, :], src)
                u = pcv.tile([128, 1024], dt.bfloat16, tag="u")
                nc.vector.tensor_scalar(
                    u[:], cin[:], 1.0 / 6.0, 0.5, op0=alu.mult, op1=alu.add
                )
                nc.vector.tensor_scalar(
                    u[:], u[:], 1.0, 0.0, op0=alu.min, op1=alu.max
                )
                hs = pcv.tile([128, 32, 32], dt.bfloat16, tag="hs")
                nc.vector.tensor_tensor(
                    hs[:].rearrange("p a b -> p (a b)"), u[:], cin[:], op=alu.mult,
                )
                pad = pcv.tile([128, 34, 34], dt.bfloat16, tag="pad")
                nc.gpsimd.memset(pad[:], 0.0)
                nc.vector.tensor_copy(pad[:, 1:33, 1:33], hs[:])
                acc = pcv.tile([128, 1024], dt.bfloat16, tag="acc")
                t2 = pcv.tile([128, 1024], dt.bfloat16, tag="t2")
                for tap in range(9):
                    dy, dx = tap // 3, tap % 3
                    view = pad[:, dy:dy + 32, dx:dx + 32]
                    wsl = tapw_sb[:, ct * 9 + tap:ct * 9 + tap + 1]
                    if tap == 0:
                        nc.vector.tensor_scalar(
                            acc[:].rearrange("p (a b) -> p a b", a=32),
                            view, wsl, None, op0=alu.mult,
                        )
                    else:
                        nc.vector.tensor_scalar(
                            t2[:].rearrange("p (a b) -> p a b", a=32),
                            view, wsl, None, op0=alu.mult,
                        )
                        nc.vector.tensor_tensor(acc[:], acc[:], t2[:], op=alu.add)
                vct = pvc.tile([128, 1024], dt.bfloat16, tag="vc")
                nc.vector.tensor_tensor(vct[:], acc[:], cpb_sb[:, ct, :], op=alu.add)
                vc_tiles.append(vct)

            def emit_attn(b):
                vc_tiles = []
                for pr in range(4):
                    heads, rows, qTt, kTt, ets = emit_scores(b, pr)
                    oT_sb = {}
                    for hh in heads:
                        oT_sb[hh] = poTs.tile(
                            [D + 1, N], dt.bfloat16, tag="oTs", name=f"oTs{hh}"
                        )
                    emit_otail(b, pr, heads, rows, qTt, kTt, ets, oT_sb)
                    emit_stage(b, heads, oT_sb)
                    emit_conv(b, pr, vc_tiles)
                vc_all[b] = vc_tiles

            # ======== phase 3: proj + out per batch ========
            def emit_phase3(b):
                vc_tiles = vc_all[b]
                O2v = o_nat[b].rearrange("n c -> (n c)").rearrange("(r m) -> r m", m=N)
                o2t = []
                qs3 = [nc.sync, nc.scalar]
                for mt, (m0, msz) in enumerate(NTT):
                    t = po2.tile([msz, DH], dt.bfloat16, tag=f"o2t{mt}")
                    if msz == 128:
                        qs3[mt % 2].dma_start_transpose(t[:], O2v[:, m0:m0 + msz])
                    else:
                        nc.sync.dma_start(
                            t[:], O2v[:, m0:m0 + msz].rearrange("a b -> b a")
                        )
                    o2t.append(t)
                xo_sb = [
                    pxo.tile([128, 1024], dt.bfloat16, tag="xo", name=f"xo{rt}")
                    for rt in range(4)
                ]
                for rt in range(4):
                    for c, jc in enumerate((0, 512)):
                        ps = psfl.tile([128, 512], dt.float32, tag="fl", name="xop")
                        for mt, (m0, msz) in enumerate(NTT):
                            nc.tensor.matmul(
                                ps[:],
                                o2t[mt][0:msz, rt * 128:(rt + 1) * 128],
                                wproj_sb[mt][0:msz, jc:jc + 512],
                                start=(mt == 0), stop=(mt == 8),
                            )
                        nc.vector.tensor_tensor(
                            xo_sb[rt][:, jc:jc + 512], ps[:],
                            vc_tiles[rt][:, jc:jc + 512], op=alu.add,
                        )

                for jt in range(8):
                    ps = psfl.tile([128, 512], dt.float32, tag="fl", name="outp")
                    for rt in range(4):
                        nc.tensor.matmul(
                            ps[:],
                            xo_sb[rt][:, jt * 128:(jt + 1) * 128],
                            wout_sb[:, rt, :],
                            start=(rt == 0), stop=(rt == 3),
                        )
                    osb = posb.tile([128, DIM], dt.float32, tag="outsb")
                    nc.vector.tensor_tensor(osb[:], ps[:], obbc[:], op=alu.add)
                    nc.scalar.dma_start(out_ext[b, jt * 128:(jt + 1) * 128, :], osb[:])

            # Emission order sets scheduler priority: each attention phase is
            # followed (in priority) by independent PE work that fills its
            # ACT-bound gaps: ph1(b1) fills attn(b0), ph3(b0) fills attn(b1).
            emit_phase1(0)
            emit_attn(0)
            emit_phase1(1)
            emit_attn(1)
            emit_phase3(0)
            emit_phase3(1)

    nc.compile()
    return nc


def _prep_weights(qkv_w, bn_gamma, bn_beta, bn_mean, bn_var,
                  conv_w, conv_b, proj_w, proj_b, out_w, out_b):
    s = bn_gamma / np.sqrt(bn_var + EPS)
    bias = bn_beta - bn_mean * s
    Wt = (qkv_w * s[:, None]).T.copy()
    bvec = bias.copy()
    scale = KD ** -0.5
    for hh in range(H):
        Wt[:, hh * 128:hh * 128 + KD] *= scale
        bvec[hh * 128:hh * 128 + KD] *= scale
    # scatter channels into 1032 layout (see _build head slicing):
    # cols 0:256 q tiles (4 heads/tile at 0,32,64,96), 256:512 k tiles,
    # 512:1032 v channels hh*65+j with a ones channel (zero weight, bias 1)
    Wn = np.zeros((DIM, 1032), Wt.dtype)
    bn = np.zeros(1032, bvec.dtype)
    for hh in range(H):
        qdst = (hh // 4) * 128 + (hh % 4) * 32
        kdst = 256 + (hh // 4) * 128 + (hh % 4) * 32
        vdst = 512 + hh * 65
        qsrc, ksrc, vsrc = hh * 128, hh * 128 + 32, hh * 128 + 64
        Wn[:, qdst:qdst + 32] = Wt[:, qsrc:qsrc + 32]
        bn[qdst:qdst + 32] = bvec[qsrc:qsrc + 32]
        Wn[:, kdst:kdst + 32] = Wt[:, ksrc:ksrc + 32]
        bn[kdst:kdst + 32] = bvec[ksrc:ksrc + 32]
        Wn[:, vdst:vdst + 64] = Wt[:, vsrc:vsrc + 64]
        bn[vdst:vdst + 64] = bvec[vsrc:vsrc + 64]
        bn[vdst + 64] = 1.0
    cpb_t = (conv_b.reshape(4, 128).T[:, :, None]
             + proj_b.reshape(1, 1, IMG))              # [128, 4, 1024]
    return {
        "wqkv": np.ascontiguousarray(Wn).astype(BF16),
        "bqkv": np.ascontiguousarray(bn[:512].reshape(4, 128).T).astype(np.float32),
        "bv": np.ascontiguousarray(bn[512:].reshape(1, 520)).astype(np.float32),
        "wproj": np.ascontiguousarray(proj_w.T).astype(BF16),
        "wout": np.ascontiguousarray(out_w.T).astype(BF16),
        "tapw": np.ascontiguousarray(
            conv_w[:, 0].reshape(4, 128, 9).transpose(1, 0, 2).reshape(128, 36)
        ).astype(np.float32),
        "cpb": np.ascontiguousarray(cpb_t).astype(BF16),
        "obp": out_b.reshape(1, DIM).astype(np.float32),
    }


def run(trace=False, tmpdir=None, **inputs):
    from concourse.bass_utils import run_bass_kernel_spmd

    if "nc" not in _cached:
        _cached["nc"] = _build()
    nc = _cached["nc"]

    w = _prep_weights(**{k: np.asarray(v) for k, v in inputs.items() if k != "x"})
    x = np.asarray(inputs["x"]).astype(BF16)
    in_maps = []
    for c in range(NCORES):
        m = dict(w)
        m["xs"] = np.ascontiguousarray(x[c * BPC:(c + 1) * BPC])
        in_maps.append(m)
    res = run_bass_kernel_spmd(
        nc, in_maps, core_ids=list(range(NCORES)), trace=trace, tmpdir=tmpdir
    )
    out = np.concatenate([np.asarray(r["out"]) for r in res.results], axis=0)
    return out.astype(np.float32), res.exec_time_ns


def kernel(**inputs):
    out, _ = run(trace=False, **inputs)
    return out


if __name__ == "__main__":
    print("building graph...")
    nc = _build()
    print("build OK:", len(nc.m.functions[0].allocations), "allocations")


# revision 40
# speedup vs baseline: 1.5876x; 1.0371x over previous
"""Trainium2 Bass kernel for nn_Attention_66546223284383.

Strategy: pure data-parallel over batch B=16 -> 2 batches per core x 8 cores.
Per core, per batch:
  qkvT = (BN-folded W)^T @ x^T           (h on partitions, n free)
  Attention m-dim padded 1044->1152 (zero k/v pad rows contribute
  exp(0)*0 = 0), so all 9 m-tiles are uniform 128 rows.
  q/k packed 4 heads per 128-partition tile at offsets {0,32,64,96};
  scores for 4 heads run CONCURRENTLY via PE row tiling
  (tile_position=(32i,0), K=32 each).
  n-chunk-major softmax: per (group, chunk<=512): per m-tile:
  4 row-tiled score MMs -> 4 psum banks -> exp on ACT -> 4 oT MMs
  ([v|1]^T @ exp, 65 rows: 64 o-dims + denominator) accumulating into
  4 more banks.  n-tail (1024:1044) handled in a merged pass: one
  [128, 9*20] accumulator + ONE exp per head.
  PE-transpose oT -> normalize (ACT Copy with per-partition reciprocal
  scale) -> stage o to DRAM (n, c) bf16.
  conv branch: v reflowed via DRAM to (channel, spatial), hardswish +
  9-tap depthwise conv on VectorE with per-partition tap weights.
  proj:     O2^T tiles read back via XBAR transpose DMA (handles the torch
            "raw reshape" (B,N,H,d)->(B,DH,N) as a flat re-chunk),
            xo = O2 @ proj_w^T + vc  (r on partitions, j free)
  out:      out = xo^T @ out_w^T + out_b -> (1024, 512) fp32.
All matmuls bf16 inputs with fp32 PSUM accumulation.
"""
import sys
import numpy as np

sys.path.insert(0, "/opt/trn_rl_repo")

import ml_dtypes  # noqa: E402

BF16 = ml_dtypes.bfloat16

KD, H, D, DH, DIM, IMG, S = 32, 8, 64, 512, 512, 1024, 32
N, B = 1044, 16
EPS = 1e-5
NCORES = 8
BPC = B // NCORES  # batches per core
MP = 1152          # padded attention m-dim (9 x 128)
NMT = 9            # m-tiles, all 128 rows
NTT = [(i * 128, 128) for i in range(8)] + [(1024, 20)]  # real-n tiles
CHUNKS = [(0, 512), (512, 512)]                          # big n chunks
TLO, TLW = 1024, 20                                      # n tail

_cached = {}


def _build():
    from concourse import bacc, tile
    import concourse.bass as bass
    import concourse.mybir as mybir
    from concourse.masks import make_identity

    dt = mybir.dt
    alu = mybir.AluOpType
    act_exp = mybir.ActivationFunctionType.Exp
    act_copy = mybir.ActivationFunctionType.Copy

    nc = bacc.Bacc(None, target_bir_lowering=False, debug=False)

    xs = nc.declare_dram_parameter("xs", [BPC, N, DIM], dt.bfloat16, isOutput=False)
    # wqkv cols: 0:256 = 2 q tiles (4 heads each at offsets 0/32/64/96),
    # 256:512 = 2 k tiles, 512:1032 = v channels hh*65+j (j==64 is a ones
    # channel: zero weights, bias 1 -> softmax denominator column)
    wqkv = nc.declare_dram_parameter("wqkv", [DIM, 1032], dt.bfloat16, isOutput=False)
    bqkv = nc.declare_dram_parameter("bqkv", [128, 4], dt.float32, isOutput=False)
    bv = nc.declare_dram_parameter("bv", [1, 520], dt.float32, isOutput=False)
    wproj = nc.declare_dram_parameter("wproj", [N, IMG], dt.bfloat16, isOutput=False)
    wout = nc.declare_dram_parameter("wout", [DH, DIM], dt.bfloat16, isOutput=False)
    wdg = nc.declare_dram_parameter("wdg", [128, 36, 128], dt.bfloat16, isOutput=False)
    cpb = nc.declare_dram_parameter("cpb", [128, 4, IMG], dt.bfloat16, isOutput=False)
    obp = nc.declare_dram_parameter("obp", [1, DIM], dt.float32, isOutput=False)
    out_ext = nc.declare_dram_parameter("out", [BPC, IMG, DIM], dt.float32, isOutput=True)

    o_nat = nc.dram_tensor("o_nat", [BPC, N, DH], dt.bfloat16)
    v_dram = nc.dram_tensor("v_dram", [BPC, H, N, D], dt.bfloat16)

    import contextlib
    with tile.TileContext(nc) as tc:
        with contextlib.ExitStack() as _st:
            ec = _st.enter_context
            pw = ec(tc.tile_pool(name="w", bufs=1))
            pxT = ec(tc.tile_pool(name="xT", bufs=5))
            pqk = ec(tc.tile_pool(name="qkvT", bufs=8))
            pvn = ec(tc.tile_pool(name="vnat", bufs=18))
            pexp = ec(tc.tile_pool(name="exp", bufs=17))
            pext = ec(tc.tile_pool(name="expt", bufs=1))
            poTs = ec(tc.tile_pool(name="oTs", bufs=4))
            psm = ec(tc.tile_pool(name="small", bufs=6))
            posb = ec(tc.tile_pool(name="osb", bufs=2))
            pcv = ec(tc.tile_pool(name="conv", bufs=1))
            pcin = ec(tc.tile_pool(name="cin", bufs=2))
            pvc = ec(tc.tile_pool(name="vc", bufs=8))
            po2 = ec(tc.tile_pool(name="o2t", bufs=1))
            pxo = ec(tc.tile_pool(name="xo", bufs=4))
            # PSUM budget (8 banks): sc 2x(128,1024)=4, ot 2x(65,512)=2,
            # fl 2x(128,512)=2 (fillers: qkv/vnat/proj/out run in PE gaps)
            pssc = ec(tc.tile_pool(name="pssc", bufs=2, space=bass.MemorySpace.PSUM))
            psfl = ec(tc.tile_pool(name="psfl", bufs=2, space=bass.MemorySpace.PSUM))
            psot = ec(tc.tile_pool(name="psot", bufs=2, space=bass.MemorySpace.PSUM))
            # ---- constants / weights ----
            id_sb = pw.tile([128, 128], dt.bfloat16, tag="id")
            make_identity(nc, id_sb[:])
            wqkv_sb = pw.tile([128, 4, 1032], dt.bfloat16, tag="wqkv")
            nc.sync.dma_start(wqkv_sb[:], wqkv[:].rearrange("(k p) h -> p k h", p=128))
            bqkv_sb = pw.tile([128, 4], dt.float32, tag="bqkv")
            nc.sync.dma_start(bqkv_sb[:], bqkv[:])
            bv_sb = pw.tile([1, 520], dt.float32, tag="bv")
            nc.sync.dma_start(bv_sb[:], bv[:])
            bvbc = pw.tile([128, 520], dt.float32, tag="bvbc")
            nc.gpsimd.partition_broadcast(bvbc[:], bv_sb[:])
            wproj_sb = []
            for mt, (m0, msz) in enumerate(NTT):
                t = pw.tile([msz, 1024], dt.bfloat16, tag=f"wproj{mt}")
                nc.sync.dma_start(t[:], wproj[m0:m0 + msz, :])
                wproj_sb.append(t)
            wout_sb = pw.tile([128, 4, DIM], dt.bfloat16, tag="wout")
            nc.sync.dma_start(wout_sb[:], wout[:].rearrange("(k p) c -> p k c", p=128))
            wdg_sb = pw.tile([128, 36, 128], dt.bfloat16, tag="wdg")
            nc.sync.dma_start(wdg_sb[:], wdg[:])
            cpb_sb = pw.tile([128, 4, IMG], dt.bfloat16, tag="cpb")
            nc.sync.dma_start(cpb_sb[:], cpb[:])
            ob_sb = pw.tile([1, DIM], dt.float32, tag="ob")
            nc.sync.dma_start(ob_sb[:], obp[:])
            obbc = pw.tile([128, DIM], dt.float32, tag="obbc")
            nc.gpsimd.partition_broadcast(obbc[:], ob_sb[:])

            qkvT_all, vnat_all, vc_all = {}, {}, {}

            # ======== phase 1: x^T + qk + v per batch ========
            def emit_phase1(b):
                xT = []
                qs = [nc.sync, nc.scalar]
                for cb4 in range(4):
                    t = pxT.tile([128, N], dt.bfloat16, tag="xT")
                    c0 = cb4 * 128
                    qs[cb4 % 2].dma_start_transpose(t[:, 0:1040], xs[b, 0:1040, c0:c0 + 128])
                    nc.sync.dma_start(
                        t[:, 1040:N], xs[b, 1040:N, c0:c0 + 128].rearrange("a b -> b a")
                    )
                    xT.append(t)

                # 4 qk tiles: t 0-1 q heads (4/tile @ 0,32,64,96), 2-3 k heads
                # k tiles padded to MP cols; pad region zeroed (never biased)
                qkvT = []
                for t4 in range(4):
                    is_k = t4 >= 2
                    t = pqk.tile([128, MP], dt.bfloat16, tag="qkvT")
                    if is_k:
                        nc.gpsimd.memset(t[:, N:MP], 0.0)
                    for (ci, cw) in ((0, 512), (512, 512), (1024, 20)):
                        ps = psfl.tile([128, 512], dt.float32, tag="fl", name="qkp")
                        for kc in range(4):
                            nc.tensor.matmul(
                                ps[:, 0:cw],
                                wqkv_sb[:, kc, t4 * 128:(t4 + 1) * 128],
                                xT[kc][:, ci:ci + cw],
                                start=(kc == 0), stop=(kc == 3),
                            )
                        nc.vector.tensor_scalar(
                            t[:, ci:ci + cw], ps[:, 0:cw],
                            bqkv_sb[:, t4:t4 + 1], None, op0=alu.add,
                        )
                    qkvT.append(t)
                qkvT_all[b] = qkvT

                # v directly in (n, 8*65) layout with ones channels; m-tile 8
                # padded with zero rows (20 real)
                vnat = []
                for mt in range(NMT):
                    m0 = mt * 128
                    mreal = 128 if mt < 8 else 20
                    t = pvn.tile([128, 520], dt.bfloat16, tag="vnat")
                    if mreal < 128:
                        nc.gpsimd.memset(t[:], 0.0)
                    for c in range(2):
                        ps = psfl.tile([128, 512], dt.float32, tag="fl", name="vnp")
                        for kc in range(4):
                            nc.tensor.matmul(
                                ps[0:mreal, 0:260],
                                xT[kc][:, m0:m0 + mreal],
                                wqkv_sb[:, kc, 512 + c * 260:512 + (c + 1) * 260],
                                start=(kc == 0), stop=(kc == 3),
                            )
                        nc.vector.tensor_tensor(
                            t[0:mreal, c * 260:(c + 1) * 260], ps[0:mreal, 0:260],
                            bvbc[0:mreal, c * 260:(c + 1) * 260], op=alu.add,
                        )
                    # stage all 8 heads' v in ONE DMA on the gpsimd queue
                    nc.gpsimd.dma_start(
                        v_dram[b].rearrange("h n d -> n h d")[m0:m0 + mreal],
                        t[0:mreal].rearrange("p (h dd) -> p h dd", h=8)[:, :, 0:D],
                    )
                    vnat.append(t)
                vnat_all[b] = vnat

            # ======== phase 2: attention, head pairs, big-exp per (h,mt) ========
            def emit_scores(b, pr):
                qkvT, vnat = qkvT_all[b], vnat_all[b]
                t4, half = pr // 2, pr % 2
                qTt, kTt = qkvT[t4], qkvT[2 + t4]
                rows = (64 * half, 64 * half + 32)
                heads = [t4 * 4 + half * 2, t4 * 4 + half * 2 + 1]
                ets = {}
                # scores + exp: one [128,1024] 2-bank tile + one exp per
                # (head, mt); the pair's score MMs run concurrently via
                # PE row tiling
                for mt in range(NMT):
                    m0 = mt * 128
                    for c, (ci, cw) in enumerate(CHUNKS):
                        sc = pssc.tile([128, 1024], dt.float32, tag="sc")
                        for i in range(2):
                            nc.tensor.matmul(
                                sc[:, i * 512:i * 512 + cw],
                                kTt[rows[i]:rows[i] + 32, m0:m0 + 128],
                                qTt[rows[i]:rows[i] + 32, ci:ci + cw],
                                start=True, stop=True, tile_position=(rows[i], 0),
                            )
                        et = pexp.tile([128, 1024], dt.bfloat16, tag="exp")
                        nc.scalar.activation(et[:], sc[:], act_exp)
                        ets[(c, mt)] = et
                return heads, rows, qTt, kTt, ets

            def emit_otail(b, pr, heads, rows, qTt, kTt, ets, oT_sb):
                vnat = vnat_all[b]
                # oT accumulation per head, both n-chunks per m-tile; et tiles
                # hold (head A | head B) halves and release after B's pass
                for i, hh in enumerate(heads):
                    ots = [
                        psot.tile([D + 1, cw], dt.float32, tag="ot", name=f"ot{c}")
                        for c, (ci, cw) in enumerate(CHUNKS)
                    ]
                    for mt in range(NMT):
                        for c, (ci, cw) in enumerate(CHUNKS):
                            nc.tensor.matmul(
                                ots[c][:],
                                vnat[mt][:, hh * 65:hh * 65 + D + 1],
                                ets[(c, mt)][:, i * 512:i * 512 + cw],
                                start=(mt == 0), stop=(mt == NMT - 1),
                            )
                    for c, (ci, cw) in enumerate(CHUNKS):
                        nc.vector.tensor_copy(oT_sb[hh][:, ci:ci + cw], ots[c][:])
                # merged n-tail pass: [128, 9*20] accumulators, 1 exp/head
                tacc = [
                    psot.tile([128, NMT * TLW], dt.float32, tag="ot", name=f"tl{i}")
                    for i in range(2)
                ]
                for mt in range(NMT):
                    m0 = mt * 128
                    for i in range(2):
                        nc.tensor.matmul(
                            tacc[i][:, mt * TLW:(mt + 1) * TLW],
                            kTt[rows[i]:rows[i] + 32, m0:m0 + 128],
                            qTt[rows[i]:rows[i] + 32, TLO:TLO + TLW],
                            start=True, stop=True, tile_position=(rows[i], 0),
                        )
                for i, hh in enumerate(heads):
                    ett = pext.tile([128, NMT * TLW], dt.bfloat16, tag="expt")
                    nc.scalar.activation(ett[:], tacc[i][:], act_exp)
                    ot_t = psot.tile([D + 1, TLW], dt.float32, tag="ot")
                    for mt in range(NMT):
                        nc.tensor.matmul(
                            ot_t[:],
                            vnat[mt][:, hh * 65:hh * 65 + D + 1],
                            ett[:, mt * TLW:(mt + 1) * TLW],
                            start=(mt == 0), stop=(mt == NMT - 1),
                        )
                    nc.vector.tensor_copy(oT_sb[hh][:, TLO:TLO + TLW], ot_t[:])

            def emit_stage(b, heads, oT_sb):
                # transpose back, normalize into a shared pair tile, stage o
                # with one DMA per (pair, n-tile), alternating queues
                for nt, (n0, nsz) in enumerate(NTT):
                    onrm = psm.tile([nsz, 2 * D], dt.bfloat16, tag="onrm")
                    for i, hh in enumerate(heads):
                        tp2 = psot.tile([nsz, D + 1], dt.bfloat16, tag="ot")
                        nc.tensor.transpose(
                            tp2[:], oT_sb[hh][:, n0:n0 + nsz], id_sb[0:D + 1, 0:D + 1]
                        )
                        rcp = psm.tile([nsz, 1], dt.float32, tag="rcp")
                        nc.vector.reciprocal(rcp[:], tp2[:, D:D + 1])
                        nc.vector.tensor_scalar(
                            onrm[:, i * D:(i + 1) * D], tp2[:, 0:D], rcp[:], None,
                            op0=alu.mult,
                        )
                    nc.sync.dma_start(
                        o_nat[b, n0:n0 + nsz, heads[0] * D:(heads[0] + 2) * D], onrm[:]
                    )

            def emit_conv(b, ct, vc_tiles):
                # conv for this pair's channel-tile (bf16 DVE ops)
                cin = pcin.tile([128, 1024], dt.bfloat16, tag="cin")
                for hl in range(2):
                    src = v_dram[b, 2 * ct + hl, 0:1024, :].rearrange(
                        "(c n2) d -> c (n2 d)", c=64
                    )
                    nc.scalar.dma_start(cin[hl * 64:(hl + 1) * 64, :], src)
                u = pcv.tile([128, 1024], dt.bfloat16, tag="u")
                nc.vector.tensor_scalar(
                    u[:], cin[:], 1.0 / 6.0, 0.5, op0=alu.mult, op1=alu.add
                )
                nc.vector.tensor_scalar(
                    u[:], u[:], 1.0, 0.0, op0=alu.min, op1=alu.max
                )
                hs = pcv.tile([128, 32, 32], dt.bfloat16, tag="hs")
                nc.vector.tensor_tensor(
                    hs[:].rearrange("p a b -> p (a b)"), u[:], cin[:], op=alu.mult,
                )
                pad = pcv.tile([128, 34, 34], dt.bfloat16, tag="pad")
                nc.gpsimd.memset(pad[:], 0.0)
                nc.vector.tensor_copy(pad[:, 1:33, 1:33], hs[:])
                # 9-tap depthwise conv as diagonal-weight matmuls on the PE
                # (PE-filler food during exp-bound attention)
                vct = pvc.tile([128, 1024], dt.bfloat16, tag="vc")
                for jc in (0, 512):
                    ps = psfl.tile([128, 512], dt.float32, tag="fl", name="cvp")
                    for tap in range(9):
                        dy, dx = tap // 3, tap % 3
                        view = pad[:, dy + jc // 32:dy + jc // 32 + 16, dx:dx + 32]
                        nc.tensor.matmul(
                            ps[:],
                            wdg_sb[:, ct * 9 + tap, :],
                            view,
                            start=(tap == 0), stop=(tap == 8),
                        )
                    nc.vector.tensor_tensor(
                        vct[:, jc:jc + 512], ps[:],
                        cpb_sb[:, ct, jc:jc + 512], op=alu.add,
                    )
                vc_tiles.append(vct)

            def emit_attn(b):
                vc_tiles = []
                for pr in range(4):
                    heads, rows, qTt, kTt, ets = emit_scores(b, pr)
                    oT_sb = {}
                    for hh in heads:
                        oT_sb[hh] = poTs.tile(
                            [D + 1, N], dt.bfloat16, tag="oTs", name=f"oTs{hh}"
                        )
                    emit_otail(b, pr, heads, rows, qTt, kTt, ets, oT_sb)
                    emit_stage(b, heads, oT_sb)
                    emit_conv(b, pr, vc_tiles)
                vc_all[b] = vc_tiles

            # ======== phase 3: proj + out per batch ========
            def emit_phase3(b):
                vc_tiles = vc_all[b]
                O2v = o_nat[b].rearrange("n c -> (n c)").rearrange("(r m) -> r m", m=N)
                o2t = []
                qs3 = [nc.sync, nc.scalar]
                for mt, (m0, msz) in enumerate(NTT):
                    t = po2.tile([msz, DH], dt.bfloat16, tag=f"o2t{mt}")
                    if msz == 128:
                        qs3[mt % 2].dma_start_transpose(t[:], O2v[:, m0:m0 + msz])
                    else:
                        nc.sync.dma_start(
                            t[:], O2v[:, m0:m0 + msz].rearrange("a b -> b a")
                        )
                    o2t.append(t)
                xo_sb = [
                    pxo.tile([128, 1024], dt.bfloat16, tag="xo", name=f"xo{rt}")
                    for rt in range(4)
                ]
                for rt in range(4):
                    for c, jc in enumerate((0, 512)):
                        ps = psfl.tile([128, 512], dt.float32, tag="fl", name="xop")
                        for mt, (m0, msz) in enumerate(NTT):
                            nc.tensor.matmul(
                                ps[:],
                                o2t[mt][0:msz, rt * 128:(rt + 1) * 128],
                                wproj_sb[mt][0:msz, jc:jc + 512],
                                start=(mt == 0), stop=(mt == 8),
                            )
                        nc.vector.tensor_tensor(
                            xo_sb[rt][:, jc:jc + 512], ps[:],
                            vc_tiles[rt][:, jc:jc + 512], op=alu.add,
                        )

                for jt in range(8):
                    ps = psfl.tile([128, 512], dt.float32, tag="fl", name="outp")
                    for rt in range(4):
                        nc.tensor.matmul(
                            ps[:],
                            xo_sb[rt][:, jt * 128:(jt + 1) * 128],
                            wout_sb[:, rt, :],
                            start=(rt == 0), stop=(rt == 3),
                        )
                    osb = posb.tile([128, DIM], dt.float32, tag="outsb")
                    nc.vector.tensor_tensor(osb[:], ps[:], obbc[:], op=alu.add)
                    nc.scalar.dma_start(out_ext[b, jt * 128:(jt + 1) * 128, :], osb[:])

            # Emission order sets scheduler priority: each attention phase is
            # followed (in priority) by independent PE work that fills its
            # ACT-bound gaps: ph1(b1) fills attn(b0), ph3(b0) fills attn(b1).
            emit_phase1(0)
            emit_attn(0)
            emit_phase1(1)
            emit_attn(1)
            emit_phase3(0)
            emit_phase3(1)

    nc.compile()
    return nc


def _diag_taps(conv_w):
    # [128 part, 4*9, 128 cols]: diag(w[ct*128+p, tap]) at [p, ct*9+tap, p]
    tw = conv_w[:, 0].reshape(4, 128, 9)            # [ct, p, tap]
    out = np.zeros((128, 36, 128), np.float32)
    idx = np.arange(128)
    for ct in range(4):
        for tap in range(9):
            out[idx, ct * 9 + tap, idx] = tw[ct, :, tap]
    return np.ascontiguousarray(out).astype(BF16)


def _prep_weights(qkv_w, bn_gamma, bn_beta, bn_mean, bn_var,
                  conv_w, conv_b, proj_w, proj_b, out_w, out_b):
    s = bn_gamma / np.sqrt(bn_var + EPS)
    bias = bn_beta - bn_mean * s
    Wt = (qkv_w * s[:, None]).T.copy()
    bvec = bias.copy()
    scale = KD ** -0.5
    for hh in range(H):
        Wt[:, hh * 128:hh * 128 + KD] *= scale
        bvec[hh * 128:hh * 128 + KD] *= scale
    # scatter channels into 1032 layout (see _build head slicing):
    # cols 0:256 q tiles (4 heads/tile at 0,32,64,96), 256:512 k tiles,
    # 512:1032 v channels hh*65+j with a ones channel (zero weight, bias 1)
    Wn = np.zeros((DIM, 1032), Wt.dtype)
    bn = np.zeros(1032, bvec.dtype)
    for hh in range(H):
        qdst = (hh // 4) * 128 + (hh % 4) * 32
        kdst = 256 + (hh // 4) * 128 + (hh % 4) * 32
        vdst = 512 + hh * 65
        qsrc, ksrc, vsrc = hh * 128, hh * 128 + 32, hh * 128 + 64
        Wn[:, qdst:qdst + 32] = Wt[:, qsrc:qsrc + 32]
        bn[qdst:qdst + 32] = bvec[qsrc:qsrc + 32]
        Wn[:, kdst:kdst + 32] = Wt[:, ksrc:ksrc + 32]
        bn[kdst:kdst + 32] = bvec[ksrc:ksrc + 32]
        Wn[:, vdst:vdst + 64] = Wt[:, vsrc:vsrc + 64]
        bn[vdst:vdst + 64] = bvec[vsrc:vsrc + 64]
        bn[vdst + 64] = 1.0
    cpb_t = (conv_b.reshape(4, 128).T[:, :, None]
             + proj_b.reshape(1, 1, IMG))              # [128, 4, 1024]
    return {
        "wqkv": np.ascontiguousarray(Wn).astype(BF16),
        "bqkv": np.ascontiguousarray(bn[:512].reshape(4, 128).T).astype(np.float32),
        "bv": np.ascontiguousarray(bn[512:].reshape(1, 520)).astype(np.float32),
        "wproj": np.ascontiguousarray(proj_w.T).astype(BF16),
        "wout": np.ascontiguousarray(out_w.T).astype(BF16),
        "wdg": _diag_taps(conv_w),
        "cpb": np.ascontiguousarray(cpb_t).astype(BF16),
        "obp": out_b.reshape(1, DIM).astype(np.float32),
    }


def run(trace=False, tmpdir=None, **inputs):
    from concourse.bass_utils import run_bass_kernel_spmd

    if "nc" not in _cached:
        _cached["nc"] = _build()
    nc = _cached["nc"]

    w = _prep_weights(**{k: np.asarray(v) for k, v in inputs.items() if k != "x"})
    x = np.asarray(inputs["x"]).astype(BF16)
    in_maps = []
    for c in range(NCORES):
        m = dict(w)
        m["xs"] = np.ascontiguousarray(x[c * BPC:(c + 1) * BPC])
        in_maps.append(m)
    res = run_bass_kernel_spmd(
        nc, in_maps, core_ids=list(range(NCORES)), trace=trace, tmpdir=tmpdir
    )
    out = np.concatenate([np.asarray(r["out"]) for r in res.results], axis=0)
    return out.astype(np.float32), res.exec_time_ns


def kernel(**inputs):
    out, _ = run(trace=False, **inputs)
    return out


if __name__ == "__main__":
    print("building graph...")
    nc = _build()
    print("build OK:", len(nc.m.functions[0].allocations), "allocations")
